# revision 8
# baseline (speedup 1.0000x reference)
"""Trainium2 Bass kernel for a dense transformer block (B=2,S=2048,D=1024,H=16,DFF=4096).

Sharding across 8 NeuronCores:
  core c: batch b=c//4, group rank r=c%4, replica groups [[0,1,2,3],[4,5,6,7]].
  - LN1 computed on own 512-token strip; hT AllGather'ed within the 4-core group.
  - Attention: head-parallel (4 heads/core, full causal sequence).
  - AllToAll redistributes attention output from head-sharded to token-sharded.
  - out_proj, LN2, FFN: token-sharded (512 tokens/core), full weights.
Matmuls run in float32r (TF32-like, full PE rate); the residual spine stays fp32.
"""
import sys

sys.path.insert(0, "/opt/trn_rl_repo")

import numpy as np

import concourse.bass as bass
import concourse.mybir as mybir
import concourse.tile as tile
from concourse import bacc
from concourse.bass_utils import run_bass_kernel_spmd
from concourse.masks import make_identity

AF = mybir.ActivationFunctionType
ALU = mybir.AluOpType
F32 = mybir.dt.float32
F32R = mybir.dt.float32r

B, S, D, H = 2, 2048, 1024, 16
DH = D // H          # 64
DFF = 4 * D          # 4096
EPS = 1e-5
NC = 8               # cores
G = 4                # cores per group (per batch)
TS = S // G          # 512 tokens per core
HC = H // G          # 4 heads per core
CC = HC * DH         # 256 head-columns per core
P = 128
KD = D // P          # 8 k-tiles over D
KF = DFF // P        # 32 k-tiles over DFF
NT = TS // P         # 4 token tiles per strip
GROUPS = [[0, 1, 2, 3], [4, 5, 6, 7]]

_CACHE = {}


def build():
    nc = bacc.Bacc(None)

    io = {}
    io["x_d"] = nc.declare_dram_parameter("x", [TS, D], F32, isOutput=False)
    io["ln1g_d"] = nc.declare_dram_parameter("ln1_g", [D], F32, isOutput=False)
    io["ln1b_d"] = nc.declare_dram_parameter("ln1_b", [D], F32, isOutput=False)
    io["wq_d"] = nc.declare_dram_parameter("Wq", [D, CC], F32R, isOutput=False)
    io["wk_d"] = nc.declare_dram_parameter("Wk", [D, CC], F32R, isOutput=False)
    io["wv_d"] = nc.declare_dram_parameter("Wv", [D, CC], F32R, isOutput=False)
    io["bq_d"] = nc.declare_dram_parameter("bq", [CC], F32R, isOutput=False)
    io["bk_d"] = nc.declare_dram_parameter("bk", [CC], F32R, isOutput=False)
    io["bv_d"] = nc.declare_dram_parameter("bv", [CC], F32R, isOutput=False)
    io["wo_d"] = nc.declare_dram_parameter("Wo", [D, D], F32R, isOutput=False)
    io["bo_d"] = nc.declare_dram_parameter("bo", [D], F32R, isOutput=False)
    io["ln2g_d"] = nc.declare_dram_parameter("ln2_g", [D], F32, isOutput=False)
    io["ln2b_d"] = nc.declare_dram_parameter("ln2_b", [D], F32, isOutput=False)
    io["w1_d"] = nc.declare_dram_parameter("W1", [D, DFF], F32R, isOutput=False)
    io["b1_d"] = nc.declare_dram_parameter("b1", [DFF], F32R, isOutput=False)
    io["w2_d"] = nc.declare_dram_parameter("W2", [DFF, D], F32R, isOutput=False)
    io["b2_d"] = nc.declare_dram_parameter("b2", [D], F32R, isOutput=False)
    io["y_d"] = nc.declare_dram_parameter("y", [TS, D], F32, isOutput=True)

    io["ag1_in"] = nc.dram_tensor("ag1_in", [D, TS], F32R)
    io["ag1_out"] = nc.dram_tensor("ag1_out", [G, D, TS], F32R)
    io["a2a_in"] = nc.dram_tensor("a2a_in", [CC, S], F32R)
    io["a2a_out"] = nc.dram_tensor("a2a_out", [G, CC, S], F32R)
    io["coff_d"] = nc.declare_dram_parameter("coff", [1, 1], mybir.dt.int32,
                                             isOutput=False)

    with tile.TileContext(nc) as tc:
        _body(nc, tc, io)
    nc.compile()
    return nc


def _body(nc, tc, t):
    with tc.tile_pool(name="const", bufs=1) as cst:
        # ---------------- constants ----------------
        ident = cst.tile([P, P], F32)
        make_identity(nc, ident[:])

        onesrow_f = cst.tile([1, TS], F32)
        nc.gpsimd.memset(onesrow_f[:], 1.0)
        ones_tok = cst.tile([1, TS], F32R)      # K=1 rhs (N=tokens)
        nc.vector.tensor_copy(ones_tok[:], onesrow_f[:])
        ones128 = cst.tile([1, P], F32R)        # K=1 lhsT (M=128 tokens)
        nc.vector.tensor_copy(ones128[:], onesrow_f[0:1, 0:P])
        ones64 = cst.tile([1, DH], F32R)
        nc.vector.tensor_copy(ones64[:], onesrow_f[0:1, 0:DH])
        onescol4 = cst.tile([P, HC, 1], F32)
        nc.gpsimd.memset(onescol4[:], 1.0)
        epsc = cst.tile([P, 1], F32)
        nc.gpsimd.memset(epsc[:], EPS)

        # causal mask: maskb[i, m] = 1.0 iff i <= m - 384
        MB = 896
        maskb = cst.tile([P, MB], F32)
        nc.gpsimd.memset(maskb[:], 1.0)
        nc.gpsimd.affine_select(
            out=maskb[:], in_=maskb[:], compare_op=ALU.is_ge,
            fill=0.0, base=-384, pattern=[[1, MB]], channel_multiplier=-1,
        )

        # layernorm gains/biases as [128, KD] (per-partition per k-tile)
        ln1g = cst.tile([P, KD], F32)
        ln1b = cst.tile([P, KD], F32)
        ln2g = cst.tile([P, KD], F32)
        ln2b = cst.tile([P, KD], F32)
        nc.sync.dma_start(ln1g[:], t["ln1g_d"].rearrange("(k p) -> p k", p=P))
        nc.sync.dma_start(ln1b[:], t["ln1b_d"].rearrange("(k p) -> p k", p=P))
        nc.sync.dma_start(ln2g[:], t["ln2g_d"].rearrange("(k p) -> p k", p=P))
        nc.sync.dma_start(ln2b[:], t["ln2b_d"].rearrange("(k p) -> p k", p=P))

        bq = cst.tile([1, CC], F32R)
        bk = cst.tile([1, CC], F32R)
        bv = cst.tile([1, CC], F32R)
        bo = cst.tile([1, D], F32R)
        b1 = cst.tile([1, DFF], F32R)
        b2 = cst.tile([1, D], F32R)
        nc.sync.dma_start(bq[:], t["bq_d"][None, :])
        nc.sync.dma_start(bk[:], t["bk_d"][None, :])
        nc.sync.dma_start(bv[:], t["bv_d"][None, :])
        nc.sync.dma_start(bo[:], t["bo_d"][None, :])
        nc.sync.dma_start(b1[:], t["b1_d"][None, :])
        nc.sync.dma_start(b2[:], t["b2_d"][None, :])

        # ---------------- helpers ----------------
        def layernorm(src_tiles, dst_tiles, sc):
            for mt in range(NT):
                xt = src_tiles[mt]
                mu = sc.tile([P, 1], F32, tag="mu", name="mu")
                nc.vector.tensor_reduce(out=mu[:], in_=xt[:], op=ALU.add,
                                        axis=mybir.AxisListType.X)
                mus = sc.tile([P, 1], F32, tag="mus", name="mus")
                nc.scalar.mul(mus[:], mu[:], 1.0 / D)
                xm = sc.tile([P, D], F32, tag="xm", name="xm")
                nc.vector.tensor_scalar(out=xm[:], in0=xt[:], scalar1=mus[:],
                                        scalar2=None, op0=ALU.subtract)
                sq = sc.tile([P, D], F32, tag="sq", name="sq")
                var = sc.tile([P, 1], F32, tag="var", name="var")
                nc.scalar.activation(sq[:], xm[:], AF.Square, accum_out=var[:])
                std = sc.tile([P, 1], F32, tag="std", name="std")
                nc.scalar.activation(std[:], var[:], AF.Sqrt, bias=epsc[:],
                                     scale=1.0 / D)
                inv = sc.tile([P, 1], F32, tag="inv", name="inv")
                nc.vector.reciprocal(inv[:], std[:])
                nc.vector.tensor_scalar(out=dst_tiles[mt][:], in0=xm[:],
                                        scalar1=inv[:], scalar2=None,
                                        op0=ALU.mult)

        def transpose_apply(src_tiles, dst_tiles, g_sb, b_sb, tp):
            for k in range(KD):
                for mt in range(NT):
                    ps = tp.tile([P, P], F32, tag="tps", name="tps")
                    nc.tensor.transpose(ps[:], src_tiles[mt][:, k * P:(k + 1) * P],
                                        ident[:])
                    nc.vector.tensor_scalar(
                        out=dst_tiles[k][:, mt * P:(mt + 1) * P], in0=ps[:],
                        scalar1=g_sb[:, k:k + 1], scalar2=b_sb[:, k:k + 1],
                        op0=ALU.mult, op1=ALU.add)

        # ============ phase A: LN1 + transpose + AllGather ============
        with tc.tile_pool(name="xsP", bufs=1) as xsp:
            xs = [xsp.tile([P, D], F32, tag=f"xs{mt}", name=f"xs{mt}") for mt in range(NT)]
            for mt in range(NT):
                nc.sync.dma_start(xs[mt][:], t["x_d"][mt * P:(mt + 1) * P, :])

            with tc.tile_pool(name="lnA", bufs=1) as sc, \
                 tc.tile_pool(name="tpA", bufs=4, space="PSUM") as tp:
                h = [sc.tile([P, D], F32, tag=f"h{mt}", name=f"h{mt}") for mt in range(NT)]
                layernorm(xs, h, sc)
                hT = [sc.tile([P, TS], F32R, tag=f"hT{k}", name=f"hT{k}") for k in range(KD)]
                transpose_apply(h, hT, ln1g, ln1b, tp)
                for k in range(KD):
                    nc.sync.dma_start(t["ag1_in"][k * P:(k + 1) * P, :], hT[k][:])

            nc.gpsimd.collective_compute(
                "AllGather", ALU.bypass, ins=[t["ag1_in"][:]],
                outs=[t["ag1_out"][:]], replica_groups=GROUPS,
            )

            # ============ phases B+C: QKV + attention ============
            with tc.tile_pool(name="qkvP", bufs=1) as qkv:
                qT = [qkv.tile([P, S], F32R, tag=f"qT{m}", name=f"qT{m}") for m in range(2)]
                kT = [qkv.tile([P, S], F32R, tag=f"kT{m}", name=f"kT{m}") for m in range(2)]
                vo = [qkv.tile([P, HC, DH + 1], F32R, tag=f"vo{tm}", name=f"vo{tm}")
                      for tm in range(S // P)]

                with tc.tile_pool(name="wqkv", bufs=1) as wp, \
                     tc.tile_pool(name="hTst", bufs=2) as st, \
                     tc.tile_pool(name="projPS", bufs=3, space="PSUM") as pps:
                    wq = [wp.tile([P, CC], F32R, tag=f"wq{k}", name=f"wq{k}") for k in range(KD)]
                    wk = [wp.tile([P, CC], F32R, tag=f"wk{k}", name=f"wk{k}") for k in range(KD)]
                    wv = [wp.tile([P, CC], F32R, tag=f"wv{k}", name=f"wv{k}") for k in range(KD)]
                    for k in range(KD):
                        nc.sync.dma_start(wq[k][:], t["wq_d"][k * P:(k + 1) * P, :])
                        nc.sync.dma_start(wk[k][:], t["wk_d"][k * P:(k + 1) * P, :])
                        nc.sync.dma_start(wv[k][:], t["wv_d"][k * P:(k + 1) * P, :])

                    for qc in range(G):
                        hTq = [st.tile([P, TS], F32R, tag=f"hTq{k}", name=f"hTq{k}")
                               for k in range(KD)]
                        for k in range(KD):
                            nc.sync.dma_start(
                                hTq[k][:], t["ag1_out"][qc, k * P:(k + 1) * P, :])
                        for (w_sb, b_sb, out_sb) in ((wq, bq, qT), (wk, bk, kT)):
                            for m in range(2):
                                ps = pps.tile([P, TS], F32, tag="pps", name="pps")
                                nc.tensor.matmul(ps[:],
                                                 b_sb[:, m * P:(m + 1) * P],
                                                 ones_tok[:], start=True,
                                                 stop=False)
                                for k in range(KD):
                                    nc.tensor.matmul(
                                        ps[:], w_sb[k][:, m * P:(m + 1) * P],
                                        hTq[k][:], start=False,
                                        stop=(k == KD - 1))
                                nc.vector.tensor_copy(
                                    out_sb[m][:, qc * TS:(qc + 1) * TS], ps[:])
                        for mt in range(NT):
                            tm = qc * NT + mt
                            ps = pps.tile([P, CC], F32, tag="vps", name="vps")
                            nc.tensor.matmul(ps[:], ones128[:], bv[:],
                                             start=True, stop=False)
                            for k in range(KD):
                                nc.tensor.matmul(
                                    ps[:], hTq[k][:, mt * P:(mt + 1) * P],
                                    wv[k][:], start=False, stop=(k == KD - 1))
                            nc.vector.tensor_copy(
                                vo[tm][:, :, 0:DH],
                                ps[:].rearrange("p (h e) -> p h e", h=HC))
                            nc.vector.tensor_copy(vo[tm][:, :, DH:DH + 1],
                                                  onescol4[:])

                # ---- attention ----
                with (
                    tc.tile_pool(name="scPS", bufs=3, space="PSUM") as scp,
                    tc.tile_pool(name="avPS", bufs=2, space="PSUM") as avp,
                    tc.tile_pool(name="rbPS", bufs=2, space="PSUM") as rbp,
                    tc.tile_pool(name="attnSB", bufs=3) as asb,
                ):
                    for h_i in range(HC):
                        m = h_i // 2
                        o = (h_i % 2) * DH
                        for qc in range(G):
                            kt_max = 4 * qc + 3
                            av = avp.tile([DH + 1, TS], F32, tag="av", name="av")
                            for kt in range(kt_max + 1):
                                sc_ps = scp.tile([P, TS], F32, tag="sc", name="sc")
                                nc.tensor.matmul(
                                    sc_ps[:],
                                    kT[m][o:o + DH, kt * P:(kt + 1) * P],
                                    qT[m][o:o + DH, qc * TS:(qc + 1) * TS],
                                    start=True, stop=True)
                                e_r = asb.tile([P, TS], F32R, tag="er", name="er")
                                if kt < 4 * qc:
                                    nc.scalar.activation(e_r[:], sc_ps[:],
                                                         AF.Exp, scale=0.125)
                                else:
                                    e_f = asb.tile([P, TS], F32, tag="ef", name="ef")
                                    nc.scalar.activation(e_f[:], sc_ps[:],
                                                         AF.Exp, scale=0.125)
                                    off = 384 + 512 * qc - 128 * kt
                                    nc.vector.tensor_tensor(
                                        out=e_r[:], in0=e_f[:],
                                        in1=maskb[:, off:off + TS],
                                        op=ALU.mult)
                                nc.tensor.matmul(av[:], vo[kt][:, h_i, :],
                                                 e_r[:], start=(kt == 0),
                                                 stop=(kt == kt_max))
                            un = asb.tile([DH + 1, TS], F32, tag="un", name="un")
                            nc.vector.tensor_copy(un[:], av[:])
                            rec = asb.tile([1, TS], F32, tag="rec", name="rec")
                            nc.vector.reciprocal(rec[:], un[DH:DH + 1, :])
                            rec_r = asb.tile([1, TS], F32R, tag="recr", name="recr")
                            nc.vector.tensor_copy(rec_r[:], rec[:])
                            rb = rbp.tile([DH, TS], F32, tag="rb", name="rb")
                            nc.tensor.matmul(rb[:], ones64[:], rec_r[:],
                                             start=True, stop=True)
                            chunk = asb.tile([DH, TS], F32R, tag="chunk", name="chunk")
                            nc.vector.tensor_tensor(out=chunk[:],
                                                    in0=un[0:DH, :],
                                                    in1=rb[:], op=ALU.mult)
                            nc.sync.dma_start(
                                t["a2a_in"][h_i * DH:(h_i + 1) * DH,
                                            qc * TS:(qc + 1) * TS],
                                chunk[:])

            nc.gpsimd.collective_compute(
                "AllGather", ALU.bypass, ins=[t["a2a_in"][:]],
                outs=[t["a2a_out"][:]], replica_groups=GROUPS,
            )

            # ============ phase D: out_proj + residual + LN2 ============
            with tc.tile_pool(name="x2P", bufs=1) as x2p:
                x2 = [x2p.tile([P, D], F32, tag=f"x2{mt}", name=f"x2{mt}") for mt in range(NT)]
                with tc.tile_pool(name="opPS", bufs=3, space="PSUM") as opp, \
                     tc.tile_pool(name="opSB", bufs=1) as osb:
                    aT = [osb.tile([P, TS], F32R, tag=f"aT{k}", name=f"aT{k}") for k in range(KD)]
                    wo = [osb.tile([P, D], F32R, tag=f"wo{k}", name=f"wo{k}") for k in range(KD)]
                    off_sb = osb.tile([1, 1], mybir.dt.int32, tag="off",
                                      name="off")
                    nc.sync.dma_start(off_sb[:], t["coff_d"][:])
                    with nc.gpsimd.register("roff") as roff:
                        nc.gpsimd.reg_load(roff, off_sb[0:1, 0:1])
                        rv = nc.snap(roff)
                        for k in range(KD):
                            nc.gpsimd.dma_start(
                                aT[k][:],
                                t["a2a_out"][k // 2,
                                             (k % 2) * P:(k % 2 + 1) * P,
                                             bass.ds(rv, TS)])
                    for k in range(KD):
                        nc.sync.dma_start(wo[k][:],
                                          t["wo_d"][k * P:(k + 1) * P, :])
                    for mt in range(NT):
                        for n in range(2):
                            ps = opp.tile([P, TS], F32, tag="op", name="op")
                            nc.tensor.matmul(ps[:], ones128[:],
                                             bo[:, n * TS:(n + 1) * TS],
                                             start=True, stop=False)
                            for k in range(KD):
                                nc.tensor.matmul(
                                    ps[:], aT[k][:, mt * P:(mt + 1) * P],
                                    wo[k][:, n * TS:(n + 1) * TS],
                                    start=False, stop=(k == KD - 1))
                            nc.vector.tensor_tensor(
                                out=x2[mt][:, n * TS:(n + 1) * TS], in0=ps[:],
                                in1=xs[mt][:, n * TS:(n + 1) * TS], op=ALU.add)

                # xs no longer needed -> xsP could close, but scoped outside;
                # SBUF is sized to tolerate it.
                with tc.tile_pool(name="h2TP", bufs=1) as h2tp:
                    h2T = [h2tp.tile([P, TS], F32R, tag=f"h2T{k}", name=f"h2T{k}")
                           for k in range(KD)]
                    with tc.tile_pool(name="lnD", bufs=1) as sc, \
                         tc.tile_pool(name="tpD", bufs=4, space="PSUM") as tp:
                        h2 = [sc.tile([P, D], F32, tag=f"h2{mt}", name=f"h2{mt}")
                              for mt in range(NT)]
                        layernorm(x2, h2, sc)
                        transpose_apply(h2, h2T, ln2g, ln2b, tp)

                    # ============ phase E: FFN ============
                    with tc.tile_pool(name="gTP", bufs=1) as gtp:
                        gT = [gtp.tile([P, TS], F32R, tag=f"gT{mf}", name=f"gT{mf}")
                              for mf in range(KF)]
                        MFB = 8     # mf tiles per w1 stream block
                        with tc.tile_pool(name="w1st", bufs=1) as w1p, \
                             tc.tile_pool(name="gPS", bufs=4, space="PSUM") as gps:
                            for blk in range(KF // MFB):
                                w1s = [w1p.tile([P, MFB * P], F32R,
                                                tag=f"w1s{k}", name=f"w1s{k}")
                                       for k in range(KD)]
                                for k in range(KD):
                                    nc.sync.dma_start(
                                        w1s[k][:],
                                        t["w1_d"][k * P:(k + 1) * P,
                                                  blk * MFB * P:(blk + 1) * MFB * P])
                                for j in range(MFB):
                                    mf = blk * MFB + j
                                    ps = gps.tile([P, TS], F32, tag="g", name="g")
                                    nc.tensor.matmul(
                                        ps[:], b1[:, mf * P:(mf + 1) * P],
                                        ones_tok[:], start=True, stop=False)
                                    for k in range(KD):
                                        nc.tensor.matmul(
                                            ps[:], w1s[k][:, j * P:(j + 1) * P],
                                            h2T[k][:], start=False,
                                            stop=(k == KD - 1))
                                    nc.scalar.activation(gT[mf][:], ps[:],
                                                         AF.Gelu)

                        with tc.tile_pool(name="w2st", bufs=4) as w2p, \
                             tc.tile_pool(name="fPS", bufs=1, space="PSUM") as fps, \
                             tc.tile_pool(name="ySB", bufs=2) as ysb:
                            f_ps = [fps.tile([P, D], F32, tag=f"f{mt}", name=f"f{mt}")
                                    for mt in range(NT)]
                            for mt in range(NT):
                                for n in range(2):
                                    nc.tensor.matmul(
                                        f_ps[mt][:, n * TS:(n + 1) * TS],
                                        ones128[:], b2[:, n * TS:(n + 1) * TS],
                                        start=True, stop=False)
                            for k2 in range(KF):
                                w2t = w2p.tile([P, D], F32R, tag="w2", name="w2")
                                nc.sync.dma_start(
                                    w2t[:], t["w2_d"][k2 * P:(k2 + 1) * P, :])
                                for mt in range(NT):
                                    for n in range(2):
                                        nc.tensor.matmul(
                                            f_ps[mt][:, n * TS:(n + 1) * TS],
                                            gT[k2][:, mt * P:(mt + 1) * P],
                                            w2t[:, n * TS:(n + 1) * TS],
                                            start=False, stop=(k2 == KF - 1))
                            for mt in range(NT):
                                yt = ysb.tile([P, D], F32, tag="y", name="y")
                                nc.vector.tensor_tensor(out=yt[:],
                                                        in0=f_ps[mt][:],
                                                        in1=x2[mt][:],
                                                        op=ALU.add)
                                nc.sync.dma_start(
                                    t["y_d"][mt * P:(mt + 1) * P, :], yt[:])


def _in_maps(inputs):
    f32 = np.float32
    maps = []
    for c in range(NC):
        b, r = c // G, c % G
        c0 = r * CC
        m = {
            "x": np.ascontiguousarray(np.asarray(inputs["x"])[b, r * TS:(r + 1) * TS, :], f32),
            "ln1_g": np.ascontiguousarray(inputs["ln1_g"], f32),
            "ln1_b": np.ascontiguousarray(inputs["ln1_b"], f32),
            "Wq": np.ascontiguousarray(np.asarray(inputs["Wq"])[:, c0:c0 + CC], f32),
            "Wk": np.ascontiguousarray(np.asarray(inputs["Wk"])[:, c0:c0 + CC], f32),
            "Wv": np.ascontiguousarray(np.asarray(inputs["Wv"])[:, c0:c0 + CC], f32),
            "bq": np.ascontiguousarray(np.asarray(inputs["bq"])[c0:c0 + CC], f32),
            "bk": np.ascontiguousarray(np.asarray(inputs["bk"])[c0:c0 + CC], f32),
            "bv": np.ascontiguousarray(np.asarray(inputs["bv"])[c0:c0 + CC], f32),
            "Wo": np.ascontiguousarray(inputs["Wo"], f32),
            "bo": np.ascontiguousarray(inputs["bo"], f32),
            "ln2_g": np.ascontiguousarray(inputs["ln2_g"], f32),
            "ln2_b": np.ascontiguousarray(inputs["ln2_b"], f32),
            "W1": np.ascontiguousarray(inputs["W1"], f32),
            "b1": np.ascontiguousarray(inputs["b1"], f32),
            "W2": np.ascontiguousarray(inputs["W2"], f32),
            "b2": np.ascontiguousarray(inputs["b2"], f32),
            "coff": np.array([[r * TS]], dtype=np.int32),
        }
        maps.append(m)
    return maps


def _run(inputs, trace=False):
    if "nc" not in _CACHE:
        _CACHE["nc"] = build()
    nc = _CACHE["nc"]
    maps = _in_maps(inputs)
    res = run_bass_kernel_spmd(nc, maps, list(range(NC)), trace=trace)
    out = np.empty((B, S, D), np.float32)
    for c in range(NC):
        b, r = c // G, c % G
        out[b, r * TS:(r + 1) * TS, :] = res.results[c]["y"]
    return out, res


def kernel(**inputs):
    out, _ = _run(inputs, trace=False)
    return out


if __name__ == "__main__":
    build()
    print("build OK")


# revision 17
# speedup vs baseline: 42.8141x; 42.8141x over previous
"""Trainium2 Bass kernel for a dense transformer block (B=2,S=2048,D=1024,H=16,DFF=4096).

Sharding across 8 NeuronCores:
  core c: batch b=c//4, group rank r=c%4, replica groups [[0,1,2,3],[4,5,6,7]].
  - LN1 computed on own 512-token strip; hT AllGather'ed within the 4-core group.
  - Attention: head-parallel (4 heads/core, full causal sequence).
  - AllToAll redistributes attention output from head-sharded to token-sharded.
  - out_proj, LN2, FFN: token-sharded (512 tokens/core), full weights.
Matmuls run in float32r (TF32-like, full PE rate); the residual spine stays fp32.
"""
import sys

sys.path.insert(0, "/opt/trn_rl_repo")

import numpy as np

import concourse.bass as bass
import concourse.mybir as mybir
import concourse.tile as tile
from concourse import bacc
from concourse.bass_utils import run_bass_kernel_spmd
from concourse.masks import make_identity

AF = mybir.ActivationFunctionType
ALU = mybir.AluOpType
F32 = mybir.dt.float32
F32R = mybir.dt.float32r

B, S, D, H = 2, 2048, 1024, 16
DH = D // H          # 64
DFF = 4 * D          # 4096
EPS = 1e-5
NC = 8               # cores
G = 4                # cores per group (per batch)
TS = S // G          # 512 tokens per core
HC = H // G          # 4 heads per core
CC = HC * DH         # 256 head-columns per core
P = 128
KD = D // P          # 8 k-tiles over D
KF = DFF // P        # 32 k-tiles over DFF
NT = TS // P         # 4 token tiles per strip
GROUPS = [[0, 1, 2, 3], [4, 5, 6, 7]]

_CACHE = {}


def build():
    nc = bacc.Bacc(None)

    io = {}
    io["x_d"] = nc.declare_dram_parameter("x", [TS, D], F32, isOutput=False)
    io["ln1g_d"] = nc.declare_dram_parameter("ln1_g", [D], F32, isOutput=False)
    io["ln1b_d"] = nc.declare_dram_parameter("ln1_b", [D], F32, isOutput=False)
    io["wq_d"] = nc.declare_dram_parameter("Wq", [D, CC], F32R, isOutput=False)
    io["wk_d"] = nc.declare_dram_parameter("Wk", [D, CC], F32R, isOutput=False)
    io["wv_d"] = nc.declare_dram_parameter("Wv", [D, CC], F32R, isOutput=False)
    io["bq_d"] = nc.declare_dram_parameter("bq", [CC], F32R, isOutput=False)
    io["bk_d"] = nc.declare_dram_parameter("bk", [CC], F32R, isOutput=False)
    io["bv_d"] = nc.declare_dram_parameter("bv", [CC], F32R, isOutput=False)
    io["wo_d"] = nc.declare_dram_parameter("Wo", [D, D], F32R, isOutput=False)
    io["bo_d"] = nc.declare_dram_parameter("bo", [D], F32R, isOutput=False)
    io["ln2g_d"] = nc.declare_dram_parameter("ln2_g", [D], F32, isOutput=False)
    io["ln2b_d"] = nc.declare_dram_parameter("ln2_b", [D], F32, isOutput=False)
    io["w1_d"] = nc.declare_dram_parameter("W1", [D, DFF], F32R, isOutput=False)
    io["b1_d"] = nc.declare_dram_parameter("b1", [DFF], F32R, isOutput=False)
    io["w2_d"] = nc.declare_dram_parameter("W2", [DFF, D], F32R, isOutput=False)
    io["b2_d"] = nc.declare_dram_parameter("b2", [D], F32R, isOutput=False)
    io["y_d"] = nc.declare_dram_parameter("y", [TS, D], F32, isOutput=True)

    io["ag1_in"] = nc.dram_tensor("ag1_in", [D, TS], F32R)
    io["ag1_out"] = nc.dram_tensor("ag1_out", [G, D, TS], F32R)
    io["a2a_in"] = nc.dram_tensor("a2a_in", [CC, S], F32R)
    io["a2a_out"] = nc.dram_tensor("a2a_out", [G, CC, S], F32R)
    io["coff_d"] = nc.declare_dram_parameter("coff", [1, 1], mybir.dt.int32,
                                             isOutput=False)

    with tile.TileContext(nc) as tc:
        _body(nc, tc, io)
    nc.compile()
    return nc


def _body(nc, tc, t):
    with tc.tile_pool(name="const", bufs=1) as cst:
        # ---------------- constants ----------------
        ident = cst.tile([P, P], F32)
        make_identity(nc, ident[:])

        onesrow_f = cst.tile([1, TS], F32)
        nc.gpsimd.memset(onesrow_f[:], 1.0)
        ones_tok = cst.tile([1, TS], F32R)      # K=1 rhs (N=tokens)
        nc.vector.tensor_copy(ones_tok[:], onesrow_f[:])
        ones128 = cst.tile([1, P], F32R)        # K=1 lhsT (M=128 tokens)
        nc.vector.tensor_copy(ones128[:], onesrow_f[0:1, 0:P])
        ones64 = cst.tile([1, DH], F32R)
        nc.vector.tensor_copy(ones64[:], onesrow_f[0:1, 0:DH])
        onescol4 = cst.tile([P, HC, 1], F32)
        nc.gpsimd.memset(onescol4[:], 1.0)
        epsc = cst.tile([P, 1], F32)
        nc.gpsimd.memset(epsc[:], EPS)

        # doubled causal masks (one per diagonal shift), mask||mask layout so a
        # single DVE op masks a two-head [128, 1024] pair tile.
        maskd = {}
        for sh in (0, -128, -256, -384):
            md = cst.tile([P, 2 * TS], F32, tag=f"maskd{sh}", name=f"maskd{sh}")
            nc.gpsimd.memset(md[:], 1.0)
            for half in range(2):
                nc.gpsimd.affine_select(
                    out=md[:, half * TS:(half + 1) * TS],
                    in_=md[:, half * TS:(half + 1) * TS],
                    compare_op=ALU.is_ge, fill=0.0, base=sh,
                    pattern=[[1, TS]], channel_multiplier=-1,
                )
            maskd[sh] = md

        # layernorm gains/biases as [128, KD] (per-partition per k-tile)
        ln1g = cst.tile([P, KD], F32)
        ln1b = cst.tile([P, KD], F32)
        ln2g = cst.tile([P, KD], F32)
        ln2b = cst.tile([P, KD], F32)
        nc.sync.dma_start(ln1g[:], t["ln1g_d"].rearrange("(k p) -> p k", p=P))
        nc.sync.dma_start(ln1b[:], t["ln1b_d"].rearrange("(k p) -> p k", p=P))
        nc.sync.dma_start(ln2g[:], t["ln2g_d"].rearrange("(k p) -> p k", p=P))
        nc.sync.dma_start(ln2b[:], t["ln2b_d"].rearrange("(k p) -> p k", p=P))

        bq = cst.tile([1, CC], F32R)
        bk = cst.tile([1, CC], F32R)
        bv = cst.tile([1, CC], F32R)
        bo = cst.tile([1, D], F32R)
        b2 = cst.tile([1, D], F32R)
        nc.sync.dma_start(bq[:], t["bq_d"][None, :])
        nc.sync.dma_start(bk[:], t["bk_d"][None, :])
        nc.sync.dma_start(bv[:], t["bv_d"][None, :])
        nc.sync.dma_start(bo[:], t["bo_d"][None, :])
        nc.sync.dma_start(b2[:], t["b2_d"][None, :])

        # ---------------- helpers ----------------
        def layernorm(src_tiles, dst_tiles, sc):
            for mt in range(NT):
                xt = src_tiles[mt]
                mu = sc.tile([P, 1], F32, tag="mu", name="mu")
                nc.vector.tensor_reduce(out=mu[:], in_=xt[:], op=ALU.add,
                                        axis=mybir.AxisListType.X)
                mus = sc.tile([P, 1], F32, tag="mus", name="mus")
                nc.scalar.mul(mus[:], mu[:], 1.0 / D)
                xm = sc.tile([P, D], F32, tag="xm", name="xm")
                nc.vector.tensor_scalar(out=xm[:], in0=xt[:], scalar1=mus[:],
                                        scalar2=None, op0=ALU.subtract)
                sq = sc.tile([P, D], F32, tag="sq", name="sq")
                var = sc.tile([P, 1], F32, tag="var", name="var")
                nc.scalar.activation(sq[:], xm[:], AF.Square, accum_out=var[:])
                std = sc.tile([P, 1], F32, tag="std", name="std")
                nc.scalar.activation(std[:], var[:], AF.Sqrt, bias=epsc[:],
                                     scale=1.0 / D)
                inv = sc.tile([P, 1], F32, tag="inv", name="inv")
                nc.vector.reciprocal(inv[:], std[:])
                nc.vector.tensor_scalar(out=dst_tiles[mt][:], in0=xm[:],
                                        scalar1=inv[:], scalar2=None,
                                        op0=ALU.mult)

        def transpose_apply(src_tiles, dst_tiles, g_sb, b_sb, tp):
            for k in range(KD):
                for mt in range(NT):
                    ps = tp.tile([P, P], F32, tag="tps", name="tps")
                    nc.tensor.transpose(ps[:], src_tiles[mt][:, k * P:(k + 1) * P],
                                        ident[:])
                    nc.vector.tensor_scalar(
                        out=dst_tiles[k][:, mt * P:(mt + 1) * P], in0=ps[:],
                        scalar1=g_sb[:, k:k + 1], scalar2=b_sb[:, k:k + 1],
                        op0=ALU.mult, op1=ALU.add)

        # ============ phase A: LN1 + transpose + AllGather ============
        with tc.tile_pool(name="xsP", bufs=1) as xsp:
            xs = [xsp.tile([P, D], F32, tag=f"xs{mt}", name=f"xs{mt}") for mt in range(NT)]
            for mt in range(NT):
                nc.sync.dma_start(xs[mt][:], t["x_d"][mt * P:(mt + 1) * P, :])

            with tc.tile_pool(name="lnA", bufs=1) as sc, \
                 tc.tile_pool(name="tpA", bufs=4, space="PSUM") as tp:
                h = [sc.tile([P, D], F32, tag=f"h{mt}", name=f"h{mt}") for mt in range(NT)]
                layernorm(xs, h, sc)
                hT = [sc.tile([P, TS], F32R, tag=f"hT{k}", name=f"hT{k}") for k in range(KD)]
                transpose_apply(h, hT, ln1g, ln1b, tp)
                for k in range(KD):
                    nc.sync.dma_start(t["ag1_in"][k * P:(k + 1) * P, :], hT[k][:])

            # ============ phases B+C: QKV + attention ============
            with tc.tile_pool(name="qkvP", bufs=1) as qkv:
                qT = [qkv.tile([P, S], F32R, tag=f"qT{m}", name=f"qT{m}") for m in range(2)]
                kT = [qkv.tile([P, S], F32R, tag=f"kT{m}", name=f"kT{m}") for m in range(2)]
                vo = [qkv.tile([P, HC, DH + 1], F32R, tag=f"vo{tm}", name=f"vo{tm}")
                      for tm in range(S // P)]

                wp_cm = tc.tile_pool(name="wqkv", bufs=1)
                wp = wp_cm.__enter__()
                # weight loads traced before the collective: DMA covers AG1
                wq = [wp.tile([P, CC], F32R, tag=f"wq{k}", name=f"wq{k}") for k in range(KD)]
                wk = [wp.tile([P, CC], F32R, tag=f"wk{k}", name=f"wk{k}") for k in range(KD)]
                wv = [wp.tile([P, CC], F32R, tag=f"wv{k}", name=f"wv{k}") for k in range(KD)]
                for k in range(KD):
                    nc.sync.dma_start(wq[k][:], t["wq_d"][k * P:(k + 1) * P, :])
                    nc.sync.dma_start(wk[k][:], t["wk_d"][k * P:(k + 1) * P, :])
                    nc.sync.dma_start(wv[k][:], t["wv_d"][k * P:(k + 1) * P, :])

                nc.gpsimd.collective_compute(
                    "AllGather", ALU.bypass, ins=[t["ag1_in"][:]],
                    outs=[t["ag1_out"][:]], replica_groups=GROUPS,
                )

                with tc.tile_pool(name="hTst", bufs=2) as st, \
                     tc.tile_pool(name="projPS", bufs=3, space="PSUM") as pps:
                    for qc in range(G):
                        hTq = [st.tile([P, TS], F32R, tag=f"hTq{k}", name=f"hTq{k}")
                               for k in range(KD)]
                        for k in range(KD):
                            nc.sync.dma_start(
                                hTq[k][:], t["ag1_out"][qc, k * P:(k + 1) * P, :])
                        for (w_sb, b_sb, out_sb) in ((wq, bq, qT), (wk, bk, kT)):
                            for m in range(2):
                                ps = pps.tile([P, TS], F32, tag="pps", name="pps")
                                nc.tensor.matmul(ps[:],
                                                 b_sb[:, m * P:(m + 1) * P],
                                                 ones_tok[:], start=True,
                                                 stop=False)
                                for k in range(KD):
                                    nc.tensor.matmul(
                                        ps[:], w_sb[k][:, m * P:(m + 1) * P],
                                        hTq[k][:], start=False,
                                        stop=(k == KD - 1))
                                nc.vector.tensor_copy(
                                    out_sb[m][:, qc * TS:(qc + 1) * TS], ps[:])
                        for mt in range(NT):
                            tm = qc * NT + mt
                            ps = pps.tile([P, CC], F32, tag="vps", name="vps")
                            nc.tensor.matmul(ps[:], ones128[:], bv[:],
                                             start=True, stop=False)
                            for k in range(KD):
                                nc.tensor.matmul(
                                    ps[:], hTq[k][:, mt * P:(mt + 1) * P],
                                    wv[k][:], start=False, stop=(k == KD - 1))
                            nc.vector.tensor_copy(
                                vo[tm][:, :, 0:DH],
                                ps[:].rearrange("p (h e) -> p h e", h=HC))
                            nc.vector.tensor_copy(vo[tm][:, :, DH:DH + 1],
                                                  onescol4[:])

                wp_cm.__exit__(None, None, None)

                # ---- attention ----
                with (
                    tc.tile_pool(name="scPS", bufs=2, space="PSUM") as scp,
                    tc.tile_pool(name="avPS", bufs=2, space="PSUM") as avp,
                    tc.tile_pool(name="attnSB", bufs=3) as asb,
                ):
                    for hp in range(HC // 2):      # head pairs at PE rows 0/64
                        for qc in range(G):
                            kt_max = 4 * qc + 3
                            avs = [avp.tile([DH + 1, TS], F32, tag=f"av{j}",
                                            name=f"av{j}") for j in range(2)]
                            for kt in range(kt_max + 1):
                                # both heads' score blocks into one 2-bank tile
                                sc_ps = scp.tile([P, 2, TS], F32,
                                                 tag="scp", name="scp")
                                for j in range(2):
                                    h_i = 2 * hp + j
                                    m = h_i // 2
                                    o = (h_i % 2) * DH
                                    nc.tensor.matmul(
                                        sc_ps[:, j, :],
                                        kT[m][o:o + DH, kt * P:(kt + 1) * P],
                                        qT[m][o:o + DH, qc * TS:(qc + 1) * TS],
                                        start=True, stop=True)
                                e_r = asb.tile([P, 2, TS], F32R,
                                               tag="erp", name="erp")
                                if kt < 4 * qc:
                                    nc.scalar.activation(
                                        e_r[:].rearrange("p a b -> p (a b)"),
                                        sc_ps[:].rearrange("p a b -> p (a b)"),
                                        AF.Exp, scale=0.125)
                                else:
                                    e_f = asb.tile([P, 2, TS], F32,
                                                   tag="efp", name="efp")
                                    nc.scalar.activation(
                                        e_f[:].rearrange("p a b -> p (a b)"),
                                        sc_ps[:].rearrange("p a b -> p (a b)"),
                                        AF.Exp, scale=0.125)
                                    sh = 512 * qc - 128 * kt
                                    nc.vector.tensor_tensor(
                                        out=e_r[:].rearrange("p a b -> p (a b)"),
                                        in0=e_f[:].rearrange("p a b -> p (a b)"),
                                        in1=maskd[sh][:],
                                        op=ALU.mult)
                                for j in range(2):
                                    h_i = 2 * hp + j
                                    nc.tensor.matmul(avs[j][:],
                                                     vo[kt][:, h_i, :],
                                                     e_r[:, j, :],
                                                     start=(kt == 0),
                                                     stop=(kt == kt_max))
                            for j in range(2):
                                h_i = 2 * hp + j
                                un = asb.tile([DH + 1, TS], F32,
                                              tag=f"un{j}", name=f"un{j}")
                                nc.vector.tensor_copy(un[:], avs[j][:])
                                rec = asb.tile([1, TS], F32,
                                               tag=f"rec{j}", name=f"rec{j}")
                                nc.vector.reciprocal(rec[:], un[DH:DH + 1, :])
                                rb = asb.tile([DH, TS], F32,
                                              tag=f"rb{j}", name=f"rb{j}")
                                nc.gpsimd.partition_broadcast(rb[:], rec[:])
                                chunk = asb.tile([DH, TS], F32R,
                                                 tag=f"chunk{j}", name=f"chunk{j}")
                                nc.vector.tensor_tensor(out=chunk[:],
                                                        in0=un[0:DH, :],
                                                        in1=rb[:], op=ALU.mult)
                                nc.sync.dma_start(
                                    t["a2a_in"][h_i * DH:(h_i + 1) * DH,
                                                qc * TS:(qc + 1) * TS],
                                    chunk[:])

            # prefetch pools traced before AG2 so DMA covers the collective
            pfw1_cm = tc.tile_pool(name="pfW1", bufs=1)
            pfw1 = pfw1_cm.__enter__()
            w1s0 = [pfw1.tile([P, 8 * P], F32R, tag=f"w1s{k}", name=f"w1s{k}")
                    for k in range(KD)]
            pfd_cm = tc.tile_pool(name="pfD", bufs=1)
            pfd = pfd_cm.__enter__()
            wo = [pfd.tile([P, D], F32R, tag=f"wo{k}", name=f"wo{k}") for k in range(KD)]
            for k in range(KD):
                nc.sync.dma_start(wo[k][:], t["wo_d"][k * P:(k + 1) * P, :])
                nc.sync.dma_start(w1s0[k][:], t["w1_d"][k * P:(k + 1) * P, 0:8 * P])

            nc.gpsimd.collective_compute(
                "AllGather", ALU.bypass, ins=[t["a2a_in"][:]],
                outs=[t["a2a_out"][:]], replica_groups=GROUPS,
            )

            # ============ phase D: out_proj + residual (in-place on xs) ====
            with tc.tile_pool(name="opPS", bufs=3, space="PSUM") as opp, \
                 tc.tile_pool(name="opSB", bufs=1) as osb:
                aT = [osb.tile([P, TS], F32R, tag=f"aT{k}", name=f"aT{k}") for k in range(KD)]
                off_sb = osb.tile([1, 1], mybir.dt.int32, tag="off", name="off")
                nc.sync.dma_start(off_sb[:], t["coff_d"][:])
                with nc.gpsimd.register("roff") as roff:
                    nc.gpsimd.reg_load(roff, off_sb[0:1, 0:1])
                    rv = nc.snap(roff)
                    for k in range(KD):
                        nc.gpsimd.dma_start(
                            aT[k][:],
                            t["a2a_out"][k // 2, (k % 2) * P:(k % 2 + 1) * P,
                                         bass.ds(rv, TS)])
                for mt in range(NT):
                    for n in range(2):
                        ps = opp.tile([P, TS], F32, tag="op", name="op")
                        nc.tensor.matmul(ps[:], ones128[:],
                                         bo[:, n * TS:(n + 1) * TS],
                                         start=True, stop=False)
                        for k in range(KD):
                            nc.tensor.matmul(
                                ps[:], aT[k][:, mt * P:(mt + 1) * P],
                                wo[k][:, n * TS:(n + 1) * TS],
                                start=False, stop=(k == KD - 1))
                        # residual written in place: xs becomes x2
                        nc.vector.tensor_tensor(
                            out=xs[mt][:, n * TS:(n + 1) * TS], in0=ps[:],
                            in1=xs[mt][:, n * TS:(n + 1) * TS], op=ALU.add)
            pfd_cm.__exit__(None, None, None)
            x2 = xs

            with tc.tile_pool(name="h2TP", bufs=1) as h2tp:
                h2T = [h2tp.tile([P, TS], F32R, tag=f"h2T{k}", name=f"h2T{k}")
                       for k in range(KD)]
                with tc.tile_pool(name="lnD", bufs=1) as sc, \
                     tc.tile_pool(name="tpD", bufs=4, space="PSUM") as tp:
                    h2 = [sc.tile([P, D], F32, tag=f"h2{mt}", name=f"h2{mt}")
                          for mt in range(NT)]
                    layernorm(x2, h2, sc)
                    transpose_apply(h2, h2T, ln2g, ln2b, tp)

                # ============ phase E: FFN ============
                with tc.tile_pool(name="gTP", bufs=1) as gtp:
                    gT = [gtp.tile([P, TS], F32R, tag=f"gT{mf}", name=f"gT{mf}")
                          for mf in range(KF)]
                    MFB = 8     # mf tiles per w1 stream block
                    with tc.tile_pool(name="w1st", bufs=1) as w1p, \
                         tc.tile_pool(name="gPS", bufs=4, space="PSUM") as gps:
                        for blk in range(KF // MFB):
                            if blk == 0:
                                w1s = w1s0
                            else:
                                w1s = [w1p.tile([P, MFB * P], F32R,
                                                tag=f"w1b{k}", name=f"w1b{k}")
                                       for k in range(KD)]
                                for k in range(KD):
                                    nc.sync.dma_start(
                                        w1s[k][:],
                                        t["w1_d"][k * P:(k + 1) * P,
                                                  blk * MFB * P:(blk + 1) * MFB * P])
                            b1s = w1p.tile([1, MFB * P], F32R,
                                           tag="b1s", name="b1s", bufs=2)
                            nc.sync.dma_start(
                                b1s[:],
                                t["b1_d"][None, blk * MFB * P:(blk + 1) * MFB * P])
                            for j in range(MFB):
                                mf = blk * MFB + j
                                ps = gps.tile([P, TS], F32, tag="g", name="g")
                                nc.tensor.matmul(
                                    ps[:], b1s[:, j * P:(j + 1) * P],
                                    ones_tok[:], start=True, stop=False)
                                for k in range(KD):
                                    nc.tensor.matmul(
                                        ps[:], w1s[k][:, j * P:(j + 1) * P],
                                        h2T[k][:], start=False,
                                        stop=(k == KD - 1))
                                nc.scalar.activation(gT[mf][:], ps[:],
                                                     AF.Gelu)

                    with tc.tile_pool(name="w2st", bufs=4) as w2p, \
                         tc.tile_pool(name="fPS", bufs=1, space="PSUM") as fps, \
                         tc.tile_pool(name="ySB", bufs=2) as ysb:
                        f_ps = [fps.tile([P, D], F32, tag=f"f{mt}", name=f"f{mt}")
                                for mt in range(NT)]
                        for mt in range(NT):
                            for n in range(2):
                                nc.tensor.matmul(
                                    f_ps[mt][:, n * TS:(n + 1) * TS],
                                    ones128[:], b2[:, n * TS:(n + 1) * TS],
                                    start=True, stop=False)
                        for k2 in range(KF):
                            w2t = w2p.tile([P, D], F32R, tag="w2", name="w2")
                            nc.sync.dma_start(
                                w2t[:], t["w2_d"][k2 * P:(k2 + 1) * P, :])
                            for mt in range(NT):
                                for n in range(2):
                                    nc.tensor.matmul(
                                        f_ps[mt][:, n * TS:(n + 1) * TS],
                                        gT[k2][:, mt * P:(mt + 1) * P],
                                        w2t[:, n * TS:(n + 1) * TS],
                                        start=False, stop=(k2 == KF - 1))
                        for mt in range(NT):
                            yt = ysb.tile([P, D], F32, tag="y", name="y")
                            nc.vector.tensor_tensor(out=yt[:],
                                                    in0=f_ps[mt][:],
                                                    in1=x2[mt][:],
                                                    op=ALU.add)
                            nc.sync.dma_start(
                                t["y_d"][mt * P:(mt + 1) * P, :], yt[:])
            pfw1_cm.__exit__(None, None, None)


def _in_maps(inputs):
    f32 = np.float32
    maps = []
    for c in range(NC):
        b, r = c // G, c % G
        c0 = r * CC
        m = {
            "x": np.ascontiguousarray(np.asarray(inputs["x"])[b, r * TS:(r + 1) * TS, :], f32),
            "ln1_g": np.ascontiguousarray(inputs["ln1_g"], f32),
            "ln1_b": np.ascontiguousarray(inputs["ln1_b"], f32),
            "Wq": np.ascontiguousarray(np.asarray(inputs["Wq"])[:, c0:c0 + CC], f32),
            "Wk": np.ascontiguousarray(np.asarray(inputs["Wk"])[:, c0:c0 + CC], f32),
            "Wv": np.ascontiguousarray(np.asarray(inputs["Wv"])[:, c0:c0 + CC], f32),
            "bq": np.ascontiguousarray(np.asarray(inputs["bq"])[c0:c0 + CC], f32),
            "bk": np.ascontiguousarray(np.asarray(inputs["bk"])[c0:c0 + CC], f32),
            "bv": np.ascontiguousarray(np.asarray(inputs["bv"])[c0:c0 + CC], f32),
            "Wo": np.ascontiguousarray(inputs["Wo"], f32),
            "bo": np.ascontiguousarray(inputs["bo"], f32),
            "ln2_g": np.ascontiguousarray(inputs["ln2_g"], f32),
            "ln2_b": np.ascontiguousarray(inputs["ln2_b"], f32),
            "W1": np.ascontiguousarray(inputs["W1"], f32),
            "b1": np.ascontiguousarray(inputs["b1"], f32),
            "W2": np.ascontiguousarray(inputs["W2"], f32),
            "b2": np.ascontiguousarray(inputs["b2"], f32),
            "coff": np.array([[r * TS]], dtype=np.int32),
        }
        maps.append(m)
    return maps


def _run(inputs, trace=False):
    if "nc" not in _CACHE:
        _CACHE["nc"] = build()
    nc = _CACHE["nc"]
    maps = _in_maps(inputs)
    res = run_bass_kernel_spmd(nc, maps, list(range(NC)), trace=trace)
    out = np.empty((B, S, D), np.float32)
    for c in range(NC):
        b, r = c // G, c % G
        out[b, r * TS:(r + 1) * TS, :] = res.results[c]["y"]
    return out, res


def kernel(**inputs):
    out, _ = _run(inputs, trace=False)
    return out


if __name__ == "__main__":
    build()
    print("build OK")


# revision 18
# speedup vs baseline: 43.2372x; 1.0099x over previous
"""Trainium2 Bass kernel for a dense transformer block (B=2,S=2048,D=1024,H=16,DFF=4096).

Sharding across 8 NeuronCores:
  core c: batch b=c//4, group rank r=c%4, replica groups [[0,1,2,3],[4,5,6,7]].
  - LN1 computed on own 512-token strip; hT AllGather'ed within the 4-core group.
  - Attention: head-parallel (4 heads/core, full causal sequence).
  - AllToAll redistributes attention output from head-sharded to token-sharded.
  - out_proj, LN2, FFN: token-sharded (512 tokens/core), full weights.
Matmuls run in float32r (TF32-like, full PE rate); the residual spine stays fp32.
"""
import sys

sys.path.insert(0, "/opt/trn_rl_repo")

import numpy as np

import concourse.bass as bass
import concourse.mybir as mybir
import concourse.tile as tile
from concourse import bacc
from concourse.bass_utils import run_bass_kernel_spmd
from concourse.masks import make_identity

AF = mybir.ActivationFunctionType
ALU = mybir.AluOpType
F32 = mybir.dt.float32
F32R = mybir.dt.float32r

B, S, D, H = 2, 2048, 1024, 16
DH = D // H          # 64
DFF = 4 * D          # 4096
EPS = 1e-5
NC = 8               # cores
G = 4                # cores per group (per batch)
TS = S // G          # 512 tokens per core
HC = H // G          # 4 heads per core
CC = HC * DH         # 256 head-columns per core
P = 128
KD = D // P          # 8 k-tiles over D
KF = DFF // P        # 32 k-tiles over DFF
NT = TS // P         # 4 token tiles per strip
GROUPS = [[0, 1, 2, 3], [4, 5, 6, 7]]

_CACHE = {}


def build():
    nc = bacc.Bacc(None)

    io = {}
    io["x_d"] = nc.declare_dram_parameter("x", [TS, D], F32, isOutput=False)
    io["ln1g_d"] = nc.declare_dram_parameter("ln1_g", [D], F32, isOutput=False)
    io["ln1b_d"] = nc.declare_dram_parameter("ln1_b", [D], F32, isOutput=False)
    io["wq_d"] = nc.declare_dram_parameter("Wq", [D, CC], F32R, isOutput=False)
    io["wk_d"] = nc.declare_dram_parameter("Wk", [D, CC], F32R, isOutput=False)
    io["wv_d"] = nc.declare_dram_parameter("Wv", [D, CC], F32R, isOutput=False)
    io["bq_d"] = nc.declare_dram_parameter("bq", [CC], F32R, isOutput=False)
    io["bk_d"] = nc.declare_dram_parameter("bk", [CC], F32R, isOutput=False)
    io["bv_d"] = nc.declare_dram_parameter("bv", [CC], F32R, isOutput=False)
    io["wo_d"] = nc.declare_dram_parameter("Wo", [D, D], F32R, isOutput=False)
    io["bo_d"] = nc.declare_dram_parameter("bo", [D], F32R, isOutput=False)
    io["ln2g_d"] = nc.declare_dram_parameter("ln2_g", [D], F32, isOutput=False)
    io["ln2b_d"] = nc.declare_dram_parameter("ln2_b", [D], F32, isOutput=False)
    io["w1_d"] = nc.declare_dram_parameter("W1", [D, DFF], F32R, isOutput=False)
    io["b1_d"] = nc.declare_dram_parameter("b1", [DFF], F32R, isOutput=False)
    io["w2_d"] = nc.declare_dram_parameter("W2", [DFF, D], F32R, isOutput=False)
    io["b2_d"] = nc.declare_dram_parameter("b2", [D], F32R, isOutput=False)
    io["y_d"] = nc.declare_dram_parameter("y", [TS, D], F32, isOutput=True)

    io["ag1_in"] = nc.dram_tensor("ag1_in", [D, TS], F32R)
    io["ag1_out"] = nc.dram_tensor("ag1_out", [G, D, TS], F32R)
    io["a2a_in"] = nc.dram_tensor("a2a_in", [CC, S], F32R)
    io["a2a_out"] = nc.dram_tensor("a2a_out", [G, CC, S], F32R)
    io["coff_d"] = nc.declare_dram_parameter("coff", [1, 1], mybir.dt.int32,
                                             isOutput=False)

    with tile.TileContext(nc) as tc:
        _body(nc, tc, io)
    nc.compile()
    return nc


def _body(nc, tc, t):
    with tc.tile_pool(name="const", bufs=1) as cst:
        # ---------------- constants ----------------
        ident = cst.tile([P, P], F32)
        make_identity(nc, ident[:])

        onesrow_f = cst.tile([1, TS], F32)
        nc.gpsimd.memset(onesrow_f[:], 1.0)
        ones_tok = cst.tile([1, TS], F32R)      # K=1 rhs (N=tokens)
        nc.vector.tensor_copy(ones_tok[:], onesrow_f[:])
        ones128 = cst.tile([1, P], F32R)        # K=1 lhsT (M=128 tokens)
        nc.vector.tensor_copy(ones128[:], onesrow_f[0:1, 0:P])
        ones64 = cst.tile([1, DH], F32R)
        nc.vector.tensor_copy(ones64[:], onesrow_f[0:1, 0:DH])
        onescol4 = cst.tile([P, HC, 1], F32)
        nc.gpsimd.memset(onescol4[:], 1.0)
        epsc = cst.tile([P, 1], F32)
        nc.gpsimd.memset(epsc[:], EPS)

        # doubled causal masks (one per diagonal shift), mask||mask layout so a
        # single DVE op masks a two-head [128, 1024] pair tile.
        maskd = {}
        for sh in (0, -128, -256, -384):
            md = cst.tile([P, 2 * TS], F32, tag=f"maskd{sh}", name=f"maskd{sh}")
            nc.gpsimd.memset(md[:], 1.0)
            for half in range(2):
                nc.gpsimd.affine_select(
                    out=md[:, half * TS:(half + 1) * TS],
                    in_=md[:, half * TS:(half + 1) * TS],
                    compare_op=ALU.is_ge, fill=0.0, base=sh,
                    pattern=[[1, TS]], channel_multiplier=-1,
                )
            maskd[sh] = md

        # layernorm gains/biases as [128, KD] (per-partition per k-tile)
        ln1g = cst.tile([P, KD], F32)
        ln1b = cst.tile([P, KD], F32)
        ln2g = cst.tile([P, KD], F32)
        ln2b = cst.tile([P, KD], F32)
        nc.sync.dma_start(ln1g[:], t["ln1g_d"].rearrange("(k p) -> p k", p=P))
        nc.sync.dma_start(ln1b[:], t["ln1b_d"].rearrange("(k p) -> p k", p=P))
        nc.sync.dma_start(ln2g[:], t["ln2g_d"].rearrange("(k p) -> p k", p=P))
        nc.sync.dma_start(ln2b[:], t["ln2b_d"].rearrange("(k p) -> p k", p=P))

        bq = cst.tile([1, CC], F32R)
        bk = cst.tile([1, CC], F32R)
        bv = cst.tile([1, CC], F32R)
        bo = cst.tile([1, D], F32R)
        b2 = cst.tile([1, D], F32R)
        nc.sync.dma_start(bq[:], t["bq_d"][None, :])
        nc.sync.dma_start(bk[:], t["bk_d"][None, :])
        nc.sync.dma_start(bv[:], t["bv_d"][None, :])
        nc.sync.dma_start(bo[:], t["bo_d"][None, :])
        nc.sync.dma_start(b2[:], t["b2_d"][None, :])

        # ---------------- helpers ----------------
        def layernorm(src_tiles, dst_tiles, sc):
            # var = E[x^2] - mu^2 (safe: |mu| << std for this data), so the
            # normalize is a single fused (x - mu) * inv DVE pass.
            for mt in range(NT):
                xt = src_tiles[mt]
                mu = sc.tile([P, 1], F32, tag="mu", name="mu")
                nc.vector.tensor_reduce(out=mu[:], in_=xt[:], op=ALU.add,
                                        axis=mybir.AxisListType.X)
                mus = sc.tile([P, 1], F32, tag="mus", name="mus")
                nc.scalar.mul(mus[:], mu[:], 1.0 / D)
                sq = sc.tile([P, D], F32, tag="sq", name="sq")
                sumsq = sc.tile([P, 1], F32, tag="sumsq", name="sumsq")
                nc.scalar.activation(sq[:], xt[:], AF.Square, accum_out=sumsq[:])
                mu2 = sc.tile([P, 1], F32, tag="mu2", name="mu2")
                nc.scalar.activation(mu2[:], mus[:], AF.Square)
                vpe = sc.tile([P, 1], F32, tag="vpe", name="vpe")
                # vpe = sumsq/D - mu2 + eps  (two tiny fused scalar ops)
                nc.vector.tensor_scalar(out=vpe[:], in0=sumsq[:],
                                        scalar1=1.0 / D, scalar2=mu2[:],
                                        op0=ALU.mult, op1=ALU.subtract)
                std = sc.tile([P, 1], F32, tag="std", name="std")
                nc.scalar.activation(std[:], vpe[:], AF.Sqrt, bias=epsc[:])
                inv = sc.tile([P, 1], F32, tag="inv", name="inv")
                nc.vector.reciprocal(inv[:], std[:])
                nc.vector.tensor_scalar(out=dst_tiles[mt][:], in0=xt[:],
                                        scalar1=mus[:], scalar2=inv[:],
                                        op0=ALU.subtract, op1=ALU.mult)

        def transpose_apply(src_tiles, dst_tiles, g_sb, b_sb, tp):
            for k in range(KD):
                for mt in range(NT):
                    ps = tp.tile([P, P], F32, tag="tps", name="tps")
                    nc.tensor.transpose(ps[:], src_tiles[mt][:, k * P:(k + 1) * P],
                                        ident[:])
                    nc.vector.tensor_scalar(
                        out=dst_tiles[k][:, mt * P:(mt + 1) * P], in0=ps[:],
                        scalar1=g_sb[:, k:k + 1], scalar2=b_sb[:, k:k + 1],
                        op0=ALU.mult, op1=ALU.add)

        # ============ phase A: LN1 + transpose + AllGather ============
        with tc.tile_pool(name="xsP", bufs=1) as xsp:
            xs = [xsp.tile([P, D], F32, tag=f"xs{mt}", name=f"xs{mt}") for mt in range(NT)]
            for mt in range(NT):
                nc.sync.dma_start(xs[mt][:], t["x_d"][mt * P:(mt + 1) * P, :])

            with tc.tile_pool(name="lnA", bufs=1) as sc, \
                 tc.tile_pool(name="tpA", bufs=4, space="PSUM") as tp:
                h = [sc.tile([P, D], F32, tag=f"h{mt}", name=f"h{mt}") for mt in range(NT)]
                layernorm(xs, h, sc)
                hT = [sc.tile([P, TS], F32R, tag=f"hT{k}", name=f"hT{k}") for k in range(KD)]
                transpose_apply(h, hT, ln1g, ln1b, tp)
                for k in range(KD):
                    nc.sync.dma_start(t["ag1_in"][k * P:(k + 1) * P, :], hT[k][:])

            # ============ phases B+C: QKV + attention ============
            with tc.tile_pool(name="qkvP", bufs=1) as qkv:
                qT = [qkv.tile([P, S], F32R, tag=f"qT{m}", name=f"qT{m}") for m in range(2)]
                kT = [qkv.tile([P, S], F32R, tag=f"kT{m}", name=f"kT{m}") for m in range(2)]
                vo = [qkv.tile([P, HC, DH + 1], F32R, tag=f"vo{tm}", name=f"vo{tm}")
                      for tm in range(S // P)]

                wp_cm = tc.tile_pool(name="wqkv", bufs=1)
                wp = wp_cm.__enter__()
                # weight loads traced before the collective: DMA covers AG1
                wq = [wp.tile([P, CC], F32R, tag=f"wq{k}", name=f"wq{k}") for k in range(KD)]
                wk = [wp.tile([P, CC], F32R, tag=f"wk{k}", name=f"wk{k}") for k in range(KD)]
                wv = [wp.tile([P, CC], F32R, tag=f"wv{k}", name=f"wv{k}") for k in range(KD)]
                for k in range(KD):
                    nc.sync.dma_start(wq[k][:], t["wq_d"][k * P:(k + 1) * P, :])
                    nc.sync.dma_start(wk[k][:], t["wk_d"][k * P:(k + 1) * P, :])
                    nc.sync.dma_start(wv[k][:], t["wv_d"][k * P:(k + 1) * P, :])

                nc.gpsimd.collective_compute(
                    "AllGather", ALU.bypass, ins=[t["ag1_in"][:]],
                    outs=[t["ag1_out"][:]], replica_groups=GROUPS,
                )

                with tc.tile_pool(name="hTst", bufs=2) as st, \
                     tc.tile_pool(name="projPS", bufs=3, space="PSUM") as pps:
                    for qc in range(G):
                        hTq = [st.tile([P, TS], F32R, tag=f"hTq{k}", name=f"hTq{k}")
                               for k in range(KD)]
                        for k in range(KD):
                            nc.sync.dma_start(
                                hTq[k][:], t["ag1_out"][qc, k * P:(k + 1) * P, :])
                        for (w_sb, b_sb, out_sb) in ((wq, bq, qT), (wk, bk, kT)):
                            for m in range(2):
                                ps = pps.tile([P, TS], F32, tag="pps", name="pps")
                                nc.tensor.matmul(ps[:],
                                                 b_sb[:, m * P:(m + 1) * P],
                                                 ones_tok[:], start=True,
                                                 stop=False)
                                for k in range(KD):
                                    nc.tensor.matmul(
                                        ps[:], w_sb[k][:, m * P:(m + 1) * P],
                                        hTq[k][:], start=False,
                                        stop=(k == KD - 1))
                                nc.vector.tensor_copy(
                                    out_sb[m][:, qc * TS:(qc + 1) * TS], ps[:])
                        for mt in range(NT):
                            tm = qc * NT + mt
                            ps = pps.tile([P, CC], F32, tag="vps", name="vps")
                            nc.tensor.matmul(ps[:], ones128[:], bv[:],
                                             start=True, stop=False)
                            for k in range(KD):
                                nc.tensor.matmul(
                                    ps[:], hTq[k][:, mt * P:(mt + 1) * P],
                                    wv[k][:], start=False, stop=(k == KD - 1))
                            nc.vector.tensor_copy(
                                vo[tm][:, :, 0:DH],
                                ps[:].rearrange("p (h e) -> p h e", h=HC))
                            nc.vector.tensor_copy(vo[tm][:, :, DH:DH + 1],
                                                  onescol4[:])

                wp_cm.__exit__(None, None, None)

                # ---- attention ----
                with (
                    tc.tile_pool(name="scPS", bufs=2, space="PSUM") as scp,
                    tc.tile_pool(name="avPS", bufs=2, space="PSUM") as avp,
                    tc.tile_pool(name="attnSB", bufs=3) as asb,
                ):
                    for hp in range(HC // 2):      # head pairs at PE rows 0/64
                        for qc in range(G):
                            kt_max = 4 * qc + 3
                            avs = [avp.tile([DH + 1, TS], F32, tag=f"av{j}",
                                            name=f"av{j}") for j in range(2)]
                            for kt in range(kt_max + 1):
                                # both heads' score blocks into one 2-bank tile
                                sc_ps = scp.tile([P, 2, TS], F32,
                                                 tag="scp", name="scp")
                                for j in range(2):
                                    h_i = 2 * hp + j
                                    m = h_i // 2
                                    o = (h_i % 2) * DH
                                    nc.tensor.matmul(
                                        sc_ps[:, j, :],
                                        kT[m][o:o + DH, kt * P:(kt + 1) * P],
                                        qT[m][o:o + DH, qc * TS:(qc + 1) * TS],
                                        start=True, stop=True)
                                e_r = asb.tile([P, 2, TS], F32R,
                                               tag="erp", name="erp")
                                if kt < 4 * qc:
                                    nc.scalar.activation(
                                        e_r[:].rearrange("p a b -> p (a b)"),
                                        sc_ps[:].rearrange("p a b -> p (a b)"),
                                        AF.Exp, scale=0.125)
                                else:
                                    e_f = asb.tile([P, 2, TS], F32,
                                                   tag="efp", name="efp")
                                    nc.scalar.activation(
                                        e_f[:].rearrange("p a b -> p (a b)"),
                                        sc_ps[:].rearrange("p a b -> p (a b)"),
                                        AF.Exp, scale=0.125)
                                    sh = 512 * qc - 128 * kt
                                    nc.vector.tensor_tensor(
                                        out=e_r[:].rearrange("p a b -> p (a b)"),
                                        in0=e_f[:].rearrange("p a b -> p (a b)"),
                                        in1=maskd[sh][:],
                                        op=ALU.mult)
                                for j in range(2):
                                    h_i = 2 * hp + j
                                    nc.tensor.matmul(avs[j][:],
                                                     vo[kt][:, h_i, :],
                                                     e_r[:, j, :],
                                                     start=(kt == 0),
                                                     stop=(kt == kt_max))
                            for j in range(2):
                                h_i = 2 * hp + j
                                un = asb.tile([DH + 1, TS], F32,
                                              tag=f"un{j}", name=f"un{j}")
                                nc.vector.tensor_copy(un[:], avs[j][:])
                                rec = asb.tile([1, TS], F32,
                                               tag=f"rec{j}", name=f"rec{j}")
                                nc.vector.reciprocal(rec[:], un[DH:DH + 1, :])
                                rb = asb.tile([DH, TS], F32,
                                              tag=f"rb{j}", name=f"rb{j}")
                                nc.gpsimd.partition_broadcast(rb[:], rec[:])
                                chunk = asb.tile([DH, TS], F32R,
                                                 tag=f"chunk{j}", name=f"chunk{j}")
                                nc.vector.tensor_tensor(out=chunk[:],
                                                        in0=un[0:DH, :],
                                                        in1=rb[:], op=ALU.mult)
                                nc.sync.dma_start(
                                    t["a2a_in"][h_i * DH:(h_i + 1) * DH,
                                                qc * TS:(qc + 1) * TS],
                                    chunk[:])

            # prefetch pools traced before AG2 so DMA covers the collective
            pfw1_cm = tc.tile_pool(name="pfW1", bufs=1)
            pfw1 = pfw1_cm.__enter__()
            w1s0 = [pfw1.tile([P, 8 * P], F32R, tag=f"w1s{k}", name=f"w1s{k}")
                    for k in range(KD)]
            pfd_cm = tc.tile_pool(name="pfD", bufs=1)
            pfd = pfd_cm.__enter__()
            wo = [pfd.tile([P, D], F32R, tag=f"wo{k}", name=f"wo{k}") for k in range(KD)]
            for k in range(KD):
                nc.sync.dma_start(wo[k][:], t["wo_d"][k * P:(k + 1) * P, :])
                nc.sync.dma_start(w1s0[k][:], t["w1_d"][k * P:(k + 1) * P, 0:8 * P])

            nc.gpsimd.collective_compute(
                "AllGather", ALU.bypass, ins=[t["a2a_in"][:]],
                outs=[t["a2a_out"][:]], replica_groups=GROUPS,
            )

            # ============ phase D: out_proj + residual (in-place on xs) ====
            with tc.tile_pool(name="opPS", bufs=3, space="PSUM") as opp, \
                 tc.tile_pool(name="opSB", bufs=1) as osb:
                aT = [osb.tile([P, TS], F32R, tag=f"aT{k}", name=f"aT{k}") for k in range(KD)]
                off_sb = osb.tile([1, 1], mybir.dt.int32, tag="off", name="off")
                nc.sync.dma_start(off_sb[:], t["coff_d"][:])
                with nc.gpsimd.register("roff") as roff:
                    nc.gpsimd.reg_load(roff, off_sb[0:1, 0:1])
                    rv = nc.snap(roff)
                    for k in range(KD):
                        nc.gpsimd.dma_start(
                            aT[k][:],
                            t["a2a_out"][k // 2, (k % 2) * P:(k % 2 + 1) * P,
                                         bass.ds(rv, TS)])
                for mt in range(NT):
                    for n in range(2):
                        ps = opp.tile([P, TS], F32, tag="op", name="op")
                        nc.tensor.matmul(ps[:], ones128[:],
                                         bo[:, n * TS:(n + 1) * TS],
                                         start=True, stop=False)
                        for k in range(KD):
                            nc.tensor.matmul(
                                ps[:], aT[k][:, mt * P:(mt + 1) * P],
                                wo[k][:, n * TS:(n + 1) * TS],
                                start=False, stop=(k == KD - 1))
                        # residual written in place: xs becomes x2
                        nc.vector.tensor_tensor(
                            out=xs[mt][:, n * TS:(n + 1) * TS], in0=ps[:],
                            in1=xs[mt][:, n * TS:(n + 1) * TS], op=ALU.add)
            pfd_cm.__exit__(None, None, None)
            x2 = xs

            with tc.tile_pool(name="h2TP", bufs=1) as h2tp:
                h2T = [h2tp.tile([P, TS], F32R, tag=f"h2T{k}", name=f"h2T{k}")
                       for k in range(KD)]
                with tc.tile_pool(name="lnD", bufs=1) as sc, \
                     tc.tile_pool(name="tpD", bufs=4, space="PSUM") as tp:
                    h2 = [sc.tile([P, D], F32, tag=f"h2{mt}", name=f"h2{mt}")
                          for mt in range(NT)]
                    layernorm(x2, h2, sc)
                    transpose_apply(h2, h2T, ln2g, ln2b, tp)

                # ============ phase E: FFN ============
                with tc.tile_pool(name="gTP", bufs=1) as gtp:
                    gT = [gtp.tile([P, TS], F32R, tag=f"gT{mf}", name=f"gT{mf}")
                          for mf in range(KF)]
                    MFB = 8     # mf tiles per w1 stream block
                    with tc.tile_pool(name="w1st", bufs=1) as w1p, \
                         tc.tile_pool(name="gPS", bufs=4, space="PSUM") as gps:
                        for blk in range(KF // MFB):
                            if blk == 0:
                                w1s = w1s0
                            else:
                                w1s = [w1p.tile([P, MFB * P], F32R,
                                                tag=f"w1b{k}", name=f"w1b{k}")
                                       for k in range(KD)]
                                for k in range(KD):
                                    nc.sync.dma_start(
                                        w1s[k][:],
                                        t["w1_d"][k * P:(k + 1) * P,
                                                  blk * MFB * P:(blk + 1) * MFB * P])
                            b1s = w1p.tile([1, MFB * P], F32R,
                                           tag="b1s", name="b1s", bufs=2)
                            nc.sync.dma_start(
                                b1s[:],
                                t["b1_d"][None, blk * MFB * P:(blk + 1) * MFB * P])
                            for j in range(MFB):
                                mf = blk * MFB + j
                                ps = gps.tile([P, TS], F32, tag="g", name="g")
                                nc.tensor.matmul(
                                    ps[:], b1s[:, j * P:(j + 1) * P],
                                    ones_tok[:], start=True, stop=False)
                                for k in range(KD):
                                    nc.tensor.matmul(
                                        ps[:], w1s[k][:, j * P:(j + 1) * P],
                                        h2T[k][:], start=False,
                                        stop=(k == KD - 1))
                                nc.scalar.activation(gT[mf][:], ps[:],
                                                     AF.Gelu)

                    with tc.tile_pool(name="w2st", bufs=4) as w2p, \
                         tc.tile_pool(name="fPS", bufs=1, space="PSUM") as fps, \
                         tc.tile_pool(name="ySB", bufs=2) as ysb:
                        f_ps = [fps.tile([P, D], F32, tag=f"f{mt}", name=f"f{mt}")
                                for mt in range(NT)]
                        for mt in range(NT):
                            for n in range(2):
                                nc.tensor.matmul(
                                    f_ps[mt][:, n * TS:(n + 1) * TS],
                                    ones128[:], b2[:, n * TS:(n + 1) * TS],
                                    start=True, stop=False)
                        for k2 in range(KF):
                            w2t = w2p.tile([P, D], F32R, tag="w2", name="w2")
                            nc.sync.dma_start(
                                w2t[:], t["w2_d"][k2 * P:(k2 + 1) * P, :])
                            for mt in range(NT):
                                for n in range(2):
                                    nc.tensor.matmul(
                                        f_ps[mt][:, n * TS:(n + 1) * TS],
                                        gT[k2][:, mt * P:(mt + 1) * P],
                                        w2t[:, n * TS:(n + 1) * TS],
                                        start=False, stop=(k2 == KF - 1))
                        for mt in range(NT):
                            yt = ysb.tile([P, D], F32, tag="y", name="y")
                            nc.vector.tensor_tensor(out=yt[:],
                                                    in0=f_ps[mt][:],
                                                    in1=x2[mt][:],
                                                    op=ALU.add)
                            nc.sync.dma_start(
                                t["y_d"][mt * P:(mt + 1) * P, :], yt[:])
            pfw1_cm.__exit__(None, None, None)


def _in_maps(inputs):
    f32 = np.float32
    maps = []
    for c in range(NC):
        b, r = c // G, c % G
        c0 = r * CC
        m = {
            "x": np.ascontiguousarray(np.asarray(inputs["x"])[b, r * TS:(r + 1) * TS, :], f32),
            "ln1_g": np.ascontiguousarray(inputs["ln1_g"], f32),
            "ln1_b": np.ascontiguousarray(inputs["ln1_b"], f32),
            "Wq": np.ascontiguousarray(np.asarray(inputs["Wq"])[:, c0:c0 + CC], f32),
            "Wk": np.ascontiguousarray(np.asarray(inputs["Wk"])[:, c0:c0 + CC], f32),
            "Wv": np.ascontiguousarray(np.asarray(inputs["Wv"])[:, c0:c0 + CC], f32),
            "bq": np.ascontiguousarray(np.asarray(inputs["bq"])[c0:c0 + CC], f32),
            "bk": np.ascontiguousarray(np.asarray(inputs["bk"])[c0:c0 + CC], f32),
            "bv": np.ascontiguousarray(np.asarray(inputs["bv"])[c0:c0 + CC], f32),
            "Wo": np.ascontiguousarray(inputs["Wo"], f32),
            "bo": np.ascontiguousarray(inputs["bo"], f32),
            "ln2_g": np.ascontiguousarray(inputs["ln2_g"], f32),
            "ln2_b": np.ascontiguousarray(inputs["ln2_b"], f32),
            "W1": np.ascontiguousarray(inputs["W1"], f32),
            "b1": np.ascontiguousarray(inputs["b1"], f32),
            "W2": np.ascontiguousarray(inputs["W2"], f32),
            "b2": np.ascontiguousarray(inputs["b2"], f32),
            "coff": np.array([[r * TS]], dtype=np.int32),
        }
        maps.append(m)
    return maps


def _run(inputs, trace=False):
    if "nc" not in _CACHE:
        _CACHE["nc"] = build()
    nc = _CACHE["nc"]
    maps = _in_maps(inputs)
    res = run_bass_kernel_spmd(nc, maps, list(range(NC)), trace=trace)
    out = np.empty((B, S, D), np.float32)
    for c in range(NC):
        b, r = c // G, c % G
        out[b, r * TS:(r + 1) * TS, :] = res.results[c]["y"]
    return out, res


def kernel(**inputs):
    out, _ = _run(inputs, trace=False)
    return out


if __name__ == "__main__":
    build()
    print("build OK")


# revision 21
# speedup vs baseline: 43.7678x; 1.0123x over previous
"""Trainium2 Bass kernel for a dense transformer block (B=2,S=2048,D=1024,H=16,DFF=4096).

Sharding across 8 NeuronCores:
  core c: batch b=c//4, group rank r=c%4, replica groups [[0,1,2,3],[4,5,6,7]].
  - LN1 computed on own 512-token strip; hT AllGather'ed within the 4-core group.
  - Attention: head-parallel (4 heads/core, full causal sequence).
  - AllToAll redistributes attention output from head-sharded to token-sharded.
  - out_proj, LN2, FFN: token-sharded (512 tokens/core), full weights.
Matmuls run in float32r (TF32-like, full PE rate); the residual spine stays fp32.
"""
import sys

sys.path.insert(0, "/opt/trn_rl_repo")

import numpy as np

import concourse.bass as bass
import concourse.mybir as mybir
import concourse.tile as tile
from concourse import bacc
from concourse.bass_utils import run_bass_kernel_spmd
from concourse.masks import make_identity

AF = mybir.ActivationFunctionType
ALU = mybir.AluOpType
F32 = mybir.dt.float32
F32R = mybir.dt.float32r

B, S, D, H = 2, 2048, 1024, 16
DH = D // H          # 64
DFF = 4 * D          # 4096
EPS = 1e-5
NC = 8               # cores
G = 4                # cores per group (per batch)
TS = S // G          # 512 tokens per core
HC = H // G          # 4 heads per core
CC = HC * DH         # 256 head-columns per core
P = 128
KD = D // P          # 8 k-tiles over D
KF = DFF // P        # 32 k-tiles over DFF
NT = TS // P         # 4 token tiles per strip
GROUPS = [[0, 1, 2, 3], [4, 5, 6, 7]]

_CACHE = {}


def build():
    nc = bacc.Bacc(None)

    io = {}
    io["x_d"] = nc.declare_dram_parameter("x", [TS, D], F32, isOutput=False)
    io["ln1g_d"] = nc.declare_dram_parameter("ln1_g", [D], F32, isOutput=False)
    io["ln1b_d"] = nc.declare_dram_parameter("ln1_b", [D], F32, isOutput=False)
    io["wq_d"] = nc.declare_dram_parameter("Wq", [D, CC], F32R, isOutput=False)
    io["wk_d"] = nc.declare_dram_parameter("Wk", [D, CC], F32R, isOutput=False)
    io["wv_d"] = nc.declare_dram_parameter("Wv", [D, CC], F32R, isOutput=False)
    io["bq_d"] = nc.declare_dram_parameter("bq", [CC], F32R, isOutput=False)
    io["bk_d"] = nc.declare_dram_parameter("bk", [CC], F32R, isOutput=False)
    io["bv_d"] = nc.declare_dram_parameter("bv", [CC], F32R, isOutput=False)
    io["wo_d"] = nc.declare_dram_parameter("Wo", [D, D], F32R, isOutput=False)
    io["bo_d"] = nc.declare_dram_parameter("bo", [D], F32R, isOutput=False)
    io["ln2g_d"] = nc.declare_dram_parameter("ln2_g", [D], F32, isOutput=False)
    io["ln2b_d"] = nc.declare_dram_parameter("ln2_b", [D], F32, isOutput=False)
    io["w1_d"] = nc.declare_dram_parameter("W1", [D, DFF], F32R, isOutput=False)
    io["b1_d"] = nc.declare_dram_parameter("b1", [DFF], F32R, isOutput=False)
    io["w2_d"] = nc.declare_dram_parameter("W2", [DFF, D], F32R, isOutput=False)
    io["b2_d"] = nc.declare_dram_parameter("b2", [D], F32R, isOutput=False)
    io["y_d"] = nc.declare_dram_parameter("y", [TS, D], F32, isOutput=True)

    io["ag1_in"] = nc.dram_tensor("ag1_in", [D, TS], F32R)
    io["ag1_out"] = nc.dram_tensor("ag1_out", [G, D, TS], F32R)
    io["a2a_in"] = nc.dram_tensor("a2a_in", [CC, S], F32R)
    io["a2a_out"] = nc.dram_tensor("a2a_out", [G, CC, S], F32R)
    io["coff_d"] = nc.declare_dram_parameter("coff", [1, 1], mybir.dt.int32,
                                             isOutput=False)

    with tile.TileContext(nc) as tc:
        _body(nc, tc, io)
    nc.compile()
    return nc


def _body(nc, tc, t):
    with tc.tile_pool(name="const", bufs=1) as cst:
        # ---------------- constants ----------------
        ident = cst.tile([P, P], F32)
        make_identity(nc, ident[:])

        onesrow_f = cst.tile([1, TS], F32)
        nc.gpsimd.memset(onesrow_f[:], 1.0)
        ones128 = cst.tile([1, P], F32R)        # K=1 lhsT (M=128 tokens)
        nc.vector.tensor_copy(ones128[:], onesrow_f[0:1, 0:P])
        onescol4 = cst.tile([P, HC, 1], F32)
        nc.gpsimd.memset(onescol4[:], 1.0)
        epsc = cst.tile([P, 1], F32)
        nc.gpsimd.memset(epsc[:], EPS)

        # doubled causal masks (one per diagonal shift), mask||mask layout so a
        # single DVE op masks a two-head [128, 1024] pair tile.
        maskd = {}
        for sh in (0, -128, -256, -384):
            md = cst.tile([P, 2 * TS], F32, tag=f"maskd{sh}", name=f"maskd{sh}")
            nc.gpsimd.memset(md[:], 1.0)
            for half in range(2):
                nc.gpsimd.affine_select(
                    out=md[:, half * TS:(half + 1) * TS],
                    in_=md[:, half * TS:(half + 1) * TS],
                    compare_op=ALU.is_ge, fill=0.0, base=sh,
                    pattern=[[1, TS]], channel_multiplier=-1,
                )
            maskd[sh] = md

        # layernorm gains/biases as [128, KD] (per-partition per k-tile)
        ln1g = cst.tile([P, KD], F32)
        ln1b = cst.tile([P, KD], F32)
        ln2g = cst.tile([P, KD], F32)
        ln2b = cst.tile([P, KD], F32)
        nc.sync.dma_start(ln1g[:], t["ln1g_d"].rearrange("(k p) -> p k", p=P))
        nc.sync.dma_start(ln1b[:], t["ln1b_d"].rearrange("(k p) -> p k", p=P))
        nc.sync.dma_start(ln2g[:], t["ln2g_d"].rearrange("(k p) -> p k", p=P))
        nc.sync.dma_start(ln2b[:], t["ln2b_d"].rearrange("(k p) -> p k", p=P))

        # bq/bk as per-partition [128, 2] (column-tile-major) for psum eviction
        bqp = cst.tile([P, 2], F32)
        bkp = cst.tile([P, 2], F32)
        nc.gpsimd.dma_start(bqp[:], t["bq_d"].rearrange("(m p) -> p m", p=P))
        nc.gpsimd.dma_start(bkp[:], t["bk_d"].rearrange("(m p) -> p m", p=P))
        # bv broadcast across partitions for the v eviction add
        bvrow = cst.tile([1, CC], F32)
        nc.gpsimd.dma_start(bvrow[:], t["bv_d"][None, :])
        bvb = cst.tile([P, CC], F32)
        nc.gpsimd.partition_broadcast(bvb[:], bvrow[:])
        # b1 as per-partition [128, KF] for the gelu bias operand
        b1p = cst.tile([P, KF], F32)
        nc.gpsimd.dma_start(b1p[:], t["b1_d"].rearrange("(k p) -> p k", p=P))
        bo = cst.tile([1, D], F32R)
        b2 = cst.tile([1, D], F32R)
        nc.sync.dma_start(bo[:], t["bo_d"][None, :])
        nc.sync.dma_start(b2[:], t["b2_d"][None, :])

        # ---------------- helpers ----------------
        def layernorm(src_tiles, dst_tiles, sc):
            # var = E[x^2] - mu^2 (safe: |mu| << std for this data), so the
            # normalize is a single fused (x - mu) * inv DVE pass.
            for mt in range(NT):
                xt = src_tiles[mt]
                mu = sc.tile([P, 1], F32, tag="mu", name="mu")
                nc.vector.tensor_reduce(out=mu[:], in_=xt[:], op=ALU.add,
                                        axis=mybir.AxisListType.X)
                mus = sc.tile([P, 1], F32, tag="mus", name="mus")
                nc.scalar.mul(mus[:], mu[:], 1.0 / D)
                sq = sc.tile([P, D], F32, tag="sq", name="sq")
                sumsq = sc.tile([P, 1], F32, tag="sumsq", name="sumsq")
                nc.scalar.activation(sq[:], xt[:], AF.Square, accum_out=sumsq[:])
                mu2 = sc.tile([P, 1], F32, tag="mu2", name="mu2")
                nc.scalar.activation(mu2[:], mus[:], AF.Square)
                vpe = sc.tile([P, 1], F32, tag="vpe", name="vpe")
                # vpe = sumsq/D - mu2 + eps  (two tiny fused scalar ops)
                nc.vector.tensor_scalar(out=vpe[:], in0=sumsq[:],
                                        scalar1=1.0 / D, scalar2=mu2[:],
                                        op0=ALU.mult, op1=ALU.subtract)
                std = sc.tile([P, 1], F32, tag="std", name="std")
                nc.scalar.activation(std[:], vpe[:], AF.Sqrt, bias=epsc[:])
                inv = sc.tile([P, 1], F32, tag="inv", name="inv")
                nc.vector.reciprocal(inv[:], std[:])
                nc.vector.tensor_scalar(out=dst_tiles[mt][:], in0=xt[:],
                                        scalar1=mus[:], scalar2=inv[:],
                                        op0=ALU.subtract, op1=ALU.mult)

        def transpose_apply(src_tiles, dst_tiles, g_sb, b_sb, tp):
            for k in range(KD):
                for mt in range(NT):
                    ps = tp.tile([P, P], F32, tag="tps", name="tps")
                    nc.tensor.transpose(ps[:], src_tiles[mt][:, k * P:(k + 1) * P],
                                        ident[:])
                    nc.vector.tensor_scalar(
                        out=dst_tiles[k][:, mt * P:(mt + 1) * P], in0=ps[:],
                        scalar1=g_sb[:, k:k + 1], scalar2=b_sb[:, k:k + 1],
                        op0=ALU.mult, op1=ALU.add)

        # ============ phase A: LN1 + transpose + AllGather ============
        with tc.tile_pool(name="xsP", bufs=1) as xsp:
            xs = [xsp.tile([P, D], F32, tag=f"xs{mt}", name=f"xs{mt}") for mt in range(NT)]
            for mt in range(NT):
                nc.sync.dma_start(xs[mt][:], t["x_d"][mt * P:(mt + 1) * P, :])

            with tc.tile_pool(name="lnA", bufs=1) as sc, \
                 tc.tile_pool(name="tpA", bufs=4, space="PSUM") as tp:
                h = [sc.tile([P, D], F32, tag=f"h{mt}", name=f"h{mt}") for mt in range(NT)]
                layernorm(xs, h, sc)
                hT = [sc.tile([P, TS], F32R, tag=f"hT{k}", name=f"hT{k}") for k in range(KD)]
                transpose_apply(h, hT, ln1g, ln1b, tp)
                for k in range(KD):
                    nc.sync.dma_start(t["ag1_in"][k * P:(k + 1) * P, :], hT[k][:])

            # ============ phases B+C: QKV + attention ============
            with tc.tile_pool(name="qkvP", bufs=1) as qkv:
                qT = [qkv.tile([P, S], F32R, tag=f"qT{m}", name=f"qT{m}") for m in range(2)]
                kT = [qkv.tile([P, S], F32R, tag=f"kT{m}", name=f"kT{m}") for m in range(2)]
                vo = [qkv.tile([P, HC, DH + 1], F32R, tag=f"vo{tm}", name=f"vo{tm}")
                      for tm in range(S // P)]

                wp_cm = tc.tile_pool(name="wqkv", bufs=1)
                wp = wp_cm.__enter__()
                # weight loads traced before the collective: DMA covers AG1
                wq = [wp.tile([P, CC], F32R, tag=f"wq{k}", name=f"wq{k}") for k in range(KD)]
                wk = [wp.tile([P, CC], F32R, tag=f"wk{k}", name=f"wk{k}") for k in range(KD)]
                wv = [wp.tile([P, CC], F32R, tag=f"wv{k}", name=f"wv{k}") for k in range(KD)]
                for k in range(KD):
                    nc.sync.dma_start(wq[k][:], t["wq_d"][k * P:(k + 1) * P, :])
                    nc.sync.dma_start(wk[k][:], t["wk_d"][k * P:(k + 1) * P, :])
                    nc.sync.dma_start(wv[k][:], t["wv_d"][k * P:(k + 1) * P, :])

                nc.gpsimd.collective_compute(
                    "AllGather", ALU.bypass, ins=[t["ag1_in"][:]],
                    outs=[t["ag1_out"][:]], replica_groups=GROUPS,
                )

                with tc.tile_pool(name="hTst", bufs=2) as st, \
                     tc.tile_pool(name="projPS", bufs=3, space="PSUM") as pps:
                    for qc in range(G):
                        hTq = [st.tile([P, TS], F32R, tag=f"hTq{k}", name=f"hTq{k}")
                               for k in range(KD)]
                        for k in range(KD):
                            nc.sync.dma_start(
                                hTq[k][:], t["ag1_out"][qc, k * P:(k + 1) * P, :])
                        for (w_sb, b_sb, out_sb) in ((wq, bqp, qT), (wk, bkp, kT)):
                            for m in range(2):
                                ps = pps.tile([P, TS], F32, tag="pps", name="pps")
                                for k in range(KD):
                                    nc.tensor.matmul(
                                        ps[:], w_sb[k][:, m * P:(m + 1) * P],
                                        hTq[k][:], start=(k == 0),
                                        stop=(k == KD - 1))
                                nc.vector.tensor_scalar(
                                    out=out_sb[m][:, qc * TS:(qc + 1) * TS],
                                    in0=ps[:], scalar1=b_sb[:, m:m + 1],
                                    scalar2=None, op0=ALU.add)
                        for mt in range(NT):
                            tm = qc * NT + mt
                            ps = pps.tile([P, CC], F32, tag="vps", name="vps")
                            for k in range(KD):
                                nc.tensor.matmul(
                                    ps[:], hTq[k][:, mt * P:(mt + 1) * P],
                                    wv[k][:], start=(k == 0), stop=(k == KD - 1))
                            nc.vector.tensor_tensor(
                                out=vo[tm][:, :, 0:DH],
                                in0=ps[:].rearrange("p (h e) -> p h e", h=HC),
                                in1=bvb[:].rearrange("p (h e) -> p h e", h=HC),
                                op=ALU.add)
                            nc.vector.tensor_copy(vo[tm][:, :, DH:DH + 1],
                                                  onescol4[:])

                wp_cm.__exit__(None, None, None)

                # ---- attention ----
                with (
                    tc.tile_pool(name="scPS", bufs=2, space="PSUM") as scp,
                    tc.tile_pool(name="avPS", bufs=2, space="PSUM") as avp,
                    tc.tile_pool(name="attnSB", bufs=3) as asb,
                ):
                    for hp in range(HC // 2):      # head pairs at PE rows 0/64
                        for qc in range(G):
                            kt_max = 4 * qc + 3
                            avs = [avp.tile([DH + 1, TS], F32, tag=f"av{j}",
                                            name=f"av{j}") for j in range(2)]
                            for kt in range(kt_max + 1):
                                # both heads' score blocks into one 2-bank tile
                                sc_ps = scp.tile([P, 2, TS], F32,
                                                 tag="scp", name="scp")
                                for j in range(2):
                                    h_i = 2 * hp + j
                                    m = h_i // 2
                                    o = (h_i % 2) * DH
                                    nc.tensor.matmul(
                                        sc_ps[:, j, :],
                                        kT[m][o:o + DH, kt * P:(kt + 1) * P],
                                        qT[m][o:o + DH, qc * TS:(qc + 1) * TS],
                                        start=True, stop=True)
                                e_r = asb.tile([P, 2, TS], F32R,
                                               tag="erp", name="erp")
                                if kt < 4 * qc:
                                    nc.scalar.activation(
                                        e_r[:].rearrange("p a b -> p (a b)"),
                                        sc_ps[:].rearrange("p a b -> p (a b)"),
                                        AF.Exp, scale=0.125)
                                else:
                                    e_f = asb.tile([P, 2, TS], F32,
                                                   tag="efp", name="efp")
                                    nc.scalar.activation(
                                        e_f[:].rearrange("p a b -> p (a b)"),
                                        sc_ps[:].rearrange("p a b -> p (a b)"),
                                        AF.Exp, scale=0.125)
                                    sh = 512 * qc - 128 * kt
                                    nc.vector.tensor_tensor(
                                        out=e_r[:].rearrange("p a b -> p (a b)"),
                                        in0=e_f[:].rearrange("p a b -> p (a b)"),
                                        in1=maskd[sh][:],
                                        op=ALU.mult)
                                for j in range(2):
                                    h_i = 2 * hp + j
                                    nc.tensor.matmul(avs[j][:],
                                                     vo[kt][:, h_i, :],
                                                     e_r[:, j, :],
                                                     start=(kt == 0),
                                                     stop=(kt == kt_max))
                            for j in range(2):
                                h_i = 2 * hp + j
                                un = asb.tile([DH + 1, TS], F32,
                                              tag=f"un{j}", name=f"un{j}")
                                nc.vector.tensor_copy(un[:], avs[j][:])
                                rec = asb.tile([1, TS], F32,
                                               tag=f"rec{j}", name=f"rec{j}")
                                nc.vector.reciprocal(rec[:], un[DH:DH + 1, :])
                                rb = asb.tile([DH, TS], F32,
                                              tag=f"rb{j}", name=f"rb{j}")
                                nc.gpsimd.partition_broadcast(rb[:], rec[:])
                                chunk = asb.tile([DH, TS], F32R,
                                                 tag=f"chunk{j}", name=f"chunk{j}")
                                nc.vector.tensor_tensor(out=chunk[:],
                                                        in0=un[0:DH, :],
                                                        in1=rb[:], op=ALU.mult)
                                nc.sync.dma_start(
                                    t["a2a_in"][h_i * DH:(h_i + 1) * DH,
                                                qc * TS:(qc + 1) * TS],
                                    chunk[:])

            # prefetch pools traced before AG2 so DMA covers the collective
            pfw1_cm = tc.tile_pool(name="pfW1", bufs=1)
            pfw1 = pfw1_cm.__enter__()
            w1s0 = [pfw1.tile([P, 8 * P], F32R, tag=f"w1s{k}", name=f"w1s{k}")
                    for k in range(KD)]
            pfd_cm = tc.tile_pool(name="pfD", bufs=1)
            pfd = pfd_cm.__enter__()
            wo = [pfd.tile([P, D], F32R, tag=f"wo{k}", name=f"wo{k}") for k in range(KD)]
            for k in range(KD):
                nc.sync.dma_start(wo[k][:], t["wo_d"][k * P:(k + 1) * P, :])
                nc.sync.dma_start(w1s0[k][:], t["w1_d"][k * P:(k + 1) * P, 0:8 * P])

            nc.gpsimd.collective_compute(
                "AllGather", ALU.bypass, ins=[t["a2a_in"][:]],
                outs=[t["a2a_out"][:]], replica_groups=GROUPS,
            )

            # ============ phase D: out_proj + residual (in-place on xs) ====
            with tc.tile_pool(name="opPS", bufs=3, space="PSUM") as opp, \
                 tc.tile_pool(name="opSB", bufs=1) as osb:
                aT = [osb.tile([P, TS], F32R, tag=f"aT{k}", name=f"aT{k}") for k in range(KD)]
                off_sb = osb.tile([1, 1], mybir.dt.int32, tag="off", name="off")
                nc.sync.dma_start(off_sb[:], t["coff_d"][:])
                with nc.gpsimd.register("roff") as roff:
                    nc.gpsimd.reg_load(roff, off_sb[0:1, 0:1])
                    rv = nc.snap(roff)
                    for k in range(KD):
                        nc.gpsimd.dma_start(
                            aT[k][:],
                            t["a2a_out"][k // 2, (k % 2) * P:(k % 2 + 1) * P,
                                         bass.ds(rv, TS)])
                for mt in range(NT):
                    for n in range(2):
                        ps = opp.tile([P, TS], F32, tag="op", name="op")
                        nc.tensor.matmul(ps[:], ones128[:],
                                         bo[:, n * TS:(n + 1) * TS],
                                         start=True, stop=False)
                        for k in range(KD):
                            nc.tensor.matmul(
                                ps[:], aT[k][:, mt * P:(mt + 1) * P],
                                wo[k][:, n * TS:(n + 1) * TS],
                                start=False, stop=(k == KD - 1))
                        # residual written in place: xs becomes x2
                        nc.vector.tensor_tensor(
                            out=xs[mt][:, n * TS:(n + 1) * TS], in0=ps[:],
                            in1=xs[mt][:, n * TS:(n + 1) * TS], op=ALU.add)
            pfd_cm.__exit__(None, None, None)
            x2 = xs

            with tc.tile_pool(name="h2TP", bufs=1) as h2tp:
                h2T = [h2tp.tile([P, TS], F32R, tag=f"h2T{k}", name=f"h2T{k}")
                       for k in range(KD)]
                with tc.tile_pool(name="lnD", bufs=1) as sc, \
                     tc.tile_pool(name="tpD", bufs=4, space="PSUM") as tp:
                    h2 = [sc.tile([P, D], F32, tag=f"h2{mt}", name=f"h2{mt}")
                          for mt in range(NT)]
                    layernorm(x2, h2, sc)
                    transpose_apply(h2, h2T, ln2g, ln2b, tp)

                # ============ phase E: FFN ============
                with tc.tile_pool(name="gTP", bufs=1) as gtp:
                    gT = [gtp.tile([P, TS], F32R, tag=f"gT{mf}", name=f"gT{mf}")
                          for mf in range(KF)]
                    MFB = 8     # mf tiles per w1 stream block
                    with tc.tile_pool(name="w1st", bufs=1) as w1p, \
                         tc.tile_pool(name="gPS", bufs=4, space="PSUM") as gps:
                        for blk in range(KF // MFB):
                            if blk == 0:
                                w1s = w1s0
                            else:
                                w1s = [w1p.tile([P, MFB * P], F32R,
                                                tag=f"w1b{k}", name=f"w1b{k}")
                                       for k in range(KD)]
                                for k in range(KD):
                                    nc.sync.dma_start(
                                        w1s[k][:],
                                        t["w1_d"][k * P:(k + 1) * P,
                                                  blk * MFB * P:(blk + 1) * MFB * P])
                            for j in range(MFB):
                                mf = blk * MFB + j
                                ps = gps.tile([P, TS], F32, tag="g", name="g")
                                for k in range(KD):
                                    nc.tensor.matmul(
                                        ps[:], w1s[k][:, j * P:(j + 1) * P],
                                        h2T[k][:], start=(k == 0),
                                        stop=(k == KD - 1))
                                nc.scalar.activation(gT[mf][:], ps[:],
                                                     AF.Gelu,
                                                     bias=b1p[:, mf:mf + 1])

                    with tc.tile_pool(name="w2st", bufs=4) as w2p, \
                         tc.tile_pool(name="fPS", bufs=1, space="PSUM") as fps, \
                         tc.tile_pool(name="ySB", bufs=2) as ysb:
                        f_ps = [fps.tile([P, D], F32, tag=f"f{mt}", name=f"f{mt}")
                                for mt in range(NT)]
                        for mt in range(NT):
                            for n in range(2):
                                nc.tensor.matmul(
                                    f_ps[mt][:, n * TS:(n + 1) * TS],
                                    ones128[:], b2[:, n * TS:(n + 1) * TS],
                                    start=True, stop=False)
                        for k2 in range(KF):
                            w2t = w2p.tile([P, D], F32R, tag="w2", name="w2")
                            nc.sync.dma_start(
                                w2t[:], t["w2_d"][k2 * P:(k2 + 1) * P, :])
                            for mt in range(NT):
                                for n in range(2):
                                    nc.tensor.matmul(
                                        f_ps[mt][:, n * TS:(n + 1) * TS],
                                        gT[k2][:, mt * P:(mt + 1) * P],
                                        w2t[:, n * TS:(n + 1) * TS],
                                        start=False, stop=(k2 == KF - 1))
                        for mt in range(NT):
                            yt = ysb.tile([P, D], F32, tag="y", name="y")
                            nc.vector.tensor_tensor(out=yt[:],
                                                    in0=f_ps[mt][:],
                                                    in1=x2[mt][:],
                                                    op=ALU.add)
                            nc.sync.dma_start(
                                t["y_d"][mt * P:(mt + 1) * P, :], yt[:])
            pfw1_cm.__exit__(None, None, None)


def _in_maps(inputs):
    f32 = np.float32
    maps = []
    for c in range(NC):
        b, r = c // G, c % G
        c0 = r * CC
        m = {
            "x": np.ascontiguousarray(np.asarray(inputs["x"])[b, r * TS:(r + 1) * TS, :], f32),
            "ln1_g": np.ascontiguousarray(inputs["ln1_g"], f32),
            "ln1_b": np.ascontiguousarray(inputs["ln1_b"], f32),
            "Wq": np.ascontiguousarray(np.asarray(inputs["Wq"])[:, c0:c0 + CC], f32),
            "Wk": np.ascontiguousarray(np.asarray(inputs["Wk"])[:, c0:c0 + CC], f32),
            "Wv": np.ascontiguousarray(np.asarray(inputs["Wv"])[:, c0:c0 + CC], f32),
            "bq": np.ascontiguousarray(np.asarray(inputs["bq"])[c0:c0 + CC], f32),
            "bk": np.ascontiguousarray(np.asarray(inputs["bk"])[c0:c0 + CC], f32),
            "bv": np.ascontiguousarray(np.asarray(inputs["bv"])[c0:c0 + CC], f32),
            "Wo": np.ascontiguousarray(inputs["Wo"], f32),
            "bo": np.ascontiguousarray(inputs["bo"], f32),
            "ln2_g": np.ascontiguousarray(inputs["ln2_g"], f32),
            "ln2_b": np.ascontiguousarray(inputs["ln2_b"], f32),
            "W1": np.ascontiguousarray(inputs["W1"], f32),
            "b1": np.ascontiguousarray(inputs["b1"], f32),
            "W2": np.ascontiguousarray(inputs["W2"], f32),
            "b2": np.ascontiguousarray(inputs["b2"], f32),
            "coff": np.array([[r * TS]], dtype=np.int32),
        }
        maps.append(m)
    return maps


def _run(inputs, trace=False):
    if "nc" not in _CACHE:
        _CACHE["nc"] = build()
    nc = _CACHE["nc"]
    maps = _in_maps(inputs)
    res = run_bass_kernel_spmd(nc, maps, list(range(NC)), trace=trace)
    out = np.empty((B, S, D), np.float32)
    for c in range(NC):
        b, r = c // G, c % G
        out[b, r * TS:(r + 1) * TS, :] = res.results[c]["y"]
    return out, res


def kernel(**inputs):
    out, _ = _run(inputs, trace=False)
    return out


if __name__ == "__main__":
    build()
    print("build OK")


# revision 23
# speedup vs baseline: 44.2426x; 1.0108x over previous
"""Trainium2 Bass kernel for a dense transformer block (B=2,S=2048,D=1024,H=16,DFF=4096).

Sharding across 8 NeuronCores:
  core c: batch b=c//4, group rank r=c%4, replica groups [[0,1,2,3],[4,5,6,7]].
  - LN1 computed on own 512-token strip; hT AllGather'ed within the 4-core group.
  - Attention: head-parallel (4 heads/core, full causal sequence).
  - AllToAll redistributes attention output from head-sharded to token-sharded.
  - out_proj, LN2, FFN: token-sharded (512 tokens/core), full weights.
Matmuls run in float32r (TF32-like, full PE rate); the residual spine stays fp32.
"""
import sys

sys.path.insert(0, "/opt/trn_rl_repo")

import numpy as np

import concourse.bass as bass
import concourse.mybir as mybir
import concourse.tile as tile
from concourse import bacc
from concourse.bass_utils import run_bass_kernel_spmd
from concourse.masks import make_identity

AF = mybir.ActivationFunctionType
ALU = mybir.AluOpType
F32 = mybir.dt.float32
F32R = mybir.dt.float32r

B, S, D, H = 2, 2048, 1024, 16
DH = D // H          # 64
DFF = 4 * D          # 4096
EPS = 1e-5
NC = 8               # cores
G = 4                # cores per group (per batch)
TS = S // G          # 512 tokens per core
HC = H // G          # 4 heads per core
CC = HC * DH         # 256 head-columns per core
P = 128
KD = D // P          # 8 k-tiles over D
KF = DFF // P        # 32 k-tiles over DFF
NT = TS // P         # 4 token tiles per strip
GROUPS = [[0, 1, 2, 3], [4, 5, 6, 7]]

_CACHE = {}


def build():
    nc = bacc.Bacc(None)

    io = {}
    io["x_d"] = nc.declare_dram_parameter("x", [TS, D], F32, isOutput=False)
    io["ln1g_d"] = nc.declare_dram_parameter("ln1_g", [D], F32, isOutput=False)
    io["ln1b_d"] = nc.declare_dram_parameter("ln1_b", [D], F32, isOutput=False)
    io["wq_d"] = nc.declare_dram_parameter("Wq", [D, CC], F32R, isOutput=False)
    io["wk_d"] = nc.declare_dram_parameter("Wk", [D, CC], F32R, isOutput=False)
    io["wv_d"] = nc.declare_dram_parameter("Wv", [D, CC], F32R, isOutput=False)
    io["bq_d"] = nc.declare_dram_parameter("bq", [CC], F32R, isOutput=False)
    io["bk_d"] = nc.declare_dram_parameter("bk", [CC], F32R, isOutput=False)
    io["bv_d"] = nc.declare_dram_parameter("bv", [CC], F32R, isOutput=False)
    io["wo_d"] = nc.declare_dram_parameter("Wo", [D, D], F32R, isOutput=False)
    io["bo_d"] = nc.declare_dram_parameter("bo", [D], F32R, isOutput=False)
    io["ln2g_d"] = nc.declare_dram_parameter("ln2_g", [D], F32, isOutput=False)
    io["ln2b_d"] = nc.declare_dram_parameter("ln2_b", [D], F32, isOutput=False)
    io["w1_d"] = nc.declare_dram_parameter("W1", [D, DFF], F32R, isOutput=False)
    io["b1_d"] = nc.declare_dram_parameter("b1", [DFF], F32R, isOutput=False)
    io["w2_d"] = nc.declare_dram_parameter("W2", [DFF, D], F32R, isOutput=False)
    io["b2_d"] = nc.declare_dram_parameter("b2", [D], F32R, isOutput=False)
    io["y_d"] = nc.declare_dram_parameter("y", [TS, D], F32, isOutput=True)

    io["ag1_in"] = nc.dram_tensor("ag1_in", [D, TS], F32R)
    io["ag1_out"] = nc.dram_tensor("ag1_out", [G, D, TS], F32R)
    io["a2a_in"] = nc.dram_tensor("a2a_in", [CC, S], F32R)
    io["a2a_out"] = nc.dram_tensor("a2a_out", [G, CC, S], F32R)
    io["coff_d"] = nc.declare_dram_parameter("coff", [1, 1], mybir.dt.int32,
                                             isOutput=False)

    with tile.TileContext(nc) as tc:
        _body(nc, tc, io)
    nc.compile()
    return nc


def _body(nc, tc, t):
    with tc.tile_pool(name="const", bufs=1) as cst:
        # ---------------- constants ----------------
        ident = cst.tile([P, P], F32)
        make_identity(nc, ident[:])

        onesrow_f = cst.tile([1, TS], F32)
        nc.gpsimd.memset(onesrow_f[:], 1.0)
        ones128 = cst.tile([1, P], F32R)        # K=1 lhsT (M=128 tokens)
        nc.vector.tensor_copy(ones128[:], onesrow_f[0:1, 0:P])
        onescol4 = cst.tile([P, HC, 1], F32)
        nc.gpsimd.memset(onescol4[:], 1.0)
        epsc = cst.tile([P, 1], F32)
        nc.gpsimd.memset(epsc[:], EPS)

        # doubled causal masks (one per diagonal shift), mask||mask layout so a
        # single DVE op masks a two-head [128, 1024] pair tile.
        maskd = {}
        for sh in (0, -128, -256, -384):
            md = cst.tile([P, 2 * TS], F32, tag=f"maskd{sh}", name=f"maskd{sh}")
            nc.gpsimd.memset(md[:], 1.0)
            for half in range(2):
                nc.gpsimd.affine_select(
                    out=md[:, half * TS:(half + 1) * TS],
                    in_=md[:, half * TS:(half + 1) * TS],
                    compare_op=ALU.is_ge, fill=0.0, base=sh,
                    pattern=[[1, TS]], channel_multiplier=-1,
                )
            maskd[sh] = md

        # layernorm gains/biases as [128, KD] (per-partition per k-tile)
        ln1g = cst.tile([P, KD], F32)
        ln1b = cst.tile([P, KD], F32)
        ln2g = cst.tile([P, KD], F32)
        ln2b = cst.tile([P, KD], F32)
        nc.sync.dma_start(ln1g[:], t["ln1g_d"].rearrange("(k p) -> p k", p=P))
        nc.sync.dma_start(ln1b[:], t["ln1b_d"].rearrange("(k p) -> p k", p=P))
        nc.sync.dma_start(ln2g[:], t["ln2g_d"].rearrange("(k p) -> p k", p=P))
        nc.sync.dma_start(ln2b[:], t["ln2b_d"].rearrange("(k p) -> p k", p=P))

        # bq/bk as per-partition [128, 2] (column-tile-major) for psum eviction
        bqp = cst.tile([P, 2], F32)
        bkp = cst.tile([P, 2], F32)
        nc.gpsimd.dma_start(bqp[:], t["bq_d"].rearrange("(m p) -> p m", p=P))
        nc.gpsimd.dma_start(bkp[:], t["bk_d"].rearrange("(m p) -> p m", p=P))
        # bv broadcast across partitions for the v eviction add
        bvrow = cst.tile([1, CC], F32)
        nc.gpsimd.dma_start(bvrow[:], t["bv_d"][None, :])
        bvb = cst.tile([P, CC], F32)
        nc.gpsimd.partition_broadcast(bvb[:], bvrow[:])
        # b1 as per-partition [128, KF] for the gelu bias operand
        b1p = cst.tile([P, KF], F32)
        nc.gpsimd.dma_start(b1p[:], t["b1_d"].rearrange("(k p) -> p k", p=P))
        bo = cst.tile([1, D], F32R)
        b2 = cst.tile([1, D], F32R)
        nc.sync.dma_start(bo[:], t["bo_d"][None, :])
        nc.sync.dma_start(b2[:], t["b2_d"][None, :])

        # ---------------- helpers ----------------
        def layernorm(src_tiles, dst_tiles, sc):
            # var = E[x^2] - mu^2 (safe: |mu| << std for this data), so the
            # normalize is a single fused (x - mu) * inv DVE pass.
            for mt in range(NT):
                xt = src_tiles[mt]
                mu = sc.tile([P, 1], F32, tag="mu", name="mu")
                nc.vector.tensor_reduce(out=mu[:], in_=xt[:], op=ALU.add,
                                        axis=mybir.AxisListType.X)
                mus = sc.tile([P, 1], F32, tag="mus", name="mus")
                nc.scalar.mul(mus[:], mu[:], 1.0 / D)
                sq = sc.tile([P, D], F32, tag="sq", name="sq")
                sumsq = sc.tile([P, 1], F32, tag="sumsq", name="sumsq")
                nc.scalar.activation(sq[:], xt[:], AF.Square, accum_out=sumsq[:])
                mu2 = sc.tile([P, 1], F32, tag="mu2", name="mu2")
                nc.scalar.activation(mu2[:], mus[:], AF.Square)
                vpe = sc.tile([P, 1], F32, tag="vpe", name="vpe")
                # vpe = sumsq/D - mu2 + eps  (two tiny fused scalar ops)
                nc.vector.tensor_scalar(out=vpe[:], in0=sumsq[:],
                                        scalar1=1.0 / D, scalar2=mu2[:],
                                        op0=ALU.mult, op1=ALU.subtract)
                std = sc.tile([P, 1], F32, tag="std", name="std")
                nc.scalar.activation(std[:], vpe[:], AF.Sqrt, bias=epsc[:])
                inv = sc.tile([P, 1], F32, tag="inv", name="inv")
                nc.vector.reciprocal(inv[:], std[:])
                nc.vector.tensor_scalar(out=dst_tiles[mt][:], in0=xt[:],
                                        scalar1=mus[:], scalar2=inv[:],
                                        op0=ALU.subtract, op1=ALU.mult)

        def transpose_apply(src_tiles, dst_tiles, g_sb, b_sb, tp):
            for k in range(KD):
                for mt in range(NT):
                    ps = tp.tile([P, P], F32, tag="tps", name="tps")
                    nc.tensor.transpose(ps[:], src_tiles[mt][:, k * P:(k + 1) * P],
                                        ident[:])
                    nc.vector.tensor_scalar(
                        out=dst_tiles[k][:, mt * P:(mt + 1) * P], in0=ps[:],
                        scalar1=g_sb[:, k:k + 1], scalar2=b_sb[:, k:k + 1],
                        op0=ALU.mult, op1=ALU.add)

        # ============ phase A: LN1 + transpose + AllGather ============
        with tc.tile_pool(name="xsP", bufs=1) as xsp:
            xs = [xsp.tile([P, D], F32, tag=f"xs{mt}", name=f"xs{mt}") for mt in range(NT)]
            for mt in range(NT):
                nc.sync.dma_start(xs[mt][:], t["x_d"][mt * P:(mt + 1) * P, :])

            with tc.tile_pool(name="lnA", bufs=1) as sc, \
                 tc.tile_pool(name="tpA", bufs=4, space="PSUM") as tp:
                h = [sc.tile([P, D], F32, tag=f"h{mt}", name=f"h{mt}") for mt in range(NT)]
                layernorm(xs, h, sc)
                hT = [sc.tile([P, TS], F32R, tag=f"hT{k}", name=f"hT{k}") for k in range(KD)]
                transpose_apply(h, hT, ln1g, ln1b, tp)
                for k in range(KD):
                    nc.sync.dma_start(t["ag1_in"][k * P:(k + 1) * P, :], hT[k][:])

            # ============ phases B+C: QKV + attention ============
            with tc.tile_pool(name="qkvP", bufs=1) as qkv:
                qT = [qkv.tile([P, S], F32R, tag=f"qT{m}", name=f"qT{m}") for m in range(2)]
                kT = [qkv.tile([P, S], F32R, tag=f"kT{m}", name=f"kT{m}") for m in range(2)]
                vo = [qkv.tile([P, HC, DH + 1], F32R, tag=f"vo{tm}", name=f"vo{tm}")
                      for tm in range(S // P)]

                wp_cm = tc.tile_pool(name="wqkv", bufs=1)
                wp = wp_cm.__enter__()
                # weight loads traced before the collective: DMA covers AG1
                wq = [wp.tile([P, CC], F32R, tag=f"wq{k}", name=f"wq{k}") for k in range(KD)]
                wk = [wp.tile([P, CC], F32R, tag=f"wk{k}", name=f"wk{k}") for k in range(KD)]
                wv = [wp.tile([P, CC], F32R, tag=f"wv{k}", name=f"wv{k}") for k in range(KD)]
                for k in range(KD):
                    nc.sync.dma_start(wq[k][:], t["wq_d"][k * P:(k + 1) * P, :])
                    nc.sync.dma_start(wk[k][:], t["wk_d"][k * P:(k + 1) * P, :])
                    nc.sync.dma_start(wv[k][:], t["wv_d"][k * P:(k + 1) * P, :])

                nc.gpsimd.collective_compute(
                    "AllGather", ALU.bypass, ins=[t["ag1_in"][:]],
                    outs=[t["ag1_out"][:]], replica_groups=GROUPS,
                )

                with tc.tile_pool(name="hTst", bufs=2) as st, \
                     tc.tile_pool(name="projPS", bufs=3, space="PSUM") as pps:
                    for qc in range(G):
                        hTq = [st.tile([P, TS], F32R, tag=f"hTq{k}", name=f"hTq{k}")
                               for k in range(KD)]
                        for k in range(KD):
                            nc.sync.dma_start(
                                hTq[k][:], t["ag1_out"][qc, k * P:(k + 1) * P, :])
                        for (w_sb, b_sb, out_sb) in ((wq, bqp, qT), (wk, bkp, kT)):
                            for m in range(2):
                                ps = pps.tile([P, TS], F32, tag="pps", name="pps")
                                for k in range(KD):
                                    nc.tensor.matmul(
                                        ps[:], w_sb[k][:, m * P:(m + 1) * P],
                                        hTq[k][:], start=(k == 0),
                                        stop=(k == KD - 1))
                                nc.vector.tensor_scalar(
                                    out=out_sb[m][:, qc * TS:(qc + 1) * TS],
                                    in0=ps[:], scalar1=b_sb[:, m:m + 1],
                                    scalar2=None, op0=ALU.add)
                        for mt in range(NT):
                            tm = qc * NT + mt
                            ps = pps.tile([P, CC], F32, tag="vps", name="vps")
                            for k in range(KD):
                                nc.tensor.matmul(
                                    ps[:], hTq[k][:, mt * P:(mt + 1) * P],
                                    wv[k][:], start=(k == 0), stop=(k == KD - 1))
                            nc.vector.tensor_tensor(
                                out=vo[tm][:, :, 0:DH],
                                in0=ps[:].rearrange("p (h e) -> p h e", h=HC),
                                in1=bvb[:].rearrange("p (h e) -> p h e", h=HC),
                                op=ALU.add)
                            nc.vector.tensor_copy(vo[tm][:, :, DH:DH + 1],
                                                  onescol4[:])

                wp_cm.__exit__(None, None, None)

                # ---- attention ----
                with (
                    tc.tile_pool(name="scPS", bufs=2, space="PSUM") as scp,
                    tc.tile_pool(name="avPS", bufs=2, space="PSUM") as avp,
                    tc.tile_pool(name="attnSB", bufs=3) as asb,
                ):
                    for hp in range(HC // 2):      # head pairs at PE rows 0/64
                        for qc in range(G):
                            kt_max = 4 * qc + 3
                            avs = [avp.tile([DH + 1, TS], F32, tag=f"av{j}",
                                            name=f"av{j}") for j in range(2)]
                            for kt in range(kt_max + 1):
                                # both heads' score blocks into one 2-bank tile
                                sc_ps = scp.tile([P, 2, TS], F32,
                                                 tag="scp", name="scp")
                                for j in range(2):
                                    h_i = 2 * hp + j
                                    m = h_i // 2
                                    o = (h_i % 2) * DH
                                    nc.tensor.matmul(
                                        sc_ps[:, j, :],
                                        kT[m][o:o + DH, kt * P:(kt + 1) * P],
                                        qT[m][o:o + DH, qc * TS:(qc + 1) * TS],
                                        start=True, stop=True)
                                e_r = asb.tile([P, 2, TS], F32R,
                                               tag="erp", name="erp")
                                if kt < 4 * qc:
                                    nc.scalar.activation(
                                        e_r[:].rearrange("p a b -> p (a b)"),
                                        sc_ps[:].rearrange("p a b -> p (a b)"),
                                        AF.Exp, scale=0.125)
                                else:
                                    e_f = asb.tile([P, 2, TS], F32,
                                                   tag="efp", name="efp")
                                    nc.scalar.activation(
                                        e_f[:].rearrange("p a b -> p (a b)"),
                                        sc_ps[:].rearrange("p a b -> p (a b)"),
                                        AF.Exp, scale=0.125)
                                    sh = 512 * qc - 128 * kt
                                    eng = nc.vector if kt % 2 == 0 else nc.gpsimd
                                    eng.tensor_tensor(
                                        out=e_r[:].rearrange("p a b -> p (a b)"),
                                        in0=e_f[:].rearrange("p a b -> p (a b)"),
                                        in1=maskd[sh][:],
                                        op=ALU.mult)
                                for j in range(2):
                                    h_i = 2 * hp + j
                                    nc.tensor.matmul(avs[j][:],
                                                     vo[kt][:, h_i, :],
                                                     e_r[:, j, :],
                                                     start=(kt == 0),
                                                     stop=(kt == kt_max))
                            for j in range(2):
                                h_i = 2 * hp + j
                                un = asb.tile([DH + 1, TS], F32,
                                              tag=f"un{j}", name=f"un{j}")
                                nc.vector.tensor_copy(un[:], avs[j][:])
                                rec = asb.tile([1, TS], F32,
                                               tag=f"rec{j}", name=f"rec{j}")
                                nc.vector.reciprocal(rec[:], un[DH:DH + 1, :])
                                rb = asb.tile([DH, TS], F32,
                                              tag=f"rb{j}", name=f"rb{j}")
                                nc.gpsimd.partition_broadcast(rb[:], rec[:])
                                chunk = asb.tile([DH, TS], F32R,
                                                 tag=f"chunk{j}", name=f"chunk{j}")
                                nc.vector.tensor_tensor(out=chunk[:],
                                                        in0=un[0:DH, :],
                                                        in1=rb[:], op=ALU.mult)
                                nc.sync.dma_start(
                                    t["a2a_in"][h_i * DH:(h_i + 1) * DH,
                                                qc * TS:(qc + 1) * TS],
                                    chunk[:])

            # prefetch pools traced before AG2 so DMA covers the collective
            pfw1_cm = tc.tile_pool(name="pfW1", bufs=1)
            pfw1 = pfw1_cm.__enter__()
            w1s0 = [pfw1.tile([P, 8 * P], F32R, tag=f"w1s{k}", name=f"w1s{k}")
                    for k in range(KD)]
            pfd_cm = tc.tile_pool(name="pfD", bufs=1)
            pfd = pfd_cm.__enter__()
            wo = [pfd.tile([P, D], F32R, tag=f"wo{k}", name=f"wo{k}") for k in range(KD)]
            for k in range(KD):
                nc.sync.dma_start(wo[k][:], t["wo_d"][k * P:(k + 1) * P, :])
                nc.sync.dma_start(w1s0[k][:], t["w1_d"][k * P:(k + 1) * P, 0:8 * P])

            nc.gpsimd.collective_compute(
                "AllGather", ALU.bypass, ins=[t["a2a_in"][:]],
                outs=[t["a2a_out"][:]], replica_groups=GROUPS,
            )

            # ============ phase D: out_proj + residual (in-place on xs) ====
            with tc.tile_pool(name="opPS", bufs=3, space="PSUM") as opp, \
                 tc.tile_pool(name="opSB", bufs=1) as osb:
                aT = [osb.tile([P, TS], F32R, tag=f"aT{k}", name=f"aT{k}") for k in range(KD)]
                off_sb = osb.tile([1, 1], mybir.dt.int32, tag="off", name="off")
                nc.sync.dma_start(off_sb[:], t["coff_d"][:])
                with nc.gpsimd.register("roff") as roff:
                    nc.gpsimd.reg_load(roff, off_sb[0:1, 0:1])
                    rv = nc.snap(roff)
                    for k in range(KD):
                        nc.gpsimd.dma_start(
                            aT[k][:],
                            t["a2a_out"][k // 2, (k % 2) * P:(k % 2 + 1) * P,
                                         bass.ds(rv, TS)])
                for mt in range(NT):
                    for n in range(2):
                        ps = opp.tile([P, TS], F32, tag="op", name="op")
                        nc.tensor.matmul(ps[:], ones128[:],
                                         bo[:, n * TS:(n + 1) * TS],
                                         start=True, stop=False)
                        for k in range(KD):
                            nc.tensor.matmul(
                                ps[:], aT[k][:, mt * P:(mt + 1) * P],
                                wo[k][:, n * TS:(n + 1) * TS],
                                start=False, stop=(k == KD - 1))
                        # residual written in place: xs becomes x2
                        nc.vector.tensor_tensor(
                            out=xs[mt][:, n * TS:(n + 1) * TS], in0=ps[:],
                            in1=xs[mt][:, n * TS:(n + 1) * TS], op=ALU.add)
            pfd_cm.__exit__(None, None, None)
            x2 = xs

            with tc.tile_pool(name="h2TP", bufs=1) as h2tp:
                h2T = [h2tp.tile([P, TS], F32R, tag=f"h2T{k}", name=f"h2T{k}")
                       for k in range(KD)]
                with tc.tile_pool(name="lnD", bufs=1) as sc, \
                     tc.tile_pool(name="tpD", bufs=4, space="PSUM") as tp:
                    h2 = [sc.tile([P, D], F32, tag=f"h2{mt}", name=f"h2{mt}")
                          for mt in range(NT)]
                    layernorm(x2, h2, sc)
                    transpose_apply(h2, h2T, ln2g, ln2b, tp)

                # ============ phase E: FFN ============
                with tc.tile_pool(name="gTP", bufs=1) as gtp:
                    gT = [gtp.tile([P, TS], F32R, tag=f"gT{mf}", name=f"gT{mf}")
                          for mf in range(KF)]
                    MFB = 4     # mf tiles per w1 stream block
                    with tc.tile_pool(name="w1st", bufs=1) as w1p, \
                         tc.tile_pool(name="gPS", bufs=4, space="PSUM") as gps:
                        for blk in range(KF // MFB):
                            if blk < 2:
                                # prefetched during AG2 (w1s0 holds blocks 0-1)
                                w1s = [w1s0[k][:, blk * MFB * P:(blk + 1) * MFB * P]
                                       for k in range(KD)]
                            else:
                                w1t = [w1p.tile([P, MFB * P], F32R,
                                                tag=f"w1b{k}", name=f"w1b{k}",
                                                bufs=2)
                                       for k in range(KD)]
                                for k in range(KD):
                                    nc.sync.dma_start(
                                        w1t[k][:],
                                        t["w1_d"][k * P:(k + 1) * P,
                                                  blk * MFB * P:(blk + 1) * MFB * P])
                                w1s = [w1t[k][:] for k in range(KD)]
                            for j in range(MFB):
                                mf = blk * MFB + j
                                ps = gps.tile([P, TS], F32, tag="g", name="g")
                                for k in range(KD):
                                    nc.tensor.matmul(
                                        ps[:], w1s[k][:, j * P:(j + 1) * P],
                                        h2T[k][:], start=(k == 0),
                                        stop=(k == KD - 1))
                                nc.scalar.activation(gT[mf][:], ps[:],
                                                     AF.Gelu,
                                                     bias=b1p[:, mf:mf + 1])

                    with tc.tile_pool(name="w2st", bufs=4) as w2p, \
                         tc.tile_pool(name="fPS", bufs=1, space="PSUM") as fps, \
                         tc.tile_pool(name="ySB", bufs=2) as ysb:
                        f_ps = [fps.tile([P, D], F32, tag=f"f{mt}", name=f"f{mt}")
                                for mt in range(NT)]
                        for mt in range(NT):
                            for n in range(2):
                                nc.tensor.matmul(
                                    f_ps[mt][:, n * TS:(n + 1) * TS],
                                    ones128[:], b2[:, n * TS:(n + 1) * TS],
                                    start=True, stop=False)
                        for k2 in range(KF):
                            w2t = w2p.tile([P, D], F32R, tag="w2", name="w2")
                            nc.sync.dma_start(
                                w2t[:], t["w2_d"][k2 * P:(k2 + 1) * P, :])
                            for mt in range(NT):
                                for n in range(2):
                                    nc.tensor.matmul(
                                        f_ps[mt][:, n * TS:(n + 1) * TS],
                                        gT[k2][:, mt * P:(mt + 1) * P],
                                        w2t[:, n * TS:(n + 1) * TS],
                                        start=False, stop=(k2 == KF - 1))
                        for mt in range(NT):
                            yt = ysb.tile([P, D], F32, tag="y", name="y")
                            nc.vector.tensor_tensor(out=yt[:],
                                                    in0=f_ps[mt][:],
                                                    in1=x2[mt][:],
                                                    op=ALU.add)
                            nc.sync.dma_start(
                                t["y_d"][mt * P:(mt + 1) * P, :], yt[:])
            pfw1_cm.__exit__(None, None, None)


def _in_maps(inputs):
    f32 = np.float32
    maps = []
    for c in range(NC):
        b, r = c // G, c % G
        c0 = r * CC
        m = {
            "x": np.ascontiguousarray(np.asarray(inputs["x"])[b, r * TS:(r + 1) * TS, :], f32),
            "ln1_g": np.ascontiguousarray(inputs["ln1_g"], f32),
            "ln1_b": np.ascontiguousarray(inputs["ln1_b"], f32),
            "Wq": np.ascontiguousarray(np.asarray(inputs["Wq"])[:, c0:c0 + CC], f32),
            "Wk": np.ascontiguousarray(np.asarray(inputs["Wk"])[:, c0:c0 + CC], f32),
            "Wv": np.ascontiguousarray(np.asarray(inputs["Wv"])[:, c0:c0 + CC], f32),
            "bq": np.ascontiguousarray(np.asarray(inputs["bq"])[c0:c0 + CC], f32),
            "bk": np.ascontiguousarray(np.asarray(inputs["bk"])[c0:c0 + CC], f32),
            "bv": np.ascontiguousarray(np.asarray(inputs["bv"])[c0:c0 + CC], f32),
            "Wo": np.ascontiguousarray(inputs["Wo"], f32),
            "bo": np.ascontiguousarray(inputs["bo"], f32),
            "ln2_g": np.ascontiguousarray(inputs["ln2_g"], f32),
            "ln2_b": np.ascontiguousarray(inputs["ln2_b"], f32),
            "W1": np.ascontiguousarray(inputs["W1"], f32),
            "b1": np.ascontiguousarray(inputs["b1"], f32),
            "W2": np.ascontiguousarray(inputs["W2"], f32),
            "b2": np.ascontiguousarray(inputs["b2"], f32),
            "coff": np.array([[r * TS]], dtype=np.int32),
        }
        maps.append(m)
    return maps


def _run(inputs, trace=False):
    if "nc" not in _CACHE:
        _CACHE["nc"] = build()
    nc = _CACHE["nc"]
    maps = _in_maps(inputs)
    res = run_bass_kernel_spmd(nc, maps, list(range(NC)), trace=trace)
    out = np.empty((B, S, D), np.float32)
    for c in range(NC):
        b, r = c // G, c % G
        out[b, r * TS:(r + 1) * TS, :] = res.results[c]["y"]
    return out, res


def kernel(**inputs):
    out, _ = _run(inputs, trace=False)
    return out


if __name__ == "__main__":
    build()
    print("build OK")


# revision 24
# speedup vs baseline: 44.4964x; 1.0057x over previous
"""Trainium2 Bass kernel for a dense transformer block (B=2,S=2048,D=1024,H=16,DFF=4096).

Sharding across 8 NeuronCores:
  core c: batch b=c//4, group rank r=c%4, replica groups [[0,1,2,3],[4,5,6,7]].
  - LN1 computed on own 512-token strip; hT AllGather'ed within the 4-core group.
  - Attention: head-parallel (4 heads/core, full causal sequence).
  - AllToAll redistributes attention output from head-sharded to token-sharded.
  - out_proj, LN2, FFN: token-sharded (512 tokens/core), full weights.
Matmuls run in float32r (TF32-like, full PE rate); the residual spine stays fp32.
"""
import sys

sys.path.insert(0, "/opt/trn_rl_repo")

import numpy as np

import concourse.bass as bass
import concourse.mybir as mybir
import concourse.tile as tile
from concourse import bacc
from concourse.bass_utils import run_bass_kernel_spmd
from concourse.masks import make_identity

AF = mybir.ActivationFunctionType
ALU = mybir.AluOpType
F32 = mybir.dt.float32
F32R = mybir.dt.float32r

B, S, D, H = 2, 2048, 1024, 16
DH = D // H          # 64
DFF = 4 * D          # 4096
EPS = 1e-5
NC = 8               # cores
G = 4                # cores per group (per batch)
TS = S // G          # 512 tokens per core
HC = H // G          # 4 heads per core
CC = HC * DH         # 256 head-columns per core
P = 128
KD = D // P          # 8 k-tiles over D
KF = DFF // P        # 32 k-tiles over DFF
NT = TS // P         # 4 token tiles per strip
GROUPS = [[0, 1, 2, 3], [4, 5, 6, 7]]

_CACHE = {}


def build():
    nc = bacc.Bacc(None)

    io = {}
    io["x_d"] = nc.declare_dram_parameter("x", [TS, D], F32, isOutput=False)
    io["ln1g_d"] = nc.declare_dram_parameter("ln1_g", [D], F32, isOutput=False)
    io["ln1b_d"] = nc.declare_dram_parameter("ln1_b", [D], F32, isOutput=False)
    io["wq_d"] = nc.declare_dram_parameter("Wq", [D, CC], F32R, isOutput=False)
    io["wk_d"] = nc.declare_dram_parameter("Wk", [D, CC], F32R, isOutput=False)
    io["wv_d"] = nc.declare_dram_parameter("Wv", [D, CC], F32R, isOutput=False)
    io["bq_d"] = nc.declare_dram_parameter("bq", [CC], F32R, isOutput=False)
    io["bk_d"] = nc.declare_dram_parameter("bk", [CC], F32R, isOutput=False)
    io["bv_d"] = nc.declare_dram_parameter("bv", [CC], F32R, isOutput=False)
    io["wo_d"] = nc.declare_dram_parameter("Wo", [D, D], F32R, isOutput=False)
    io["bo_d"] = nc.declare_dram_parameter("bo", [D], F32R, isOutput=False)
    io["ln2g_d"] = nc.declare_dram_parameter("ln2_g", [D], F32, isOutput=False)
    io["ln2b_d"] = nc.declare_dram_parameter("ln2_b", [D], F32, isOutput=False)
    io["w1_d"] = nc.declare_dram_parameter("W1", [D, DFF], F32R, isOutput=False)
    io["b1_d"] = nc.declare_dram_parameter("b1", [DFF], F32R, isOutput=False)
    io["w2_d"] = nc.declare_dram_parameter("W2", [DFF, D], F32R, isOutput=False)
    io["b2_d"] = nc.declare_dram_parameter("b2", [D], F32R, isOutput=False)
    io["y_d"] = nc.declare_dram_parameter("y", [TS, D], F32, isOutput=True)

    io["ag1_in"] = nc.dram_tensor("ag1_in", [D, TS], F32R)
    io["ag1_out"] = nc.dram_tensor("ag1_out", [G, D, TS], F32R)
    io["a2a_in"] = nc.dram_tensor("a2a_in", [CC, S], F32R)
    io["a2a_out"] = nc.dram_tensor("a2a_out", [G, CC, S], F32R)
    io["coff_d"] = nc.declare_dram_parameter("coff", [1, 1], mybir.dt.int32,
                                             isOutput=False)

    with tile.TileContext(nc) as tc:
        _body(nc, tc, io)
    nc.compile()
    return nc


def _body(nc, tc, t):
    with tc.tile_pool(name="const", bufs=1) as cst:
        # ---------------- constants ----------------
        ident = cst.tile([P, P], F32)
        make_identity(nc, ident[:])

        onesrow_f = cst.tile([1, TS], F32)
        nc.gpsimd.memset(onesrow_f[:], 1.0)
        ones128 = cst.tile([1, P], F32R)        # K=1 lhsT (M=128 tokens)
        nc.vector.tensor_copy(ones128[:], onesrow_f[0:1, 0:P])
        onescol4 = cst.tile([P, HC, 1], F32)
        nc.gpsimd.memset(onescol4[:], 1.0)
        epsc = cst.tile([P, 1], F32)
        nc.gpsimd.memset(epsc[:], EPS)

        # doubled causal masks (one per diagonal shift), mask||mask layout so a
        # single DVE op masks a two-head [128, 1024] pair tile.
        maskd = {}
        for sh in (0, -128, -256, -384):
            md = cst.tile([P, 2 * TS], F32, tag=f"maskd{sh}", name=f"maskd{sh}")
            nc.gpsimd.memset(md[:], 1.0)
            for half in range(2):
                nc.gpsimd.affine_select(
                    out=md[:, half * TS:(half + 1) * TS],
                    in_=md[:, half * TS:(half + 1) * TS],
                    compare_op=ALU.is_ge, fill=0.0, base=sh,
                    pattern=[[1, TS]], channel_multiplier=-1,
                )
            maskd[sh] = md

        # layernorm gains/biases as [128, KD] (per-partition per k-tile)
        ln1g = cst.tile([P, KD], F32)
        ln1b = cst.tile([P, KD], F32)
        ln2g = cst.tile([P, KD], F32)
        ln2b = cst.tile([P, KD], F32)
        nc.sync.dma_start(ln1g[:], t["ln1g_d"].rearrange("(k p) -> p k", p=P))
        nc.sync.dma_start(ln1b[:], t["ln1b_d"].rearrange("(k p) -> p k", p=P))
        nc.sync.dma_start(ln2g[:], t["ln2g_d"].rearrange("(k p) -> p k", p=P))
        nc.sync.dma_start(ln2b[:], t["ln2b_d"].rearrange("(k p) -> p k", p=P))

        # bq/bk as per-partition [128, 2] (column-tile-major) for psum eviction
        bqp = cst.tile([P, 2], F32)
        bkp = cst.tile([P, 2], F32)
        nc.gpsimd.dma_start(bqp[:], t["bq_d"].rearrange("(m p) -> p m", p=P))
        nc.gpsimd.dma_start(bkp[:], t["bk_d"].rearrange("(m p) -> p m", p=P))
        # bv broadcast across partitions for the v eviction add
        bvrow = cst.tile([1, CC], F32)
        nc.gpsimd.dma_start(bvrow[:], t["bv_d"][None, :])
        bvb = cst.tile([P, CC], F32)
        nc.gpsimd.partition_broadcast(bvb[:], bvrow[:])
        # b1 as per-partition [128, KF] for the gelu bias operand
        b1p = cst.tile([P, KF], F32)
        nc.gpsimd.dma_start(b1p[:], t["b1_d"].rearrange("(k p) -> p k", p=P))
        bo = cst.tile([1, D], F32R)
        b2 = cst.tile([1, D], F32R)
        nc.sync.dma_start(bo[:], t["bo_d"][None, :])
        nc.sync.dma_start(b2[:], t["b2_d"][None, :])

        # ---------------- helpers ----------------
        def layernorm(src_tiles, dst_tiles, sc):
            # var = E[x^2] - mu^2 (safe: |mu| << std for this data), so the
            # normalize is a single fused (x - mu) * inv DVE pass.
            for mt in range(NT):
                xt = src_tiles[mt]
                mu = sc.tile([P, 1], F32, tag="mu", name="mu")
                nc.vector.tensor_reduce(out=mu[:], in_=xt[:], op=ALU.add,
                                        axis=mybir.AxisListType.X)
                mus = sc.tile([P, 1], F32, tag="mus", name="mus")
                nc.scalar.mul(mus[:], mu[:], 1.0 / D)
                sq = sc.tile([P, D], F32, tag="sq", name="sq")
                sumsq = sc.tile([P, 1], F32, tag="sumsq", name="sumsq")
                nc.scalar.activation(sq[:], xt[:], AF.Square, accum_out=sumsq[:])
                mu2 = sc.tile([P, 1], F32, tag="mu2", name="mu2")
                nc.scalar.activation(mu2[:], mus[:], AF.Square)
                vpe = sc.tile([P, 1], F32, tag="vpe", name="vpe")
                # vpe = sumsq/D - mu2 + eps  (two tiny fused scalar ops)
                nc.vector.tensor_scalar(out=vpe[:], in0=sumsq[:],
                                        scalar1=1.0 / D, scalar2=mu2[:],
                                        op0=ALU.mult, op1=ALU.subtract)
                std = sc.tile([P, 1], F32, tag="std", name="std")
                nc.scalar.activation(std[:], vpe[:], AF.Sqrt, bias=epsc[:])
                inv = sc.tile([P, 1], F32, tag="inv", name="inv")
                nc.vector.reciprocal(inv[:], std[:])
                nc.vector.tensor_scalar(out=dst_tiles[mt][:], in0=xt[:],
                                        scalar1=mus[:], scalar2=inv[:],
                                        op0=ALU.subtract, op1=ALU.mult)

        def transpose_apply(src_tiles, dst_tiles, g_sb, b_sb, tp):
            for k in range(KD):
                for mt in range(NT):
                    ps = tp.tile([P, P], F32, tag="tps", name="tps")
                    nc.tensor.transpose(ps[:], src_tiles[mt][:, k * P:(k + 1) * P],
                                        ident[:])
                    nc.vector.tensor_scalar(
                        out=dst_tiles[k][:, mt * P:(mt + 1) * P], in0=ps[:],
                        scalar1=g_sb[:, k:k + 1], scalar2=b_sb[:, k:k + 1],
                        op0=ALU.mult, op1=ALU.add)

        # ============ phase A: LN1 + transpose + AllGather ============
        with tc.tile_pool(name="xsP", bufs=1) as xsp:
            xs = [xsp.tile([P, D], F32, tag=f"xs{mt}", name=f"xs{mt}") for mt in range(NT)]
            for mt in range(NT):
                nc.sync.dma_start(xs[mt][:], t["x_d"][mt * P:(mt + 1) * P, :])

            with tc.tile_pool(name="lnA", bufs=1) as sc, \
                 tc.tile_pool(name="tpA", bufs=4, space="PSUM") as tp:
                h = [sc.tile([P, D], F32, tag=f"h{mt}", name=f"h{mt}") for mt in range(NT)]
                layernorm(xs, h, sc)
                hT = [sc.tile([P, TS], F32R, tag=f"hT{k}", name=f"hT{k}") for k in range(KD)]
                transpose_apply(h, hT, ln1g, ln1b, tp)
                for k in range(KD):
                    nc.sync.dma_start(t["ag1_in"][k * P:(k + 1) * P, :], hT[k][:])

            # ============ phases B+C: QKV + attention ============
            with tc.tile_pool(name="qkvP", bufs=1) as qkv:
                qT = [qkv.tile([P, S], F32R, tag=f"qT{m}", name=f"qT{m}") for m in range(2)]
                kT = [qkv.tile([P, S], F32R, tag=f"kT{m}", name=f"kT{m}") for m in range(2)]
                vo = [qkv.tile([P, HC, DH + 1], F32R, tag=f"vo{tm}", name=f"vo{tm}")
                      for tm in range(S // P)]

                wp_cm = tc.tile_pool(name="wqkv", bufs=1)
                wp = wp_cm.__enter__()
                # weight loads traced before the collective: DMA covers AG1
                wq = [wp.tile([P, CC], F32R, tag=f"wq{k}", name=f"wq{k}") for k in range(KD)]
                wk = [wp.tile([P, CC], F32R, tag=f"wk{k}", name=f"wk{k}") for k in range(KD)]
                wv = [wp.tile([P, CC], F32R, tag=f"wv{k}", name=f"wv{k}") for k in range(KD)]
                for k in range(KD):
                    nc.sync.dma_start(wq[k][:], t["wq_d"][k * P:(k + 1) * P, :])
                    nc.sync.dma_start(wk[k][:], t["wk_d"][k * P:(k + 1) * P, :])
                    nc.sync.dma_start(wv[k][:], t["wv_d"][k * P:(k + 1) * P, :])

                nc.gpsimd.collective_compute(
                    "AllGather", ALU.bypass, ins=[t["ag1_in"][:]],
                    outs=[t["ag1_out"][:]], replica_groups=GROUPS,
                )

                with tc.tile_pool(name="hTst", bufs=2) as st, \
                     tc.tile_pool(name="projPS", bufs=3, space="PSUM") as pps:
                    for qc in range(G):
                        hTq = [st.tile([P, TS], F32R, tag=f"hTq{k}", name=f"hTq{k}")
                               for k in range(KD)]
                        for k in range(KD):
                            nc.sync.dma_start(
                                hTq[k][:], t["ag1_out"][qc, k * P:(k + 1) * P, :])
                        for (w_sb, b_sb, out_sb) in ((wq, bqp, qT), (wk, bkp, kT)):
                            for m in range(2):
                                ps = pps.tile([P, TS], F32, tag="pps", name="pps")
                                for k in range(KD):
                                    nc.tensor.matmul(
                                        ps[:], w_sb[k][:, m * P:(m + 1) * P],
                                        hTq[k][:], start=(k == 0),
                                        stop=(k == KD - 1))
                                nc.vector.tensor_scalar(
                                    out=out_sb[m][:, qc * TS:(qc + 1) * TS],
                                    in0=ps[:], scalar1=b_sb[:, m:m + 1],
                                    scalar2=None, op0=ALU.add)
                        for mt in range(NT):
                            tm = qc * NT + mt
                            ps = pps.tile([P, CC], F32, tag="vps", name="vps")
                            for k in range(KD):
                                nc.tensor.matmul(
                                    ps[:], hTq[k][:, mt * P:(mt + 1) * P],
                                    wv[k][:], start=(k == 0), stop=(k == KD - 1))
                            nc.vector.tensor_tensor(
                                out=vo[tm][:, :, 0:DH],
                                in0=ps[:].rearrange("p (h e) -> p h e", h=HC),
                                in1=bvb[:].rearrange("p (h e) -> p h e", h=HC),
                                op=ALU.add)
                            nc.vector.tensor_copy(vo[tm][:, :, DH:DH + 1],
                                                  onescol4[:])

                wp_cm.__exit__(None, None, None)

                # ---- attention ----
                with (
                    tc.tile_pool(name="scPS", bufs=2, space="PSUM") as scp,
                    tc.tile_pool(name="avPS", bufs=2, space="PSUM") as avp,
                    tc.tile_pool(name="attnSB", bufs=3) as asb,
                ):
                    for hp in range(HC // 2):      # head pairs at PE rows 0/64
                        for qc in range(G):
                            kt_max = 4 * qc + 3
                            avs = [avp.tile([DH + 1, TS], F32, tag=f"av{j}",
                                            name=f"av{j}") for j in range(2)]
                            for kt in range(kt_max + 1):
                                # both heads' score blocks into one 2-bank tile
                                sc_ps = scp.tile([P, 2, TS], F32,
                                                 tag="scp", name="scp")
                                for j in range(2):
                                    h_i = 2 * hp + j
                                    m = h_i // 2
                                    o = (h_i % 2) * DH
                                    nc.tensor.matmul(
                                        sc_ps[:, j, :],
                                        kT[m][o:o + DH, kt * P:(kt + 1) * P],
                                        qT[m][o:o + DH, qc * TS:(qc + 1) * TS],
                                        start=True, stop=True)
                                e_r = asb.tile([P, 2, TS], F32R,
                                               tag="erp", name="erp")
                                if kt < 4 * qc:
                                    nc.scalar.activation(
                                        e_r[:].rearrange("p a b -> p (a b)"),
                                        sc_ps[:].rearrange("p a b -> p (a b)"),
                                        AF.Exp, scale=0.125)
                                else:
                                    e_f = asb.tile([P, 2, TS], F32,
                                                   tag="efp", name="efp")
                                    nc.scalar.activation(
                                        e_f[:].rearrange("p a b -> p (a b)"),
                                        sc_ps[:].rearrange("p a b -> p (a b)"),
                                        AF.Exp, scale=0.125)
                                    sh = 512 * qc - 128 * kt
                                    nc.vector.tensor_tensor(
                                        out=e_r[:].rearrange("p a b -> p (a b)"),
                                        in0=e_f[:].rearrange("p a b -> p (a b)"),
                                        in1=maskd[sh][:],
                                        op=ALU.mult)
                                for j in range(2):
                                    h_i = 2 * hp + j
                                    nc.tensor.matmul(avs[j][:],
                                                     vo[kt][:, h_i, :],
                                                     e_r[:, j, :],
                                                     start=(kt == 0),
                                                     stop=(kt == kt_max))
                            for j in range(2):
                                h_i = 2 * hp + j
                                un = asb.tile([DH + 1, TS], F32,
                                              tag=f"un{j}", name=f"un{j}")
                                nc.vector.tensor_copy(un[:], avs[j][:])
                                rec = asb.tile([1, TS], F32,
                                               tag=f"rec{j}", name=f"rec{j}")
                                nc.vector.reciprocal(rec[:], un[DH:DH + 1, :])
                                rb = asb.tile([DH, TS], F32,
                                              tag=f"rb{j}", name=f"rb{j}")
                                nc.gpsimd.partition_broadcast(rb[:], rec[:])
                                chunk = asb.tile([DH, TS], F32R,
                                                 tag=f"chunk{j}", name=f"chunk{j}")
                                nc.vector.tensor_tensor(out=chunk[:],
                                                        in0=un[0:DH, :],
                                                        in1=rb[:], op=ALU.mult)
                                nc.sync.dma_start(
                                    t["a2a_in"][h_i * DH:(h_i + 1) * DH,
                                                qc * TS:(qc + 1) * TS],
                                    chunk[:])

            # prefetch pools traced before AG2 so DMA covers the collective
            pfw1_cm = tc.tile_pool(name="pfW1", bufs=1)
            pfw1 = pfw1_cm.__enter__()
            w1s0 = [pfw1.tile([P, 8 * P], F32R, tag=f"w1s{k}", name=f"w1s{k}")
                    for k in range(KD)]
            pfd_cm = tc.tile_pool(name="pfD", bufs=1)
            pfd = pfd_cm.__enter__()
            wo = [pfd.tile([P, D], F32R, tag=f"wo{k}", name=f"wo{k}") for k in range(KD)]
            for k in range(KD):
                nc.sync.dma_start(wo[k][:], t["wo_d"][k * P:(k + 1) * P, :])
                nc.sync.dma_start(w1s0[k][:], t["w1_d"][k * P:(k + 1) * P, 0:8 * P])

            nc.gpsimd.collective_compute(
                "AllGather", ALU.bypass, ins=[t["a2a_in"][:]],
                outs=[t["a2a_out"][:]], replica_groups=GROUPS,
            )

            # ============ phase D: out_proj + residual (in-place on xs) ====
            with tc.tile_pool(name="opPS", bufs=3, space="PSUM") as opp, \
                 tc.tile_pool(name="opSB", bufs=1) as osb:
                aT = [osb.tile([P, TS], F32R, tag=f"aT{k}", name=f"aT{k}") for k in range(KD)]
                off_sb = osb.tile([1, 1], mybir.dt.int32, tag="off", name="off")
                nc.sync.dma_start(off_sb[:], t["coff_d"][:])
                with nc.gpsimd.register("roff") as roff:
                    nc.gpsimd.reg_load(roff, off_sb[0:1, 0:1])
                    rv = nc.snap(roff)
                    for k in range(KD):
                        nc.gpsimd.dma_start(
                            aT[k][:],
                            t["a2a_out"][k // 2, (k % 2) * P:(k % 2 + 1) * P,
                                         bass.ds(rv, TS)])
                for mt in range(NT):
                    for n in range(2):
                        ps = opp.tile([P, TS], F32, tag="op", name="op")
                        nc.tensor.matmul(ps[:], ones128[:],
                                         bo[:, n * TS:(n + 1) * TS],
                                         start=True, stop=False)
                        for k in range(KD):
                            nc.tensor.matmul(
                                ps[:], aT[k][:, mt * P:(mt + 1) * P],
                                wo[k][:, n * TS:(n + 1) * TS],
                                start=False, stop=(k == KD - 1))
                        # residual written in place: xs becomes x2
                        nc.vector.tensor_tensor(
                            out=xs[mt][:, n * TS:(n + 1) * TS], in0=ps[:],
                            in1=xs[mt][:, n * TS:(n + 1) * TS], op=ALU.add)
            pfd_cm.__exit__(None, None, None)
            x2 = xs

            with tc.tile_pool(name="h2TP", bufs=1) as h2tp:
                h2T = [h2tp.tile([P, TS], F32R, tag=f"h2T{k}", name=f"h2T{k}")
                       for k in range(KD)]
                with tc.tile_pool(name="lnD", bufs=1) as sc, \
                     tc.tile_pool(name="tpD", bufs=4, space="PSUM") as tp:
                    h2 = [sc.tile([P, D], F32, tag=f"h2{mt}", name=f"h2{mt}")
                          for mt in range(NT)]
                    layernorm(x2, h2, sc)
                    transpose_apply(h2, h2T, ln2g, ln2b, tp)

                # ============ phase E: FFN ============
                with tc.tile_pool(name="gTP", bufs=1) as gtp:
                    gT = [gtp.tile([P, TS], F32R, tag=f"gT{mf}", name=f"gT{mf}")
                          for mf in range(KF)]
                    MFB = 4     # mf tiles per w1 stream block
                    with tc.tile_pool(name="w1st", bufs=1) as w1p, \
                         tc.tile_pool(name="gPS", bufs=4, space="PSUM") as gps:
                        for blk in range(KF // MFB):
                            if blk < 2:
                                # prefetched during AG2 (w1s0 holds blocks 0-1)
                                w1s = [w1s0[k][:, blk * MFB * P:(blk + 1) * MFB * P]
                                       for k in range(KD)]
                            else:
                                w1t = [w1p.tile([P, MFB * P], F32R,
                                                tag=f"w1b{k}", name=f"w1b{k}",
                                                bufs=2)
                                       for k in range(KD)]
                                for k in range(KD):
                                    nc.sync.dma_start(
                                        w1t[k][:],
                                        t["w1_d"][k * P:(k + 1) * P,
                                                  blk * MFB * P:(blk + 1) * MFB * P])
                                w1s = [w1t[k][:] for k in range(KD)]
                            for j in range(MFB):
                                mf = blk * MFB + j
                                ps = gps.tile([P, TS], F32, tag="g", name="g")
                                for k in range(KD):
                                    nc.tensor.matmul(
                                        ps[:], w1s[k][:, j * P:(j + 1) * P],
                                        h2T[k][:], start=(k == 0),
                                        stop=(k == KD - 1))
                                nc.scalar.activation(gT[mf][:], ps[:],
                                                     AF.Gelu,
                                                     bias=b1p[:, mf:mf + 1])

                    with tc.tile_pool(name="w2st", bufs=4) as w2p, \
                         tc.tile_pool(name="fPS", bufs=1, space="PSUM") as fps, \
                         tc.tile_pool(name="ySB", bufs=2) as ysb:
                        f_ps = [fps.tile([P, D], F32, tag=f"f{mt}", name=f"f{mt}")
                                for mt in range(NT)]
                        for mt in range(NT):
                            for n in range(2):
                                nc.tensor.matmul(
                                    f_ps[mt][:, n * TS:(n + 1) * TS],
                                    ones128[:], b2[:, n * TS:(n + 1) * TS],
                                    start=True, stop=False)
                        for k2 in range(KF):
                            w2t = w2p.tile([P, D], F32R, tag="w2", name="w2")
                            nc.sync.dma_start(
                                w2t[:], t["w2_d"][k2 * P:(k2 + 1) * P, :])
                            for mt in range(NT):
                                for n in range(2):
                                    nc.tensor.matmul(
                                        f_ps[mt][:, n * TS:(n + 1) * TS],
                                        gT[k2][:, mt * P:(mt + 1) * P],
                                        w2t[:, n * TS:(n + 1) * TS],
                                        start=False, stop=(k2 == KF - 1))
                        for mt in range(NT):
                            yt = ysb.tile([P, D], F32, tag="y", name="y")
                            nc.vector.tensor_tensor(out=yt[:],
                                                    in0=f_ps[mt][:],
                                                    in1=x2[mt][:],
                                                    op=ALU.add)
                            nc.sync.dma_start(
                                t["y_d"][mt * P:(mt + 1) * P, :], yt[:])
            pfw1_cm.__exit__(None, None, None)


def _in_maps(inputs):
    f32 = np.float32
    maps = []
    for c in range(NC):
        b, r = c // G, c % G
        c0 = r * CC
        m = {
            "x": np.ascontiguousarray(np.asarray(inputs["x"])[b, r * TS:(r + 1) * TS, :], f32),
            "ln1_g": np.ascontiguousarray(inputs["ln1_g"], f32),
            "ln1_b": np.ascontiguousarray(inputs["ln1_b"], f32),
            "Wq": np.ascontiguousarray(np.asarray(inputs["Wq"])[:, c0:c0 + CC], f32),
            "Wk": np.ascontiguousarray(np.asarray(inputs["Wk"])[:, c0:c0 + CC], f32),
            "Wv": np.ascontiguousarray(np.asarray(inputs["Wv"])[:, c0:c0 + CC], f32),
            "bq": np.ascontiguousarray(np.asarray(inputs["bq"])[c0:c0 + CC], f32),
            "bk": np.ascontiguousarray(np.asarray(inputs["bk"])[c0:c0 + CC], f32),
            "bv": np.ascontiguousarray(np.asarray(inputs["bv"])[c0:c0 + CC], f32),
            "Wo": np.ascontiguousarray(inputs["Wo"], f32),
            "bo": np.ascontiguousarray(inputs["bo"], f32),
            "ln2_g": np.ascontiguousarray(inputs["ln2_g"], f32),
            "ln2_b": np.ascontiguousarray(inputs["ln2_b"], f32),
            "W1": np.ascontiguousarray(inputs["W1"], f32),
            "b1": np.ascontiguousarray(inputs["b1"], f32),
            "W2": np.ascontiguousarray(inputs["W2"], f32),
            "b2": np.ascontiguousarray(inputs["b2"], f32),
            "coff": np.array([[r * TS]], dtype=np.int32),
        }
        maps.append(m)
    return maps


def _run(inputs, trace=False):
    if "nc" not in _CACHE:
        _CACHE["nc"] = build()
    nc = _CACHE["nc"]
    maps = _in_maps(inputs)
    res = run_bass_kernel_spmd(nc, maps, list(range(NC)), trace=trace)
    out = np.empty((B, S, D), np.float32)
    for c in range(NC):
        b, r = c // G, c % G
        out[b, r * TS:(r + 1) * TS, :] = res.results[c]["y"]
    return out, res


def kernel(**inputs):
    out, _ = _run(inputs, trace=False)
    return out


if __name__ == "__main__":
    build()
    print("build OK")


# revision 25
# speedup vs baseline: 45.1319x; 1.0143x over previous
"""Trainium2 Bass kernel for a dense transformer block (B=2,S=2048,D=1024,H=16,DFF=4096).

Sharding across 8 NeuronCores:
  core c: batch b=c//4, group rank r=c%4, replica groups [[0,1,2,3],[4,5,6,7]].
  - LN1 computed on own 512-token strip; hT AllGather'ed within the 4-core group.
  - Attention: head-parallel (4 heads/core, full causal sequence).
  - AllToAll redistributes attention output from head-sharded to token-sharded.
  - out_proj, LN2, FFN: token-sharded (512 tokens/core), full weights.
Matmuls run in float32r (TF32-like, full PE rate); the residual spine stays fp32.
"""
import sys

sys.path.insert(0, "/opt/trn_rl_repo")

import numpy as np

import concourse.bass as bass
import concourse.mybir as mybir
import concourse.tile as tile
from concourse import bacc
from concourse.bass_utils import run_bass_kernel_spmd
from concourse.masks import make_identity

AF = mybir.ActivationFunctionType
ALU = mybir.AluOpType
F32 = mybir.dt.float32
F32R = mybir.dt.float32r

B, S, D, H = 2, 2048, 1024, 16
DH = D // H          # 64
DFF = 4 * D          # 4096
EPS = 1e-5
NC = 8               # cores
G = 4                # cores per group (per batch)
TS = S // G          # 512 tokens per core
HC = H // G          # 4 heads per core
CC = HC * DH         # 256 head-columns per core
P = 128
KD = D // P          # 8 k-tiles over D
KF = DFF // P        # 32 k-tiles over DFF
NT = TS // P         # 4 token tiles per strip
GROUPS = [[0, 1, 2, 3], [4, 5, 6, 7]]

_CACHE = {}


def build():
    nc = bacc.Bacc(None)

    io = {}
    io["x_d"] = nc.declare_dram_parameter("x", [TS, D], F32, isOutput=False)
    io["ln1g_d"] = nc.declare_dram_parameter("ln1_g", [D], F32, isOutput=False)
    io["ln1b_d"] = nc.declare_dram_parameter("ln1_b", [D], F32, isOutput=False)
    io["wq_d"] = nc.declare_dram_parameter("Wq", [D, CC], F32R, isOutput=False)
    io["wk_d"] = nc.declare_dram_parameter("Wk", [D, CC], F32R, isOutput=False)
    io["wv_d"] = nc.declare_dram_parameter("Wv", [D, CC], F32R, isOutput=False)
    io["bq_d"] = nc.declare_dram_parameter("bq", [CC], F32R, isOutput=False)
    io["bk_d"] = nc.declare_dram_parameter("bk", [CC], F32R, isOutput=False)
    io["bv_d"] = nc.declare_dram_parameter("bv", [CC], F32R, isOutput=False)
    io["wo_d"] = nc.declare_dram_parameter("Wo", [D, D], F32R, isOutput=False)
    io["bo_d"] = nc.declare_dram_parameter("bo", [D], F32R, isOutput=False)
    io["ln2g_d"] = nc.declare_dram_parameter("ln2_g", [D], F32, isOutput=False)
    io["ln2b_d"] = nc.declare_dram_parameter("ln2_b", [D], F32, isOutput=False)
    io["w1_d"] = nc.declare_dram_parameter("W1", [D, DFF], F32R, isOutput=False)
    io["b1_d"] = nc.declare_dram_parameter("b1", [DFF], F32R, isOutput=False)
    io["w2_d"] = nc.declare_dram_parameter("W2", [DFF, D], F32R, isOutput=False)
    io["b2_d"] = nc.declare_dram_parameter("b2", [D], F32R, isOutput=False)
    io["y_d"] = nc.declare_dram_parameter("y", [TS, D], F32, isOutput=True)

    io["ag1_in"] = nc.dram_tensor("ag1_in", [D, TS], F32R)
    io["ag1_out"] = nc.dram_tensor("ag1_out", [G, D, TS], F32R)
    io["a2a_in"] = nc.dram_tensor("a2a_in", [CC, S], F32R)
    io["a2a_out"] = nc.dram_tensor("a2a_out", [G, CC, S], F32R)
    io["coff_d"] = nc.declare_dram_parameter("coff", [1, 1], mybir.dt.int32,
                                             isOutput=False)

    with tile.TileContext(nc) as tc:
        _body(nc, tc, io)
    nc.compile()
    return nc


def _body(nc, tc, t):
    with tc.tile_pool(name="const", bufs=1) as cst:
        # ---------------- constants ----------------
        ident = cst.tile([P, P], F32)
        make_identity(nc, ident[:])

        onesrow_f = cst.tile([1, TS], F32)
        nc.gpsimd.memset(onesrow_f[:], 1.0)
        ones128 = cst.tile([1, P], F32R)        # K=1 lhsT (M=128 tokens)
        nc.vector.tensor_copy(ones128[:], onesrow_f[0:1, 0:P])
        onescol4 = cst.tile([P, HC, 1], F32)
        nc.gpsimd.memset(onescol4[:], 1.0)
        epsc = cst.tile([P, 1], F32)
        nc.gpsimd.memset(epsc[:], EPS)

        # doubled causal masks (one per diagonal shift), mask||mask layout so a
        # single DVE op masks a two-head [128, 1024] pair tile.
        maskd = {}
        for sh in (0, -128, -256, -384):
            md = cst.tile([P, 2 * TS], F32, tag=f"maskd{sh}", name=f"maskd{sh}")
            nc.gpsimd.memset(md[:], 1.0)
            for half in range(2):
                nc.gpsimd.affine_select(
                    out=md[:, half * TS:(half + 1) * TS],
                    in_=md[:, half * TS:(half + 1) * TS],
                    compare_op=ALU.is_ge, fill=0.0, base=sh,
                    pattern=[[1, TS]], channel_multiplier=-1,
                )
            maskd[sh] = md

        # layernorm gains/biases as [128, KD] (per-partition per k-tile)
        ln1g = cst.tile([P, KD], F32)
        ln1b = cst.tile([P, KD], F32)
        ln2g = cst.tile([P, KD], F32)
        ln2b = cst.tile([P, KD], F32)
        nc.sync.dma_start(ln1g[:], t["ln1g_d"].rearrange("(k p) -> p k", p=P))
        nc.sync.dma_start(ln1b[:], t["ln1b_d"].rearrange("(k p) -> p k", p=P))
        nc.sync.dma_start(ln2g[:], t["ln2g_d"].rearrange("(k p) -> p k", p=P))
        nc.sync.dma_start(ln2b[:], t["ln2b_d"].rearrange("(k p) -> p k", p=P))

        # bq/bk as per-partition [128, 2] (column-tile-major) for psum eviction
        bqp = cst.tile([P, 2], F32)
        bkp = cst.tile([P, 2], F32)
        nc.gpsimd.dma_start(bqp[:], t["bq_d"].rearrange("(m p) -> p m", p=P))
        nc.gpsimd.dma_start(bkp[:], t["bk_d"].rearrange("(m p) -> p m", p=P))
        # bv broadcast across partitions for the v eviction add
        bvrow = cst.tile([1, CC], F32)
        nc.gpsimd.dma_start(bvrow[:], t["bv_d"][None, :])
        bvb = cst.tile([P, CC], F32)
        nc.gpsimd.partition_broadcast(bvb[:], bvrow[:])
        # b1 as per-partition [128, KF] for the gelu bias operand
        b1p = cst.tile([P, KF], F32)
        nc.gpsimd.dma_start(b1p[:], t["b1_d"].rearrange("(k p) -> p k", p=P))
        bo = cst.tile([1, D], F32R)
        b2 = cst.tile([1, D], F32R)
        nc.sync.dma_start(bo[:], t["bo_d"][None, :])
        nc.sync.dma_start(b2[:], t["b2_d"][None, :])

        # ---------------- helpers ----------------
        def layernorm(src_tiles, dst_tiles, sc):
            # var = E[x^2] - mu^2 (safe: |mu| << std for this data), so the
            # normalize is a single fused (x - mu) * inv DVE pass.
            for mt in range(NT):
                xt = src_tiles[mt]
                mu = sc.tile([P, 1], F32, tag="mu", name="mu")
                nc.vector.tensor_reduce(out=mu[:], in_=xt[:], op=ALU.add,
                                        axis=mybir.AxisListType.X)
                mus = sc.tile([P, 1], F32, tag="mus", name="mus")
                nc.scalar.mul(mus[:], mu[:], 1.0 / D)
                sq = sc.tile([P, D], F32, tag="sq", name="sq")
                sumsq = sc.tile([P, 1], F32, tag="sumsq", name="sumsq")
                nc.scalar.activation(sq[:], xt[:], AF.Square, accum_out=sumsq[:])
                mu2 = sc.tile([P, 1], F32, tag="mu2", name="mu2")
                nc.scalar.activation(mu2[:], mus[:], AF.Square)
                vpe = sc.tile([P, 1], F32, tag="vpe", name="vpe")
                # vpe = sumsq/D - mu2 + eps  (two tiny fused scalar ops)
                nc.vector.tensor_scalar(out=vpe[:], in0=sumsq[:],
                                        scalar1=1.0 / D, scalar2=mu2[:],
                                        op0=ALU.mult, op1=ALU.subtract)
                std = sc.tile([P, 1], F32, tag="std", name="std")
                nc.scalar.activation(std[:], vpe[:], AF.Sqrt, bias=epsc[:])
                inv = sc.tile([P, 1], F32, tag="inv", name="inv")
                nc.vector.reciprocal(inv[:], std[:])
                nc.vector.tensor_scalar(out=dst_tiles[mt][:], in0=xt[:],
                                        scalar1=mus[:], scalar2=inv[:],
                                        op0=ALU.subtract, op1=ALU.mult)

        def transpose_apply(src_tiles, dst_tiles, g_sb, b_sb, tp):
            for k in range(KD):
                for mt in range(NT):
                    ps = tp.tile([P, P], F32, tag="tps", name="tps")
                    nc.tensor.transpose(ps[:], src_tiles[mt][:, k * P:(k + 1) * P],
                                        ident[:])
                    nc.vector.tensor_scalar(
                        out=dst_tiles[k][:, mt * P:(mt + 1) * P], in0=ps[:],
                        scalar1=g_sb[:, k:k + 1], scalar2=b_sb[:, k:k + 1],
                        op0=ALU.mult, op1=ALU.add)

        # ============ phase A: LN1 + transpose + AllGather ============
        with tc.tile_pool(name="xsP", bufs=1) as xsp:
            xs = [xsp.tile([P, D], F32, tag=f"xs{mt}", name=f"xs{mt}") for mt in range(NT)]
            for mt in range(NT):
                nc.sync.dma_start(xs[mt][:], t["x_d"][mt * P:(mt + 1) * P, :])

            with tc.tile_pool(name="lnA", bufs=1) as sc, \
                 tc.tile_pool(name="tpA", bufs=4, space="PSUM") as tp:
                h = [sc.tile([P, D], F32, tag=f"h{mt}", name=f"h{mt}") for mt in range(NT)]
                layernorm(xs, h, sc)
                hT = [sc.tile([P, TS], F32R, tag=f"hT{k}", name=f"hT{k}") for k in range(KD)]
                transpose_apply(h, hT, ln1g, ln1b, tp)
                for k in range(KD):
                    nc.sync.dma_start(t["ag1_in"][k * P:(k + 1) * P, :], hT[k][:])

            # ============ phases B+C: QKV + attention ============
            with tc.tile_pool(name="qkvP", bufs=1) as qkv:
                qT = [qkv.tile([P, S], F32R, tag=f"qT{m}", name=f"qT{m}") for m in range(2)]
                kT = [qkv.tile([P, S], F32R, tag=f"kT{m}", name=f"kT{m}") for m in range(2)]
                vo = [qkv.tile([P, HC, DH + 1], F32R, tag=f"vo{tm}", name=f"vo{tm}")
                      for tm in range(S // P)]

                wp_cm = tc.tile_pool(name="wqkv", bufs=1)
                wp = wp_cm.__enter__()
                # weight loads traced before the collective: DMA covers AG1
                wq = [wp.tile([P, CC], F32R, tag=f"wq{k}", name=f"wq{k}") for k in range(KD)]
                wk = [wp.tile([P, CC], F32R, tag=f"wk{k}", name=f"wk{k}") for k in range(KD)]
                wv = [wp.tile([P, CC], F32R, tag=f"wv{k}", name=f"wv{k}") for k in range(KD)]
                for k in range(KD):
                    nc.sync.dma_start(wq[k][:], t["wq_d"][k * P:(k + 1) * P, :])
                    nc.sync.dma_start(wk[k][:], t["wk_d"][k * P:(k + 1) * P, :])
                    nc.sync.dma_start(wv[k][:], t["wv_d"][k * P:(k + 1) * P, :])

                nc.gpsimd.collective_compute(
                    "AllGather", ALU.bypass, ins=[t["ag1_in"][:]],
                    outs=[t["ag1_out"][:]], replica_groups=GROUPS,
                )

                with tc.tile_pool(name="hTst", bufs=2) as st, \
                     tc.tile_pool(name="projPS", bufs=3, space="PSUM") as pps:
                    for qc in range(G):
                        hTq = [st.tile([P, TS], F32R, tag=f"hTq{k}", name=f"hTq{k}")
                               for k in range(KD)]
                        for k in range(KD):
                            nc.sync.dma_start(
                                hTq[k][:], t["ag1_out"][qc, k * P:(k + 1) * P, :])
                        for (w_sb, b_sb, out_sb) in ((wq, bqp, qT), (wk, bkp, kT)):
                            for m in range(2):
                                ps = pps.tile([P, TS], F32, tag="pps", name="pps")
                                for k in range(KD):
                                    nc.tensor.matmul(
                                        ps[:], w_sb[k][:, m * P:(m + 1) * P],
                                        hTq[k][:], start=(k == 0),
                                        stop=(k == KD - 1))
                                nc.vector.tensor_scalar(
                                    out=out_sb[m][:, qc * TS:(qc + 1) * TS],
                                    in0=ps[:], scalar1=b_sb[:, m:m + 1],
                                    scalar2=None, op0=ALU.add)
                        for mt in range(NT):
                            tm = qc * NT + mt
                            ps = pps.tile([P, CC], F32, tag="vps", name="vps")
                            for k in range(KD):
                                nc.tensor.matmul(
                                    ps[:], hTq[k][:, mt * P:(mt + 1) * P],
                                    wv[k][:], start=(k == 0), stop=(k == KD - 1))
                            nc.vector.tensor_tensor(
                                out=vo[tm][:, :, 0:DH],
                                in0=ps[:].rearrange("p (h e) -> p h e", h=HC),
                                in1=bvb[:].rearrange("p (h e) -> p h e", h=HC),
                                op=ALU.add)
                            nc.vector.tensor_copy(vo[tm][:, :, DH:DH + 1],
                                                  onescol4[:])

                wp_cm.__exit__(None, None, None)

                # ---- attention ----
                with (
                    tc.tile_pool(name="scPS", bufs=2, space="PSUM") as scp,
                    tc.tile_pool(name="avPS", bufs=2, space="PSUM") as avp,
                    tc.tile_pool(name="attnSB", bufs=3) as asb,
                ):
                    for hp in range(HC // 2):      # head pairs at PE rows 0/64
                        for qc in range(G):
                            kt_max = 4 * qc + 3
                            avs = [avp.tile([DH + 1, TS], F32, tag=f"av{j}",
                                            name=f"av{j}") for j in range(2)]
                            for kt in range(kt_max + 1):
                                # both heads' score blocks into one 2-bank tile
                                sc_ps = scp.tile([P, 2, TS], F32,
                                                 tag="scp", name="scp")
                                for j in range(2):
                                    h_i = 2 * hp + j
                                    m = h_i // 2
                                    o = (h_i % 2) * DH
                                    nc.tensor.matmul(
                                        sc_ps[:, j, :],
                                        kT[m][o:o + DH, kt * P:(kt + 1) * P],
                                        qT[m][o:o + DH, qc * TS:(qc + 1) * TS],
                                        start=True, stop=True)
                                e_r = asb.tile([P, 2, TS], F32R,
                                               tag="erp", name="erp")
                                if kt < 4 * qc:
                                    v0 = 0      # valid columns start
                                    nc.scalar.activation(
                                        e_r[:].rearrange("p a b -> p (a b)"),
                                        sc_ps[:].rearrange("p a b -> p (a b)"),
                                        AF.Exp, scale=0.125)
                                else:
                                    # diag block, shift s=-128*d: cols < 128*d
                                    # are fully masked -- never compute/read them
                                    d = kt - 4 * qc
                                    v0 = P * d
                                    e_f = asb.tile([P, 2, TS], F32,
                                                   tag="efp", name="efp")
                                    nc.scalar.activation(
                                        e_f[:, :, v0:], sc_ps[:, :, v0:],
                                        AF.Exp, scale=0.125)
                                    sh = 512 * qc - 128 * kt
                                    mdv = maskd[sh][:].rearrange(
                                        "p (a b) -> p a b", a=2)
                                    nc.vector.tensor_tensor(
                                        out=e_r[:, :, v0:],
                                        in0=e_f[:, :, v0:],
                                        in1=mdv[:, :, v0:],
                                        op=ALU.mult)
                                for j in range(2):
                                    h_i = 2 * hp + j
                                    nc.tensor.matmul(avs[j][:, v0:],
                                                     vo[kt][:, h_i, :],
                                                     e_r[:, j, v0:],
                                                     start=(kt == 0),
                                                     stop=(kt == kt_max))
                            for j in range(2):
                                h_i = 2 * hp + j
                                un = asb.tile([DH + 1, TS], F32,
                                              tag=f"un{j}", name=f"un{j}")
                                nc.vector.tensor_copy(un[:], avs[j][:])
                                rec = asb.tile([1, TS], F32,
                                               tag=f"rec{j}", name=f"rec{j}")
                                nc.vector.reciprocal(rec[:], un[DH:DH + 1, :])
                                rb = asb.tile([DH, TS], F32,
                                              tag=f"rb{j}", name=f"rb{j}")
                                nc.gpsimd.partition_broadcast(rb[:], rec[:])
                                chunk = asb.tile([DH, TS], F32R,
                                                 tag=f"chunk{j}", name=f"chunk{j}")
                                nc.vector.tensor_tensor(out=chunk[:],
                                                        in0=un[0:DH, :],
                                                        in1=rb[:], op=ALU.mult)
                                nc.sync.dma_start(
                                    t["a2a_in"][h_i * DH:(h_i + 1) * DH,
                                                qc * TS:(qc + 1) * TS],
                                    chunk[:])

            # prefetch pools traced before AG2 so DMA covers the collective
            pfw1_cm = tc.tile_pool(name="pfW1", bufs=1)
            pfw1 = pfw1_cm.__enter__()
            w1s0 = [pfw1.tile([P, 8 * P], F32R, tag=f"w1s{k}", name=f"w1s{k}")
                    for k in range(KD)]
            pfd_cm = tc.tile_pool(name="pfD", bufs=1)
            pfd = pfd_cm.__enter__()
            wo = [pfd.tile([P, D], F32R, tag=f"wo{k}", name=f"wo{k}") for k in range(KD)]
            for k in range(KD):
                nc.sync.dma_start(wo[k][:], t["wo_d"][k * P:(k + 1) * P, :])
                nc.sync.dma_start(w1s0[k][:], t["w1_d"][k * P:(k + 1) * P, 0:8 * P])

            nc.gpsimd.collective_compute(
                "AllGather", ALU.bypass, ins=[t["a2a_in"][:]],
                outs=[t["a2a_out"][:]], replica_groups=GROUPS,
            )

            # ============ phase D: out_proj + residual (in-place on xs) ====
            with tc.tile_pool(name="opPS", bufs=3, space="PSUM") as opp, \
                 tc.tile_pool(name="opSB", bufs=1) as osb:
                aT = [osb.tile([P, TS], F32R, tag=f"aT{k}", name=f"aT{k}") for k in range(KD)]
                off_sb = osb.tile([1, 1], mybir.dt.int32, tag="off", name="off")
                nc.sync.dma_start(off_sb[:], t["coff_d"][:])
                with nc.gpsimd.register("roff") as roff:
                    nc.gpsimd.reg_load(roff, off_sb[0:1, 0:1])
                    rv = nc.snap(roff)
                    for k in range(KD):
                        nc.gpsimd.dma_start(
                            aT[k][:],
                            t["a2a_out"][k // 2, (k % 2) * P:(k % 2 + 1) * P,
                                         bass.ds(rv, TS)])
                for mt in range(NT):
                    for n in range(2):
                        ps = opp.tile([P, TS], F32, tag="op", name="op")
                        nc.tensor.matmul(ps[:], ones128[:],
                                         bo[:, n * TS:(n + 1) * TS],
                                         start=True, stop=False)
                        for k in range(KD):
                            nc.tensor.matmul(
                                ps[:], aT[k][:, mt * P:(mt + 1) * P],
                                wo[k][:, n * TS:(n + 1) * TS],
                                start=False, stop=(k == KD - 1))
                        # residual written in place: xs becomes x2
                        nc.vector.tensor_tensor(
                            out=xs[mt][:, n * TS:(n + 1) * TS], in0=ps[:],
                            in1=xs[mt][:, n * TS:(n + 1) * TS], op=ALU.add)
            pfd_cm.__exit__(None, None, None)
            x2 = xs

            with tc.tile_pool(name="h2TP", bufs=1) as h2tp:
                h2T = [h2tp.tile([P, TS], F32R, tag=f"h2T{k}", name=f"h2T{k}")
                       for k in range(KD)]
                with tc.tile_pool(name="lnD", bufs=1) as sc, \
                     tc.tile_pool(name="tpD", bufs=4, space="PSUM") as tp:
                    h2 = [sc.tile([P, D], F32, tag=f"h2{mt}", name=f"h2{mt}")
                          for mt in range(NT)]
                    layernorm(x2, h2, sc)
                    transpose_apply(h2, h2T, ln2g, ln2b, tp)

                # ============ phase E: FFN ============
                with tc.tile_pool(name="gTP", bufs=1) as gtp:
                    gT = [gtp.tile([P, TS], F32R, tag=f"gT{mf}", name=f"gT{mf}")
                          for mf in range(KF)]
                    MFB = 4     # mf tiles per w1 stream block
                    with tc.tile_pool(name="w1st", bufs=1) as w1p, \
                         tc.tile_pool(name="gPS", bufs=4, space="PSUM") as gps:
                        for blk in range(KF // MFB):
                            if blk < 2:
                                # prefetched during AG2 (w1s0 holds blocks 0-1)
                                w1s = [w1s0[k][:, blk * MFB * P:(blk + 1) * MFB * P]
                                       for k in range(KD)]
                            else:
                                w1t = [w1p.tile([P, MFB * P], F32R,
                                                tag=f"w1b{k}", name=f"w1b{k}",
                                                bufs=2)
                                       for k in range(KD)]
                                for k in range(KD):
                                    nc.sync.dma_start(
                                        w1t[k][:],
                                        t["w1_d"][k * P:(k + 1) * P,
                                                  blk * MFB * P:(blk + 1) * MFB * P])
                                w1s = [w1t[k][:] for k in range(KD)]
                            for j in range(MFB):
                                mf = blk * MFB + j
                                ps = gps.tile([P, TS], F32, tag="g", name="g")
                                for k in range(KD):
                                    nc.tensor.matmul(
                                        ps[:], w1s[k][:, j * P:(j + 1) * P],
                                        h2T[k][:], start=(k == 0),
                                        stop=(k == KD - 1))
                                nc.scalar.activation(gT[mf][:], ps[:],
                                                     AF.Gelu,
                                                     bias=b1p[:, mf:mf + 1])

                    with tc.tile_pool(name="w2st", bufs=4) as w2p, \
                         tc.tile_pool(name="fPS", bufs=1, space="PSUM") as fps, \
                         tc.tile_pool(name="ySB", bufs=2) as ysb:
                        f_ps = [fps.tile([P, D], F32, tag=f"f{mt}", name=f"f{mt}")
                                for mt in range(NT)]
                        for mt in range(NT):
                            for n in range(2):
                                nc.tensor.matmul(
                                    f_ps[mt][:, n * TS:(n + 1) * TS],
                                    ones128[:], b2[:, n * TS:(n + 1) * TS],
                                    start=True, stop=False)
                        for k2 in range(KF):
                            w2t = w2p.tile([P, D], F32R, tag="w2", name="w2")
                            nc.sync.dma_start(
                                w2t[:], t["w2_d"][k2 * P:(k2 + 1) * P, :])
                            for mt in range(NT):
                                for n in range(2):
                                    nc.tensor.matmul(
                                        f_ps[mt][:, n * TS:(n + 1) * TS],
                                        gT[k2][:, mt * P:(mt + 1) * P],
                                        w2t[:, n * TS:(n + 1) * TS],
                                        start=False, stop=(k2 == KF - 1))
                        for mt in range(NT):
                            yt = ysb.tile([P, D], F32, tag="y", name="y")
                            nc.vector.tensor_tensor(out=yt[:],
                                                    in0=f_ps[mt][:],
                                                    in1=x2[mt][:],
                                                    op=ALU.add)
                            nc.sync.dma_start(
                                t["y_d"][mt * P:(mt + 1) * P, :], yt[:])
            pfw1_cm.__exit__(None, None, None)


def _in_maps(inputs):
    f32 = np.float32
    maps = []
    for c in range(NC):
        b, r = c // G, c % G
        c0 = r * CC
        m = {
            "x": np.ascontiguousarray(np.asarray(inputs["x"])[b, r * TS:(r + 1) * TS, :], f32),
            "ln1_g": np.ascontiguousarray(inputs["ln1_g"], f32),
            "ln1_b": np.ascontiguousarray(inputs["ln1_b"], f32),
            "Wq": np.ascontiguousarray(np.asarray(inputs["Wq"])[:, c0:c0 + CC], f32),
            "Wk": np.ascontiguousarray(np.asarray(inputs["Wk"])[:, c0:c0 + CC], f32),
            "Wv": np.ascontiguousarray(np.asarray(inputs["Wv"])[:, c0:c0 + CC], f32),
            "bq": np.ascontiguousarray(np.asarray(inputs["bq"])[c0:c0 + CC], f32),
            "bk": np.ascontiguousarray(np.asarray(inputs["bk"])[c0:c0 + CC], f32),
            "bv": np.ascontiguousarray(np.asarray(inputs["bv"])[c0:c0 + CC], f32),
            "Wo": np.ascontiguousarray(inputs["Wo"], f32),
            "bo": np.ascontiguousarray(inputs["bo"], f32),
            "ln2_g": np.ascontiguousarray(inputs["ln2_g"], f32),
            "ln2_b": np.ascontiguousarray(inputs["ln2_b"], f32),
            "W1": np.ascontiguousarray(inputs["W1"], f32),
            "b1": np.ascontiguousarray(inputs["b1"], f32),
            "W2": np.ascontiguousarray(inputs["W2"], f32),
            "b2": np.ascontiguousarray(inputs["b2"], f32),
            "coff": np.array([[r * TS]], dtype=np.int32),
        }
        maps.append(m)
    return maps


def _run(inputs, trace=False):
    if "nc" not in _CACHE:
        _CACHE["nc"] = build()
    nc = _CACHE["nc"]
    maps = _in_maps(inputs)
    res = run_bass_kernel_spmd(nc, maps, list(range(NC)), trace=trace)
    out = np.empty((B, S, D), np.float32)
    for c in range(NC):
        b, r = c // G, c % G
        out[b, r * TS:(r + 1) * TS, :] = res.results[c]["y"]
    return out, res


def kernel(**inputs):
    out, _ = _run(inputs, trace=False)
    return out


if __name__ == "__main__":
    build()
    print("build OK")


# revision 26
# speedup vs baseline: 45.3249x; 1.0043x over previous
"""Trainium2 Bass kernel for a dense transformer block (B=2,S=2048,D=1024,H=16,DFF=4096).

Sharding across 8 NeuronCores:
  core c: batch b=c//4, group rank r=c%4, replica groups [[0,1,2,3],[4,5,6,7]].
  - LN1 computed on own 512-token strip; hT AllGather'ed within the 4-core group.
  - Attention: head-parallel (4 heads/core, full causal sequence).
  - AllToAll redistributes attention output from head-sharded to token-sharded.
  - out_proj, LN2, FFN: token-sharded (512 tokens/core), full weights.
Matmuls run in float32r (TF32-like, full PE rate); the residual spine stays fp32.
"""
import sys

sys.path.insert(0, "/opt/trn_rl_repo")

import numpy as np

import concourse.bass as bass
import concourse.mybir as mybir
import concourse.tile as tile
from concourse import bacc
from concourse.bass_utils import run_bass_kernel_spmd
from concourse.masks import make_identity

AF = mybir.ActivationFunctionType
ALU = mybir.AluOpType
F32 = mybir.dt.float32
F32R = mybir.dt.float32r

B, S, D, H = 2, 2048, 1024, 16
DH = D // H          # 64
DFF = 4 * D          # 4096
EPS = 1e-5
NC = 8               # cores
G = 4                # cores per group (per batch)
TS = S // G          # 512 tokens per core
HC = H // G          # 4 heads per core
CC = HC * DH         # 256 head-columns per core
P = 128
KD = D // P          # 8 k-tiles over D
KF = DFF // P        # 32 k-tiles over DFF
NT = TS // P         # 4 token tiles per strip
GROUPS = [[0, 1, 2, 3], [4, 5, 6, 7]]

_CACHE = {}


def build():
    nc = bacc.Bacc(None)

    io = {}
    io["x_d"] = nc.declare_dram_parameter("x", [TS, D], F32, isOutput=False)
    io["ln1g_d"] = nc.declare_dram_parameter("ln1_g", [D], F32, isOutput=False)
    io["ln1b_d"] = nc.declare_dram_parameter("ln1_b", [D], F32, isOutput=False)
    io["wq_d"] = nc.declare_dram_parameter("Wq", [D, CC], F32R, isOutput=False)
    io["wk_d"] = nc.declare_dram_parameter("Wk", [D, CC], F32R, isOutput=False)
    io["wv_d"] = nc.declare_dram_parameter("Wv", [D, CC], F32R, isOutput=False)
    io["bq_d"] = nc.declare_dram_parameter("bq", [CC], F32R, isOutput=False)
    io["bk_d"] = nc.declare_dram_parameter("bk", [CC], F32R, isOutput=False)
    io["bv_d"] = nc.declare_dram_parameter("bv", [CC], F32R, isOutput=False)
    io["wo_d"] = nc.declare_dram_parameter("Wo", [D, D], F32R, isOutput=False)
    io["bo_d"] = nc.declare_dram_parameter("bo", [D], F32R, isOutput=False)
    io["ln2g_d"] = nc.declare_dram_parameter("ln2_g", [D], F32, isOutput=False)
    io["ln2b_d"] = nc.declare_dram_parameter("ln2_b", [D], F32, isOutput=False)
    io["w1_d"] = nc.declare_dram_parameter("W1", [D, DFF], F32R, isOutput=False)
    io["b1_d"] = nc.declare_dram_parameter("b1", [DFF], F32R, isOutput=False)
    io["w2_d"] = nc.declare_dram_parameter("W2", [DFF, D], F32R, isOutput=False)
    io["b2_d"] = nc.declare_dram_parameter("b2", [D], F32R, isOutput=False)
    io["y_d"] = nc.declare_dram_parameter("y", [TS, D], F32, isOutput=True)

    io["ag1_in"] = nc.dram_tensor("ag1_in", [D, TS], F32R)
    io["ag1_out"] = nc.dram_tensor("ag1_out", [G, D, TS], F32R)
    io["a2a_in"] = nc.dram_tensor("a2a_in", [CC, S], F32R)
    io["a2a_out"] = nc.dram_tensor("a2a_out", [G, CC, S], F32R)
    io["coff_d"] = nc.declare_dram_parameter("coff", [1, 1], mybir.dt.int32,
                                             isOutput=False)

    with tile.TileContext(nc) as tc:
        _body(nc, tc, io)
    nc.compile()
    return nc


def _body(nc, tc, t):
    with tc.tile_pool(name="const", bufs=1) as cst:
        # x strip loads first: they gate the LN1 -> transpose -> AG1 chain
        xsp_cm = tc.tile_pool(name="xsP", bufs=1)
        xsp = xsp_cm.__enter__()
        xs = [xsp.tile([P, D], F32, tag=f"xs{mt}", name=f"xs{mt}")
              for mt in range(NT)]
        for mt in range(NT):
            nc.sync.dma_start(xs[mt][:], t["x_d"][mt * P:(mt + 1) * P, :])

        # ---------------- constants ----------------
        ident = cst.tile([P, P], F32)
        make_identity(nc, ident[:])

        onesrow_f = cst.tile([1, TS], F32)
        nc.gpsimd.memset(onesrow_f[:], 1.0)
        ones128 = cst.tile([1, P], F32R)        # K=1 lhsT (M=128 tokens)
        nc.vector.tensor_copy(ones128[:], onesrow_f[0:1, 0:P])
        onescol4 = cst.tile([P, HC, 1], F32)
        nc.gpsimd.memset(onescol4[:], 1.0)
        epsc = cst.tile([P, 1], F32)
        nc.gpsimd.memset(epsc[:], EPS)

        # doubled causal masks (one per diagonal shift), mask||mask layout so a
        # single DVE op masks a two-head [128, 1024] pair tile.
        maskd = {}
        for sh in (0, -128, -256, -384):
            md = cst.tile([P, 2 * TS], F32, tag=f"maskd{sh}", name=f"maskd{sh}")
            nc.gpsimd.memset(md[:], 1.0)
            for half in range(2):
                nc.gpsimd.affine_select(
                    out=md[:, half * TS:(half + 1) * TS],
                    in_=md[:, half * TS:(half + 1) * TS],
                    compare_op=ALU.is_ge, fill=0.0, base=sh,
                    pattern=[[1, TS]], channel_multiplier=-1,
                )
            maskd[sh] = md

        # layernorm gains/biases as [128, KD] (per-partition per k-tile)
        ln1g = cst.tile([P, KD], F32)
        ln1b = cst.tile([P, KD], F32)
        ln2g = cst.tile([P, KD], F32)
        ln2b = cst.tile([P, KD], F32)
        nc.sync.dma_start(ln1g[:], t["ln1g_d"].rearrange("(k p) -> p k", p=P))
        nc.sync.dma_start(ln1b[:], t["ln1b_d"].rearrange("(k p) -> p k", p=P))
        nc.sync.dma_start(ln2g[:], t["ln2g_d"].rearrange("(k p) -> p k", p=P))
        nc.sync.dma_start(ln2b[:], t["ln2b_d"].rearrange("(k p) -> p k", p=P))

        # bq/bk as per-partition [128, 2] (column-tile-major) for psum eviction
        bqp = cst.tile([P, 2], F32)
        bkp = cst.tile([P, 2], F32)
        nc.gpsimd.dma_start(bqp[:], t["bq_d"].rearrange("(m p) -> p m", p=P))
        nc.gpsimd.dma_start(bkp[:], t["bk_d"].rearrange("(m p) -> p m", p=P))
        # bv broadcast across partitions for the v eviction add
        bvrow = cst.tile([1, CC], F32)
        nc.gpsimd.dma_start(bvrow[:], t["bv_d"][None, :])
        bvb = cst.tile([P, CC], F32)
        nc.gpsimd.partition_broadcast(bvb[:], bvrow[:])
        # b1 as per-partition [128, KF] for the gelu bias operand
        b1p = cst.tile([P, KF], F32)
        nc.gpsimd.dma_start(b1p[:], t["b1_d"].rearrange("(k p) -> p k", p=P))
        bo = cst.tile([1, D], F32R)
        b2 = cst.tile([1, D], F32R)
        nc.sync.dma_start(bo[:], t["bo_d"][None, :])
        nc.sync.dma_start(b2[:], t["b2_d"][None, :])

        # ---------------- helpers ----------------
        def layernorm(src_tiles, dst_tiles, sc):
            # var = E[x^2] - mu^2 (safe: |mu| << std for this data), so the
            # normalize is a single fused (x - mu) * inv DVE pass.
            for mt in range(NT):
                xt = src_tiles[mt]
                mu = sc.tile([P, 1], F32, tag="mu", name="mu")
                nc.vector.tensor_reduce(out=mu[:], in_=xt[:], op=ALU.add,
                                        axis=mybir.AxisListType.X)
                mus = sc.tile([P, 1], F32, tag="mus", name="mus")
                nc.scalar.mul(mus[:], mu[:], 1.0 / D)
                sq = sc.tile([P, D], F32, tag="sq", name="sq")
                sumsq = sc.tile([P, 1], F32, tag="sumsq", name="sumsq")
                nc.scalar.activation(sq[:], xt[:], AF.Square, accum_out=sumsq[:])
                mu2 = sc.tile([P, 1], F32, tag="mu2", name="mu2")
                nc.scalar.activation(mu2[:], mus[:], AF.Square)
                vpe = sc.tile([P, 1], F32, tag="vpe", name="vpe")
                # vpe = sumsq/D - mu2 + eps  (two tiny fused scalar ops)
                nc.vector.tensor_scalar(out=vpe[:], in0=sumsq[:],
                                        scalar1=1.0 / D, scalar2=mu2[:],
                                        op0=ALU.mult, op1=ALU.subtract)
                std = sc.tile([P, 1], F32, tag="std", name="std")
                nc.scalar.activation(std[:], vpe[:], AF.Sqrt, bias=epsc[:])
                inv = sc.tile([P, 1], F32, tag="inv", name="inv")
                nc.vector.reciprocal(inv[:], std[:])
                nc.vector.tensor_scalar(out=dst_tiles[mt][:], in0=xt[:],
                                        scalar1=mus[:], scalar2=inv[:],
                                        op0=ALU.subtract, op1=ALU.mult)

        def transpose_apply(src_tiles, dst_tiles, g_sb, b_sb, tp):
            for k in range(KD):
                for mt in range(NT):
                    ps = tp.tile([P, P], F32, tag="tps", name="tps")
                    nc.tensor.transpose(ps[:], src_tiles[mt][:, k * P:(k + 1) * P],
                                        ident[:])
                    nc.vector.tensor_scalar(
                        out=dst_tiles[k][:, mt * P:(mt + 1) * P], in0=ps[:],
                        scalar1=g_sb[:, k:k + 1], scalar2=b_sb[:, k:k + 1],
                        op0=ALU.mult, op1=ALU.add)

        # ============ phase A: LN1 + transpose + AllGather ============
        if True:
            with tc.tile_pool(name="lnA", bufs=1) as sc, \
                 tc.tile_pool(name="tpA", bufs=4, space="PSUM") as tp:
                h = [sc.tile([P, D], F32, tag=f"h{mt}", name=f"h{mt}") for mt in range(NT)]
                layernorm(xs, h, sc)
                hT = [sc.tile([P, TS], F32R, tag=f"hT{k}", name=f"hT{k}") for k in range(KD)]
                transpose_apply(h, hT, ln1g, ln1b, tp)
                for k in range(KD):
                    nc.sync.dma_start(t["ag1_in"][k * P:(k + 1) * P, :], hT[k][:])

            # ============ phases B+C: QKV + attention ============
            with tc.tile_pool(name="qkvP", bufs=1) as qkv:
                qT = [qkv.tile([P, S], F32R, tag=f"qT{m}", name=f"qT{m}") for m in range(2)]
                kT = [qkv.tile([P, S], F32R, tag=f"kT{m}", name=f"kT{m}") for m in range(2)]
                vo = [qkv.tile([P, HC, DH + 1], F32R, tag=f"vo{tm}", name=f"vo{tm}")
                      for tm in range(S // P)]

                wp_cm = tc.tile_pool(name="wqkv", bufs=1)
                wp = wp_cm.__enter__()
                # weight loads traced before the collective: DMA covers AG1
                wq = [wp.tile([P, CC], F32R, tag=f"wq{k}", name=f"wq{k}") for k in range(KD)]
                wk = [wp.tile([P, CC], F32R, tag=f"wk{k}", name=f"wk{k}") for k in range(KD)]
                wv = [wp.tile([P, CC], F32R, tag=f"wv{k}", name=f"wv{k}") for k in range(KD)]
                for k in range(KD):
                    nc.sync.dma_start(wq[k][:], t["wq_d"][k * P:(k + 1) * P, :])
                    nc.sync.dma_start(wk[k][:], t["wk_d"][k * P:(k + 1) * P, :])
                    nc.sync.dma_start(wv[k][:], t["wv_d"][k * P:(k + 1) * P, :])

                nc.gpsimd.collective_compute(
                    "AllGather", ALU.bypass, ins=[t["ag1_in"][:]],
                    outs=[t["ag1_out"][:]], replica_groups=GROUPS,
                )

                with tc.tile_pool(name="hTst", bufs=2) as st, \
                     tc.tile_pool(name="projPS", bufs=3, space="PSUM") as pps:
                    for qc in range(G):
                        hTq = [st.tile([P, TS], F32R, tag=f"hTq{k}", name=f"hTq{k}")
                               for k in range(KD)]
                        for k in range(KD):
                            nc.sync.dma_start(
                                hTq[k][:], t["ag1_out"][qc, k * P:(k + 1) * P, :])
                        for (w_sb, b_sb, out_sb) in ((wq, bqp, qT), (wk, bkp, kT)):
                            for m in range(2):
                                ps = pps.tile([P, TS], F32, tag="pps", name="pps")
                                for k in range(KD):
                                    nc.tensor.matmul(
                                        ps[:], w_sb[k][:, m * P:(m + 1) * P],
                                        hTq[k][:], start=(k == 0),
                                        stop=(k == KD - 1))
                                nc.vector.tensor_scalar(
                                    out=out_sb[m][:, qc * TS:(qc + 1) * TS],
                                    in0=ps[:], scalar1=b_sb[:, m:m + 1],
                                    scalar2=None, op0=ALU.add)
                        for mt in range(NT):
                            tm = qc * NT + mt
                            ps = pps.tile([P, CC], F32, tag="vps", name="vps")
                            for k in range(KD):
                                nc.tensor.matmul(
                                    ps[:], hTq[k][:, mt * P:(mt + 1) * P],
                                    wv[k][:], start=(k == 0), stop=(k == KD - 1))
                            nc.vector.tensor_tensor(
                                out=vo[tm][:, :, 0:DH],
                                in0=ps[:].rearrange("p (h e) -> p h e", h=HC),
                                in1=bvb[:].rearrange("p (h e) -> p h e", h=HC),
                                op=ALU.add)
                            nc.vector.tensor_copy(vo[tm][:, :, DH:DH + 1],
                                                  onescol4[:])

                wp_cm.__exit__(None, None, None)

                # ---- attention ----
                with (
                    tc.tile_pool(name="scPS", bufs=2, space="PSUM") as scp,
                    tc.tile_pool(name="avPS", bufs=2, space="PSUM") as avp,
                    tc.tile_pool(name="attnSB", bufs=3) as asb,
                ):
                    for hp in range(HC // 2):      # head pairs at PE rows 0/64
                        for qc in range(G):
                            kt_max = 4 * qc + 3
                            avs = [avp.tile([DH + 1, TS], F32, tag=f"av{j}",
                                            name=f"av{j}") for j in range(2)]
                            for kt in range(kt_max + 1):
                                # both heads' score blocks into one 2-bank tile
                                sc_ps = scp.tile([P, 2, TS], F32,
                                                 tag="scp", name="scp")
                                for j in range(2):
                                    h_i = 2 * hp + j
                                    m = h_i // 2
                                    o = (h_i % 2) * DH
                                    nc.tensor.matmul(
                                        sc_ps[:, j, :],
                                        kT[m][o:o + DH, kt * P:(kt + 1) * P],
                                        qT[m][o:o + DH, qc * TS:(qc + 1) * TS],
                                        start=True, stop=True)
                                e_r = asb.tile([P, 2, TS], F32R,
                                               tag="erp", name="erp")
                                if kt < 4 * qc:
                                    v0 = 0      # valid columns start
                                    nc.scalar.activation(
                                        e_r[:].rearrange("p a b -> p (a b)"),
                                        sc_ps[:].rearrange("p a b -> p (a b)"),
                                        AF.Exp, scale=0.125)
                                else:
                                    # diag block, shift s=-128*d: cols < 128*d
                                    # are fully masked -- never compute/read them
                                    d = kt - 4 * qc
                                    v0 = P * d
                                    e_f = asb.tile([P, 2, TS], F32,
                                                   tag="efp", name="efp")
                                    nc.scalar.activation(
                                        e_f[:, :, v0:], sc_ps[:, :, v0:],
                                        AF.Exp, scale=0.125)
                                    sh = 512 * qc - 128 * kt
                                    mdv = maskd[sh][:].rearrange(
                                        "p (a b) -> p a b", a=2)
                                    nc.vector.tensor_tensor(
                                        out=e_r[:, :, v0:],
                                        in0=e_f[:, :, v0:],
                                        in1=mdv[:, :, v0:],
                                        op=ALU.mult)
                                for j in range(2):
                                    h_i = 2 * hp + j
                                    nc.tensor.matmul(avs[j][:, v0:],
                                                     vo[kt][:, h_i, :],
                                                     e_r[:, j, v0:],
                                                     start=(kt == 0),
                                                     stop=(kt == kt_max))
                            for j in range(2):
                                h_i = 2 * hp + j
                                un = asb.tile([DH + 1, TS], F32,
                                              tag=f"un{j}", name=f"un{j}")
                                nc.vector.tensor_copy(un[:], avs[j][:])
                                rec = asb.tile([1, TS], F32,
                                               tag=f"rec{j}", name=f"rec{j}")
                                nc.vector.reciprocal(rec[:], un[DH:DH + 1, :])
                                rb = asb.tile([DH, TS], F32,
                                              tag=f"rb{j}", name=f"rb{j}")
                                nc.gpsimd.partition_broadcast(rb[:], rec[:])
                                chunk = asb.tile([DH, TS], F32R,
                                                 tag=f"chunk{j}", name=f"chunk{j}")
                                nc.vector.tensor_tensor(out=chunk[:],
                                                        in0=un[0:DH, :],
                                                        in1=rb[:], op=ALU.mult)
                                nc.sync.dma_start(
                                    t["a2a_in"][h_i * DH:(h_i + 1) * DH,
                                                qc * TS:(qc + 1) * TS],
                                    chunk[:])

            # prefetch pools traced before AG2 so DMA covers the collective
            pfw1_cm = tc.tile_pool(name="pfW1", bufs=1)
            pfw1 = pfw1_cm.__enter__()
            w1s0 = [pfw1.tile([P, 8 * P], F32R, tag=f"w1s{k}", name=f"w1s{k}")
                    for k in range(KD)]
            pfd_cm = tc.tile_pool(name="pfD", bufs=1)
            pfd = pfd_cm.__enter__()
            wo = [pfd.tile([P, D], F32R, tag=f"wo{k}", name=f"wo{k}") for k in range(KD)]
            for k in range(KD):
                nc.sync.dma_start(wo[k][:], t["wo_d"][k * P:(k + 1) * P, :])
                nc.sync.dma_start(w1s0[k][:], t["w1_d"][k * P:(k + 1) * P, 0:8 * P])

            nc.gpsimd.collective_compute(
                "AllGather", ALU.bypass, ins=[t["a2a_in"][:]],
                outs=[t["a2a_out"][:]], replica_groups=GROUPS,
            )

            # ============ phase D: out_proj + residual (in-place on xs) ====
            with tc.tile_pool(name="opPS", bufs=3, space="PSUM") as opp, \
                 tc.tile_pool(name="opSB", bufs=1) as osb:
                aT = [osb.tile([P, TS], F32R, tag=f"aT{k}", name=f"aT{k}") for k in range(KD)]
                off_sb = osb.tile([1, 1], mybir.dt.int32, tag="off", name="off")
                nc.sync.dma_start(off_sb[:], t["coff_d"][:])
                with nc.gpsimd.register("roff") as roff:
                    nc.gpsimd.reg_load(roff, off_sb[0:1, 0:1])
                    rv = nc.snap(roff)
                    for k in range(KD):
                        nc.gpsimd.dma_start(
                            aT[k][:],
                            t["a2a_out"][k // 2, (k % 2) * P:(k % 2 + 1) * P,
                                         bass.ds(rv, TS)])
                for mt in range(NT):
                    for n in range(2):
                        ps = opp.tile([P, TS], F32, tag="op", name="op")
                        nc.tensor.matmul(ps[:], ones128[:],
                                         bo[:, n * TS:(n + 1) * TS],
                                         start=True, stop=False)
                        for k in range(KD):
                            nc.tensor.matmul(
                                ps[:], aT[k][:, mt * P:(mt + 1) * P],
                                wo[k][:, n * TS:(n + 1) * TS],
                                start=False, stop=(k == KD - 1))
                        # residual written in place: xs becomes x2
                        nc.vector.tensor_tensor(
                            out=xs[mt][:, n * TS:(n + 1) * TS], in0=ps[:],
                            in1=xs[mt][:, n * TS:(n + 1) * TS], op=ALU.add)
            pfd_cm.__exit__(None, None, None)
            x2 = xs

            with tc.tile_pool(name="h2TP", bufs=1) as h2tp:
                h2T = [h2tp.tile([P, TS], F32R, tag=f"h2T{k}", name=f"h2T{k}")
                       for k in range(KD)]
                with tc.tile_pool(name="lnD", bufs=1) as sc, \
                     tc.tile_pool(name="tpD", bufs=4, space="PSUM") as tp:
                    h2 = [sc.tile([P, D], F32, tag=f"h2{mt}", name=f"h2{mt}")
                          for mt in range(NT)]
                    layernorm(x2, h2, sc)
                    transpose_apply(h2, h2T, ln2g, ln2b, tp)

                # ============ phase E: FFN ============
                with tc.tile_pool(name="gTP", bufs=1) as gtp:
                    gT = [gtp.tile([P, TS], F32R, tag=f"gT{mf}", name=f"gT{mf}")
                          for mf in range(KF)]
                    MFB = 4     # mf tiles per w1 stream block
                    with tc.tile_pool(name="w1st", bufs=1) as w1p, \
                         tc.tile_pool(name="gPS", bufs=4, space="PSUM") as gps:
                        for blk in range(KF // MFB):
                            if blk < 2:
                                # prefetched during AG2 (w1s0 holds blocks 0-1)
                                w1s = [w1s0[k][:, blk * MFB * P:(blk + 1) * MFB * P]
                                       for k in range(KD)]
                            else:
                                w1t = [w1p.tile([P, MFB * P], F32R,
                                                tag=f"w1b{k}", name=f"w1b{k}",
                                                bufs=2)
                                       for k in range(KD)]
                                for k in range(KD):
                                    nc.sync.dma_start(
                                        w1t[k][:],
                                        t["w1_d"][k * P:(k + 1) * P,
                                                  blk * MFB * P:(blk + 1) * MFB * P])
                                w1s = [w1t[k][:] for k in range(KD)]
                            for j in range(MFB):
                                mf = blk * MFB + j
                                ps = gps.tile([P, TS], F32, tag="g", name="g")
                                for k in range(KD):
                                    nc.tensor.matmul(
                                        ps[:], w1s[k][:, j * P:(j + 1) * P],
                                        h2T[k][:], start=(k == 0),
                                        stop=(k == KD - 1))
                                nc.scalar.activation(gT[mf][:], ps[:],
                                                     AF.Gelu,
                                                     bias=b1p[:, mf:mf + 1])

                    with tc.tile_pool(name="w2st", bufs=4) as w2p, \
                         tc.tile_pool(name="fPS", bufs=1, space="PSUM") as fps, \
                         tc.tile_pool(name="ySB", bufs=2) as ysb:
                        f_ps = [fps.tile([P, D], F32, tag=f"f{mt}", name=f"f{mt}")
                                for mt in range(NT)]
                        for mt in range(NT):
                            for n in range(2):
                                nc.tensor.matmul(
                                    f_ps[mt][:, n * TS:(n + 1) * TS],
                                    ones128[:], b2[:, n * TS:(n + 1) * TS],
                                    start=True, stop=False)
                        for k2 in range(KF):
                            w2t = w2p.tile([P, D], F32R, tag="w2", name="w2")
                            nc.sync.dma_start(
                                w2t[:], t["w2_d"][k2 * P:(k2 + 1) * P, :])
                            for mt in range(NT):
                                for n in range(2):
                                    nc.tensor.matmul(
                                        f_ps[mt][:, n * TS:(n + 1) * TS],
                                        gT[k2][:, mt * P:(mt + 1) * P],
                                        w2t[:, n * TS:(n + 1) * TS],
                                        start=False, stop=(k2 == KF - 1))
                        for mt in range(NT):
                            yt = ysb.tile([P, D], F32, tag="y", name="y")
                            nc.vector.tensor_tensor(out=yt[:],
                                                    in0=f_ps[mt][:],
                                                    in1=x2[mt][:],
                                                    op=ALU.add)
                            nc.sync.dma_start(
                                t["y_d"][mt * P:(mt + 1) * P, :], yt[:])
            pfw1_cm.__exit__(None, None, None)
            xsp_cm.__exit__(None, None, None)


def _in_maps(inputs):
    f32 = np.float32
    maps = []
    for c in range(NC):
        b, r = c // G, c % G
        c0 = r * CC
        m = {
            "x": np.ascontiguousarray(np.asarray(inputs["x"])[b, r * TS:(r + 1) * TS, :], f32),
            "ln1_g": np.ascontiguousarray(inputs["ln1_g"], f32),
            "ln1_b": np.ascontiguousarray(inputs["ln1_b"], f32),
            "Wq": np.ascontiguousarray(np.asarray(inputs["Wq"])[:, c0:c0 + CC], f32),
            "Wk": np.ascontiguousarray(np.asarray(inputs["Wk"])[:, c0:c0 + CC], f32),
            "Wv": np.ascontiguousarray(np.asarray(inputs["Wv"])[:, c0:c0 + CC], f32),
            "bq": np.ascontiguousarray(np.asarray(inputs["bq"])[c0:c0 + CC], f32),
            "bk": np.ascontiguousarray(np.asarray(inputs["bk"])[c0:c0 + CC], f32),
            "bv": np.ascontiguousarray(np.asarray(inputs["bv"])[c0:c0 + CC], f32),
            "Wo": np.ascontiguousarray(inputs["Wo"], f32),
            "bo": np.ascontiguousarray(inputs["bo"], f32),
            "ln2_g": np.ascontiguousarray(inputs["ln2_g"], f32),
            "ln2_b": np.ascontiguousarray(inputs["ln2_b"], f32),
            "W1": np.ascontiguousarray(inputs["W1"], f32),
            "b1": np.ascontiguousarray(inputs["b1"], f32),
            "W2": np.ascontiguousarray(inputs["W2"], f32),
            "b2": np.ascontiguousarray(inputs["b2"], f32),
            "coff": np.array([[r * TS]], dtype=np.int32),
        }
        maps.append(m)
    return maps


def _run(inputs, trace=False):
    if "nc" not in _CACHE:
        _CACHE["nc"] = build()
    nc = _CACHE["nc"]
    maps = _in_maps(inputs)
    res = run_bass_kernel_spmd(nc, maps, list(range(NC)), trace=trace)
    out = np.empty((B, S, D), np.float32)
    for c in range(NC):
        b, r = c // G, c % G
        out[b, r * TS:(r + 1) * TS, :] = res.results[c]["y"]
    return out, res


def kernel(**inputs):
    out, _ = _run(inputs, trace=False)
    return out


if __name__ == "__main__":
    build()
    print("build OK")


# revision 27
# speedup vs baseline: 45.3427x; 1.0004x over previous
"""Trainium2 Bass kernel for a dense transformer block (B=2,S=2048,D=1024,H=16,DFF=4096).

Sharding across 8 NeuronCores:
  core c: batch b=c//4, group rank r=c%4, replica groups [[0,1,2,3],[4,5,6,7]].
  - LN1 computed on own 512-token strip; hT AllGather'ed within the 4-core group.
  - Attention: head-parallel (4 heads/core, full causal sequence).
  - AllToAll redistributes attention output from head-sharded to token-sharded.
  - out_proj, LN2, FFN: token-sharded (512 tokens/core), full weights.
Matmuls run in float32r (TF32-like, full PE rate); the residual spine stays fp32.
"""
import sys

sys.path.insert(0, "/opt/trn_rl_repo")

import numpy as np

import concourse.bass as bass
import concourse.mybir as mybir
import concourse.tile as tile
from concourse import bacc
from concourse.bass_utils import run_bass_kernel_spmd
from concourse.masks import make_identity

AF = mybir.ActivationFunctionType
ALU = mybir.AluOpType
F32 = mybir.dt.float32
F32R = mybir.dt.float32r

B, S, D, H = 2, 2048, 1024, 16
DH = D // H          # 64
DFF = 4 * D          # 4096
EPS = 1e-5
NC = 8               # cores
G = 4                # cores per group (per batch)
TS = S // G          # 512 tokens per core
HC = H // G          # 4 heads per core
CC = HC * DH         # 256 head-columns per core
P = 128
KD = D // P          # 8 k-tiles over D
KF = DFF // P        # 32 k-tiles over DFF
NT = TS // P         # 4 token tiles per strip
GROUPS = [[0, 1, 2, 3], [4, 5, 6, 7]]

_CACHE = {}


def build():
    nc = bacc.Bacc(None)

    io = {}
    io["x_d"] = nc.declare_dram_parameter("x", [TS, D], F32, isOutput=False)
    io["ln1g_d"] = nc.declare_dram_parameter("ln1_g", [D], F32, isOutput=False)
    io["ln1b_d"] = nc.declare_dram_parameter("ln1_b", [D], F32, isOutput=False)
    io["wq_d"] = nc.declare_dram_parameter("Wq", [D, CC], F32R, isOutput=False)
    io["wk_d"] = nc.declare_dram_parameter("Wk", [D, CC], F32R, isOutput=False)
    io["wv_d"] = nc.declare_dram_parameter("Wv", [D, CC], F32R, isOutput=False)
    io["bq_d"] = nc.declare_dram_parameter("bq", [CC], F32R, isOutput=False)
    io["bk_d"] = nc.declare_dram_parameter("bk", [CC], F32R, isOutput=False)
    io["bv_d"] = nc.declare_dram_parameter("bv", [CC], F32R, isOutput=False)
    io["wo_d"] = nc.declare_dram_parameter("Wo", [D, D], F32R, isOutput=False)
    io["bo_d"] = nc.declare_dram_parameter("bo", [D], F32R, isOutput=False)
    io["ln2g_d"] = nc.declare_dram_parameter("ln2_g", [D], F32, isOutput=False)
    io["ln2b_d"] = nc.declare_dram_parameter("ln2_b", [D], F32, isOutput=False)
    io["w1_d"] = nc.declare_dram_parameter("W1", [D, DFF], F32R, isOutput=False)
    io["b1_d"] = nc.declare_dram_parameter("b1", [DFF], F32R, isOutput=False)
    io["w2_d"] = nc.declare_dram_parameter("W2", [DFF, D], F32R, isOutput=False)
    io["b2_d"] = nc.declare_dram_parameter("b2", [D], F32R, isOutput=False)
    io["y_d"] = nc.declare_dram_parameter("y", [TS, D], F32, isOutput=True)

    io["ag1_in"] = nc.dram_tensor("ag1_in", [D, TS], F32R)
    io["ag1_out"] = nc.dram_tensor("ag1_out", [G, D, TS], F32R)
    io["a2a_in"] = nc.dram_tensor("a2a_in", [CC, S], F32R)
    io["a2a_out"] = nc.dram_tensor("a2a_out", [G, CC, S], F32R)
    io["coff_d"] = nc.declare_dram_parameter("coff", [1, 1], mybir.dt.int32,
                                             isOutput=False)

    with tile.TileContext(nc) as tc:
        _body(nc, tc, io)
    nc.compile()
    return nc


def _body(nc, tc, t):
    with tc.tile_pool(name="const", bufs=1) as cst:
        # x strip loads first: they gate the LN1 -> transpose -> AG1 chain
        xsp_cm = tc.tile_pool(name="xsP", bufs=1)
        xsp = xsp_cm.__enter__()
        xs = [xsp.tile([P, D], F32, tag=f"xs{mt}", name=f"xs{mt}")
              for mt in range(NT)]
        for mt in range(NT):
            nc.sync.dma_start(xs[mt][:], t["x_d"][mt * P:(mt + 1) * P, :])

        # ---------------- constants ----------------
        ident = cst.tile([P, P], F32)
        make_identity(nc, ident[:])

        onesrow_f = cst.tile([1, TS], F32)
        nc.gpsimd.memset(onesrow_f[:], 1.0)
        ones128 = cst.tile([1, P], F32R)        # K=1 lhsT (M=128 tokens)
        nc.vector.tensor_copy(ones128[:], onesrow_f[0:1, 0:P])
        onescol4 = cst.tile([P, HC, 1], F32)
        nc.gpsimd.memset(onescol4[:], 1.0)
        epsc = cst.tile([P, 1], F32)
        nc.gpsimd.memset(epsc[:], EPS)

        # doubled causal masks (one per diagonal shift), mask||mask layout so a
        # single DVE op masks a two-head [128, 1024] pair tile.
        maskd = {}
        for sh in (0, -128, -256, -384):
            md = cst.tile([P, 2 * TS], F32, tag=f"maskd{sh}", name=f"maskd{sh}")
            nc.gpsimd.memset(md[:], 1.0)
            for half in range(2):
                nc.gpsimd.affine_select(
                    out=md[:, half * TS:(half + 1) * TS],
                    in_=md[:, half * TS:(half + 1) * TS],
                    compare_op=ALU.is_ge, fill=0.0, base=sh,
                    pattern=[[1, TS]], channel_multiplier=-1,
                )
            maskd[sh] = md

        # layernorm gains/biases as [128, KD] (per-partition per k-tile)
        ln1g = cst.tile([P, KD], F32)
        ln1b = cst.tile([P, KD], F32)
        ln2g = cst.tile([P, KD], F32)
        ln2b = cst.tile([P, KD], F32)
        nc.sync.dma_start(ln1g[:], t["ln1g_d"].rearrange("(k p) -> p k", p=P))
        nc.sync.dma_start(ln1b[:], t["ln1b_d"].rearrange("(k p) -> p k", p=P))
        nc.sync.dma_start(ln2g[:], t["ln2g_d"].rearrange("(k p) -> p k", p=P))
        nc.sync.dma_start(ln2b[:], t["ln2b_d"].rearrange("(k p) -> p k", p=P))

        # bq/bk as per-partition [128, 2] (column-tile-major) for psum eviction
        bqp = cst.tile([P, 2], F32)
        bkp = cst.tile([P, 2], F32)
        nc.gpsimd.dma_start(bqp[:], t["bq_d"].rearrange("(m p) -> p m", p=P))
        nc.gpsimd.dma_start(bkp[:], t["bk_d"].rearrange("(m p) -> p m", p=P))
        # bv broadcast across partitions for the v eviction add
        bvrow = cst.tile([1, CC], F32)
        nc.gpsimd.dma_start(bvrow[:], t["bv_d"][None, :])
        bvb = cst.tile([P, CC], F32)
        nc.gpsimd.partition_broadcast(bvb[:], bvrow[:])
        # b1 as per-partition [128, KF] for the gelu bias operand
        b1p = cst.tile([P, KF], F32)
        nc.gpsimd.dma_start(b1p[:], t["b1_d"].rearrange("(k p) -> p k", p=P))
        bo = cst.tile([1, D], F32R)
        b2 = cst.tile([1, D], F32R)
        nc.sync.dma_start(bo[:], t["bo_d"][None, :])
        nc.sync.dma_start(b2[:], t["b2_d"][None, :])

        # ---------------- helpers ----------------
        def layernorm(src_tiles, dst_tiles, sc):
            # var = E[x^2] - mu^2 (safe: |mu| << std for this data), so the
            # normalize is a single fused (x - mu) * inv DVE pass.
            for mt in range(NT):
                xt = src_tiles[mt]
                mu = sc.tile([P, 1], F32, tag="mu", name="mu")
                nc.vector.tensor_reduce(out=mu[:], in_=xt[:], op=ALU.add,
                                        axis=mybir.AxisListType.X)
                mus = sc.tile([P, 1], F32, tag="mus", name="mus")
                nc.scalar.mul(mus[:], mu[:], 1.0 / D)
                sq = sc.tile([P, D], F32, tag="sq", name="sq")
                sumsq = sc.tile([P, 1], F32, tag="sumsq", name="sumsq")
                nc.scalar.activation(sq[:], xt[:], AF.Square, accum_out=sumsq[:])
                mu2 = sc.tile([P, 1], F32, tag="mu2", name="mu2")
                nc.scalar.activation(mu2[:], mus[:], AF.Square)
                vpe = sc.tile([P, 1], F32, tag="vpe", name="vpe")
                # vpe = sumsq/D - mu2 + eps  (two tiny fused scalar ops)
                nc.vector.tensor_scalar(out=vpe[:], in0=sumsq[:],
                                        scalar1=1.0 / D, scalar2=mu2[:],
                                        op0=ALU.mult, op1=ALU.subtract)
                std = sc.tile([P, 1], F32, tag="std", name="std")
                nc.scalar.activation(std[:], vpe[:], AF.Sqrt, bias=epsc[:])
                inv = sc.tile([P, 1], F32, tag="inv", name="inv")
                nc.vector.reciprocal(inv[:], std[:])
                nc.vector.tensor_scalar(out=dst_tiles[mt][:], in0=xt[:],
                                        scalar1=mus[:], scalar2=inv[:],
                                        op0=ALU.subtract, op1=ALU.mult)

        def transpose_apply(src_tiles, dst_tiles, g_sb, b_sb, tp):
            for k in range(KD):
                for mt in range(NT):
                    ps = tp.tile([P, P], F32, tag="tps", name="tps")
                    nc.tensor.transpose(ps[:], src_tiles[mt][:, k * P:(k + 1) * P],
                                        ident[:])
                    nc.vector.tensor_scalar(
                        out=dst_tiles[k][:, mt * P:(mt + 1) * P], in0=ps[:],
                        scalar1=g_sb[:, k:k + 1], scalar2=b_sb[:, k:k + 1],
                        op0=ALU.mult, op1=ALU.add)

        # ============ phase A: LN1 + transpose + AllGather ============
        if True:
            with tc.tile_pool(name="lnA", bufs=1) as sc, \
                 tc.tile_pool(name="tpA", bufs=4, space="PSUM") as tp:
                h = [sc.tile([P, D], F32, tag=f"h{mt}", name=f"h{mt}") for mt in range(NT)]
                layernorm(xs, h, sc)
                hT = [sc.tile([P, TS], F32R, tag=f"hT{k}", name=f"hT{k}") for k in range(KD)]
                transpose_apply(h, hT, ln1g, ln1b, tp)
                for k in range(KD):
                    nc.sync.dma_start(t["ag1_in"][k * P:(k + 1) * P, :], hT[k][:])

            # ============ phases B+C: QKV + attention ============
            with tc.tile_pool(name="qkvP", bufs=1) as qkv:
                qT = [qkv.tile([P, S], F32R, tag=f"qT{m}", name=f"qT{m}") for m in range(2)]
                kT = [qkv.tile([P, S], F32R, tag=f"kT{m}", name=f"kT{m}") for m in range(2)]
                vo = [qkv.tile([P, HC, DH + 1], F32R, tag=f"vo{tm}", name=f"vo{tm}")
                      for tm in range(S // P)]

                wp_cm = tc.tile_pool(name="wqkv", bufs=1)
                wp = wp_cm.__enter__()
                # weight loads traced before the collective: DMA covers AG1
                wq = [wp.tile([P, CC], F32R, tag=f"wq{k}", name=f"wq{k}") for k in range(KD)]
                wk = [wp.tile([P, CC], F32R, tag=f"wk{k}", name=f"wk{k}") for k in range(KD)]
                wv = [wp.tile([P, CC], F32R, tag=f"wv{k}", name=f"wv{k}") for k in range(KD)]
                for k in range(KD):
                    nc.sync.dma_start(wq[k][:], t["wq_d"][k * P:(k + 1) * P, :])
                    nc.sync.dma_start(wk[k][:], t["wk_d"][k * P:(k + 1) * P, :])
                    nc.sync.dma_start(wv[k][:], t["wv_d"][k * P:(k + 1) * P, :])

                nc.gpsimd.collective_compute(
                    "AllGather", ALU.bypass, ins=[t["ag1_in"][:]],
                    outs=[t["ag1_out"][:]], replica_groups=GROUPS,
                )

                with tc.tile_pool(name="hTst", bufs=2) as st, \
                     tc.tile_pool(name="projPS", bufs=3, space="PSUM") as pps:
                    for qc in range(G):
                        hTq = [st.tile([P, TS], F32R, tag=f"hTq{k}", name=f"hTq{k}")
                               for k in range(KD)]
                        for k in range(KD):
                            nc.sync.dma_start(
                                hTq[k][:], t["ag1_out"][qc, k * P:(k + 1) * P, :])
                        for (w_sb, b_sb, out_sb) in ((wq, bqp, qT), (wk, bkp, kT)):
                            for m in range(2):
                                ps = pps.tile([P, TS], F32, tag="pps", name="pps")
                                for k in range(KD):
                                    nc.tensor.matmul(
                                        ps[:], w_sb[k][:, m * P:(m + 1) * P],
                                        hTq[k][:], start=(k == 0),
                                        stop=(k == KD - 1))
                                nc.vector.tensor_scalar(
                                    out=out_sb[m][:, qc * TS:(qc + 1) * TS],
                                    in0=ps[:], scalar1=b_sb[:, m:m + 1],
                                    scalar2=None, op0=ALU.add)
                        for mt in range(NT):
                            tm = qc * NT + mt
                            ps = pps.tile([P, CC], F32, tag="vps", name="vps")
                            for k in range(KD):
                                nc.tensor.matmul(
                                    ps[:], hTq[k][:, mt * P:(mt + 1) * P],
                                    wv[k][:], start=(k == 0), stop=(k == KD - 1))
                            nc.vector.tensor_tensor(
                                out=vo[tm][:, :, 0:DH],
                                in0=ps[:].rearrange("p (h e) -> p h e", h=HC),
                                in1=bvb[:].rearrange("p (h e) -> p h e", h=HC),
                                op=ALU.add)
                            nc.vector.tensor_copy(vo[tm][:, :, DH:DH + 1],
                                                  onescol4[:])

                wp_cm.__exit__(None, None, None)

                # ---- attention ----
                with (
                    tc.tile_pool(name="scPS", bufs=2, space="PSUM") as scp,
                    tc.tile_pool(name="avPS", bufs=2, space="PSUM") as avp,
                    tc.tile_pool(name="attnSB", bufs=3) as asb,
                ):
                    for hp in range(HC // 2):      # head pairs at PE rows 0/64
                        for qc in range(G):
                            kt_max = 4 * qc + 3
                            avs = [avp.tile([DH + 1, TS], F32, tag=f"av{j}",
                                            name=f"av{j}") for j in range(2)]
                            for kt in range(kt_max + 1):
                                # diag blocks: only columns >= v0 are ever read
                                w0 = P * max(0, kt - 4 * qc)
                                # both heads' score blocks into one 2-bank tile
                                sc_ps = scp.tile([P, 2, TS], F32,
                                                 tag="scp", name="scp")
                                for j in range(2):
                                    h_i = 2 * hp + j
                                    m = h_i // 2
                                    o = (h_i % 2) * DH
                                    nc.tensor.matmul(
                                        sc_ps[:, j, w0:],
                                        kT[m][o:o + DH, kt * P:(kt + 1) * P],
                                        qT[m][o:o + DH,
                                              qc * TS + w0:(qc + 1) * TS],
                                        start=True, stop=True)
                                e_r = asb.tile([P, 2, TS], F32R,
                                               tag="erp", name="erp")
                                if kt < 4 * qc:
                                    v0 = 0      # valid columns start
                                    nc.scalar.activation(
                                        e_r[:].rearrange("p a b -> p (a b)"),
                                        sc_ps[:].rearrange("p a b -> p (a b)"),
                                        AF.Exp, scale=0.125)
                                else:
                                    # diag block, shift s=-128*d: cols < 128*d
                                    # are fully masked -- never compute/read them
                                    d = kt - 4 * qc
                                    v0 = P * d
                                    e_f = asb.tile([P, 2, TS], F32,
                                                   tag="efp", name="efp")
                                    nc.scalar.activation(
                                        e_f[:, :, v0:], sc_ps[:, :, v0:],
                                        AF.Exp, scale=0.125)
                                    sh = 512 * qc - 128 * kt
                                    mdv = maskd[sh][:].rearrange(
                                        "p (a b) -> p a b", a=2)
                                    nc.vector.tensor_tensor(
                                        out=e_r[:, :, v0:],
                                        in0=e_f[:, :, v0:],
                                        in1=mdv[:, :, v0:],
                                        op=ALU.mult)
                                for j in range(2):
                                    h_i = 2 * hp + j
                                    nc.tensor.matmul(avs[j][:, v0:],
                                                     vo[kt][:, h_i, :],
                                                     e_r[:, j, v0:],
                                                     start=(kt == 0),
                                                     stop=(kt == kt_max))
                            for j in range(2):
                                h_i = 2 * hp + j
                                un = asb.tile([DH + 1, TS], F32,
                                              tag=f"un{j}", name=f"un{j}")
                                nc.vector.tensor_copy(un[:], avs[j][:])
                                rec = asb.tile([1, TS], F32,
                                               tag=f"rec{j}", name=f"rec{j}")
                                nc.vector.reciprocal(rec[:], un[DH:DH + 1, :])
                                rb = asb.tile([DH, TS], F32,
                                              tag=f"rb{j}", name=f"rb{j}")
                                nc.gpsimd.partition_broadcast(rb[:], rec[:])
                                chunk = asb.tile([DH, TS], F32R,
                                                 tag=f"chunk{j}", name=f"chunk{j}")
                                nc.vector.tensor_tensor(out=chunk[:],
                                                        in0=un[0:DH, :],
                                                        in1=rb[:], op=ALU.mult)
                                nc.sync.dma_start(
                                    t["a2a_in"][h_i * DH:(h_i + 1) * DH,
                                                qc * TS:(qc + 1) * TS],
                                    chunk[:])

            # prefetch pools traced before AG2 so DMA covers the collective
            pfw1_cm = tc.tile_pool(name="pfW1", bufs=1)
            pfw1 = pfw1_cm.__enter__()
            w1s0 = [pfw1.tile([P, 8 * P], F32R, tag=f"w1s{k}", name=f"w1s{k}")
                    for k in range(KD)]
            pfd_cm = tc.tile_pool(name="pfD", bufs=1)
            pfd = pfd_cm.__enter__()
            wo = [pfd.tile([P, D], F32R, tag=f"wo{k}", name=f"wo{k}") for k in range(KD)]
            for k in range(KD):
                nc.sync.dma_start(wo[k][:], t["wo_d"][k * P:(k + 1) * P, :])
                nc.sync.dma_start(w1s0[k][:], t["w1_d"][k * P:(k + 1) * P, 0:8 * P])

            nc.gpsimd.collective_compute(
                "AllGather", ALU.bypass, ins=[t["a2a_in"][:]],
                outs=[t["a2a_out"][:]], replica_groups=GROUPS,
            )

            # ============ phase D: out_proj + residual (in-place on xs) ====
            with tc.tile_pool(name="opPS", bufs=3, space="PSUM") as opp, \
                 tc.tile_pool(name="opSB", bufs=1) as osb:
                aT = [osb.tile([P, TS], F32R, tag=f"aT{k}", name=f"aT{k}") for k in range(KD)]
                off_sb = osb.tile([1, 1], mybir.dt.int32, tag="off", name="off")
                nc.sync.dma_start(off_sb[:], t["coff_d"][:])
                with nc.gpsimd.register("roff") as roff:
                    nc.gpsimd.reg_load(roff, off_sb[0:1, 0:1])
                    rv = nc.snap(roff)
                    for k in range(KD):
                        nc.gpsimd.dma_start(
                            aT[k][:],
                            t["a2a_out"][k // 2, (k % 2) * P:(k % 2 + 1) * P,
                                         bass.ds(rv, TS)])
                for mt in range(NT):
                    for n in range(2):
                        ps = opp.tile([P, TS], F32, tag="op", name="op")
                        nc.tensor.matmul(ps[:], ones128[:],
                                         bo[:, n * TS:(n + 1) * TS],
                                         start=True, stop=False)
                        for k in range(KD):
                            nc.tensor.matmul(
                                ps[:], aT[k][:, mt * P:(mt + 1) * P],
                                wo[k][:, n * TS:(n + 1) * TS],
                                start=False, stop=(k == KD - 1))
                        # residual written in place: xs becomes x2
                        nc.vector.tensor_tensor(
                            out=xs[mt][:, n * TS:(n + 1) * TS], in0=ps[:],
                            in1=xs[mt][:, n * TS:(n + 1) * TS], op=ALU.add)
            pfd_cm.__exit__(None, None, None)
            x2 = xs

            with tc.tile_pool(name="h2TP", bufs=1) as h2tp:
                h2T = [h2tp.tile([P, TS], F32R, tag=f"h2T{k}", name=f"h2T{k}")
                       for k in range(KD)]
                with tc.tile_pool(name="lnD", bufs=1) as sc, \
                     tc.tile_pool(name="tpD", bufs=4, space="PSUM") as tp:
                    h2 = [sc.tile([P, D], F32, tag=f"h2{mt}", name=f"h2{mt}")
                          for mt in range(NT)]
                    layernorm(x2, h2, sc)
                    transpose_apply(h2, h2T, ln2g, ln2b, tp)

                # ============ phase E: FFN ============
                with tc.tile_pool(name="gTP", bufs=1) as gtp:
                    gT = [gtp.tile([P, TS], F32R, tag=f"gT{mf}", name=f"gT{mf}")
                          for mf in range(KF)]
                    MFB = 4     # mf tiles per w1 stream block
                    with tc.tile_pool(name="w1st", bufs=1) as w1p, \
                         tc.tile_pool(name="gPS", bufs=4, space="PSUM") as gps:
                        for blk in range(KF // MFB):
                            if blk < 2:
                                # prefetched during AG2 (w1s0 holds blocks 0-1)
                                w1s = [w1s0[k][:, blk * MFB * P:(blk + 1) * MFB * P]
                                       for k in range(KD)]
                            else:
                                w1t = [w1p.tile([P, MFB * P], F32R,
                                                tag=f"w1b{k}", name=f"w1b{k}",
                                                bufs=2)
                                       for k in range(KD)]
                                for k in range(KD):
                                    nc.sync.dma_start(
                                        w1t[k][:],
                                        t["w1_d"][k * P:(k + 1) * P,
                                                  blk * MFB * P:(blk + 1) * MFB * P])
                                w1s = [w1t[k][:] for k in range(KD)]
                            for j in range(MFB):
                                mf = blk * MFB + j
                                ps = gps.tile([P, TS], F32, tag="g", name="g")
                                for k in range(KD):
                                    nc.tensor.matmul(
                                        ps[:], w1s[k][:, j * P:(j + 1) * P],
                                        h2T[k][:], start=(k == 0),
                                        stop=(k == KD - 1))
                                nc.scalar.activation(gT[mf][:], ps[:],
                                                     AF.Gelu,
                                                     bias=b1p[:, mf:mf + 1])

                    with tc.tile_pool(name="w2st", bufs=4) as w2p, \
                         tc.tile_pool(name="fPS", bufs=1, space="PSUM") as fps, \
                         tc.tile_pool(name="ySB", bufs=2) as ysb:
                        f_ps = [fps.tile([P, D], F32, tag=f"f{mt}", name=f"f{mt}")
                                for mt in range(NT)]
                        for mt in range(NT):
                            for n in range(2):
                                nc.tensor.matmul(
                                    f_ps[mt][:, n * TS:(n + 1) * TS],
                                    ones128[:], b2[:, n * TS:(n + 1) * TS],
                                    start=True, stop=False)
                        for k2 in range(KF):
                            w2t = w2p.tile([P, D], F32R, tag="w2", name="w2")
                            nc.sync.dma_start(
                                w2t[:], t["w2_d"][k2 * P:(k2 + 1) * P, :])
                            for mt in range(NT):
                                for n in range(2):
                                    nc.tensor.matmul(
                                        f_ps[mt][:, n * TS:(n + 1) * TS],
                                        gT[k2][:, mt * P:(mt + 1) * P],
                                        w2t[:, n * TS:(n + 1) * TS],
                                        start=False, stop=(k2 == KF - 1))
                        for mt in range(NT):
                            yt = ysb.tile([P, D], F32, tag="y", name="y")
                            nc.vector.tensor_tensor(out=yt[:],
                                                    in0=f_ps[mt][:],
                                                    in1=x2[mt][:],
                                                    op=ALU.add)
                            nc.sync.dma_start(
                                t["y_d"][mt * P:(mt + 1) * P, :], yt[:])
            pfw1_cm.__exit__(None, None, None)
            xsp_cm.__exit__(None, None, None)


def _in_maps(inputs):
    f32 = np.float32
    maps = []
    for c in range(NC):
        b, r = c // G, c % G
        c0 = r * CC
        m = {
            "x": np.ascontiguousarray(np.asarray(inputs["x"])[b, r * TS:(r + 1) * TS, :], f32),
            "ln1_g": np.ascontiguousarray(inputs["ln1_g"], f32),
            "ln1_b": np.ascontiguousarray(inputs["ln1_b"], f32),
            "Wq": np.ascontiguousarray(np.asarray(inputs["Wq"])[:, c0:c0 + CC], f32),
            "Wk": np.ascontiguousarray(np.asarray(inputs["Wk"])[:, c0:c0 + CC], f32),
            "Wv": np.ascontiguousarray(np.asarray(inputs["Wv"])[:, c0:c0 + CC], f32),
            "bq": np.ascontiguousarray(np.asarray(inputs["bq"])[c0:c0 + CC], f32),
            "bk": np.ascontiguousarray(np.asarray(inputs["bk"])[c0:c0 + CC], f32),
            "bv": np.ascontiguousarray(np.asarray(inputs["bv"])[c0:c0 + CC], f32),
            "Wo": np.ascontiguousarray(inputs["Wo"], f32),
            "bo": np.ascontiguousarray(inputs["bo"], f32),
            "ln2_g": np.ascontiguousarray(inputs["ln2_g"], f32),
            "ln2_b": np.ascontiguousarray(inputs["ln2_b"], f32),
            "W1": np.ascontiguousarray(inputs["W1"], f32),
            "b1": np.ascontiguousarray(inputs["b1"], f32),
            "W2": np.ascontiguousarray(inputs["W2"], f32),
            "b2": np.ascontiguousarray(inputs["b2"], f32),
            "coff": np.array([[r * TS]], dtype=np.int32),
        }
        maps.append(m)
    return maps


def _run(inputs, trace=False):
    if "nc" not in _CACHE:
        _CACHE["nc"] = build()
    nc = _CACHE["nc"]
    maps = _in_maps(inputs)
    res = run_bass_kernel_spmd(nc, maps, list(range(NC)), trace=trace)
    out = np.empty((B, S, D), np.float32)
    for c in range(NC):
        b, r = c // G, c % G
        out[b, r * TS:(r + 1) * TS, :] = res.results[c]["y"]
    return out, res


def kernel(**inputs):
    out, _ = _run(inputs, trace=False)
    return out


if __name__ == "__main__":
    build()
    print("build OK")


# revision 28
# speedup vs baseline: 45.3550x; 1.0003x over previous
"""Trainium2 Bass kernel for a dense transformer block (B=2,S=2048,D=1024,H=16,DFF=4096).

Sharding across 8 NeuronCores:
  core c: batch b=c//4, group rank r=c%4, replica groups [[0,1,2,3],[4,5,6,7]].
  - LN1 computed on own 512-token strip; hT AllGather'ed within the 4-core group.
  - Attention: head-parallel (4 heads/core, full causal sequence).
  - AllToAll redistributes attention output from head-sharded to token-sharded.
  - out_proj, LN2, FFN: token-sharded (512 tokens/core), full weights.
Matmuls run in float32r (TF32-like, full PE rate); the residual spine stays fp32.
"""
import sys

sys.path.insert(0, "/opt/trn_rl_repo")

import numpy as np

import concourse.bass as bass
import concourse.mybir as mybir
import concourse.tile as tile
from concourse import bacc
from concourse.bass_utils import run_bass_kernel_spmd
from concourse.masks import make_identity

AF = mybir.ActivationFunctionType
ALU = mybir.AluOpType
F32 = mybir.dt.float32
F32R = mybir.dt.float32r

B, S, D, H = 2, 2048, 1024, 16
DH = D // H          # 64
DFF = 4 * D          # 4096
EPS = 1e-5
NC = 8               # cores
G = 4                # cores per group (per batch)
TS = S // G          # 512 tokens per core
HC = H // G          # 4 heads per core
CC = HC * DH         # 256 head-columns per core
P = 128
KD = D // P          # 8 k-tiles over D
KF = DFF // P        # 32 k-tiles over DFF
NT = TS // P         # 4 token tiles per strip
GROUPS = [[0, 1, 2, 3], [4, 5, 6, 7]]

_CACHE = {}


def build():
    nc = bacc.Bacc(None)

    io = {}
    io["x_d"] = nc.declare_dram_parameter("x", [TS, D], F32, isOutput=False)
    io["ln1g_d"] = nc.declare_dram_parameter("ln1_g", [D], F32, isOutput=False)
    io["ln1b_d"] = nc.declare_dram_parameter("ln1_b", [D], F32, isOutput=False)
    io["wq_d"] = nc.declare_dram_parameter("Wq", [D, CC], F32R, isOutput=False)
    io["wk_d"] = nc.declare_dram_parameter("Wk", [D, CC], F32R, isOutput=False)
    io["wv_d"] = nc.declare_dram_parameter("Wv", [D, CC], F32R, isOutput=False)
    io["bq_d"] = nc.declare_dram_parameter("bq", [CC], F32R, isOutput=False)
    io["bk_d"] = nc.declare_dram_parameter("bk", [CC], F32R, isOutput=False)
    io["bv_d"] = nc.declare_dram_parameter("bv", [CC], F32R, isOutput=False)
    io["wo_d"] = nc.declare_dram_parameter("Wo", [D, D], F32R, isOutput=False)
    io["bo_d"] = nc.declare_dram_parameter("bo", [D], F32R, isOutput=False)
    io["ln2g_d"] = nc.declare_dram_parameter("ln2_g", [D], F32, isOutput=False)
    io["ln2b_d"] = nc.declare_dram_parameter("ln2_b", [D], F32, isOutput=False)
    io["w1_d"] = nc.declare_dram_parameter("W1", [D, DFF], F32R, isOutput=False)
    io["b1_d"] = nc.declare_dram_parameter("b1", [DFF], F32R, isOutput=False)
    io["w2_d"] = nc.declare_dram_parameter("W2", [DFF, D], F32R, isOutput=False)
    io["b2_d"] = nc.declare_dram_parameter("b2", [D], F32R, isOutput=False)
    io["y_d"] = nc.declare_dram_parameter("y", [TS, D], F32, isOutput=True)

    io["ag1_in"] = nc.dram_tensor("ag1_in", [D, TS], F32R)
    io["ag1_out"] = nc.dram_tensor("ag1_out", [G, D, TS], F32R)
    io["a2a_in"] = nc.dram_tensor("a2a_in", [CC, S], F32R)
    io["a2a_out"] = nc.dram_tensor("a2a_out", [G, CC, S], F32R)
    io["coff_d"] = nc.declare_dram_parameter("coff", [1, 1], mybir.dt.int32,
                                             isOutput=False)

    with tile.TileContext(nc) as tc:
        _body(nc, tc, io)
    nc.compile()
    return nc


def _body(nc, tc, t):
    with tc.tile_pool(name="const", bufs=1) as cst:
        # x strip loads first: they gate the LN1 -> transpose -> AG1 chain
        xsp_cm = tc.tile_pool(name="xsP", bufs=1)
        xsp = xsp_cm.__enter__()
        xs = [xsp.tile([P, D], F32, tag=f"xs{mt}", name=f"xs{mt}")
              for mt in range(NT)]
        for mt in range(NT):
            nc.sync.dma_start(xs[mt][:], t["x_d"][mt * P:(mt + 1) * P, :])

        # ---------------- constants ----------------
        ident = cst.tile([P, P], F32)
        make_identity(nc, ident[:])

        onesrow_f = cst.tile([1, TS], F32)
        nc.gpsimd.memset(onesrow_f[:], 1.0)
        ones128 = cst.tile([1, P], F32R)        # K=1 lhsT (M=128 tokens)
        nc.vector.tensor_copy(ones128[:], onesrow_f[0:1, 0:P])
        onescol4 = cst.tile([P, HC, 1], F32)
        nc.gpsimd.memset(onescol4[:], 1.0)
        epsc = cst.tile([P, 1], F32)
        nc.gpsimd.memset(epsc[:], EPS)

        # doubled causal masks (one per diagonal shift), mask||mask layout so a
        # single DVE op masks a two-head [128, 1024] pair tile.
        maskd = {}
        for sh in (0, -128, -256, -384):
            md = cst.tile([P, 2 * TS], F32, tag=f"maskd{sh}", name=f"maskd{sh}")
            nc.gpsimd.memset(md[:], 1.0)
            for half in range(2):
                nc.gpsimd.affine_select(
                    out=md[:, half * TS:(half + 1) * TS],
                    in_=md[:, half * TS:(half + 1) * TS],
                    compare_op=ALU.is_ge, fill=0.0, base=sh,
                    pattern=[[1, TS]], channel_multiplier=-1,
                )
            maskd[sh] = md

        # layernorm gains/biases as [128, KD] (per-partition per k-tile)
        ln1g = cst.tile([P, KD], F32)
        ln1b = cst.tile([P, KD], F32)
        ln2g = cst.tile([P, KD], F32)
        ln2b = cst.tile([P, KD], F32)
        nc.sync.dma_start(ln1g[:], t["ln1g_d"].rearrange("(k p) -> p k", p=P))
        nc.sync.dma_start(ln1b[:], t["ln1b_d"].rearrange("(k p) -> p k", p=P))
        nc.sync.dma_start(ln2g[:], t["ln2g_d"].rearrange("(k p) -> p k", p=P))
        nc.sync.dma_start(ln2b[:], t["ln2b_d"].rearrange("(k p) -> p k", p=P))

        # bq/bk as per-partition [128, 2] (column-tile-major) for psum eviction
        bqp = cst.tile([P, 2], F32)
        bkp = cst.tile([P, 2], F32)
        nc.gpsimd.dma_start(bqp[:], t["bq_d"].rearrange("(m p) -> p m", p=P))
        nc.gpsimd.dma_start(bkp[:], t["bk_d"].rearrange("(m p) -> p m", p=P))
        # bv broadcast across partitions for the v eviction add
        bvrow = cst.tile([1, CC], F32)
        nc.gpsimd.dma_start(bvrow[:], t["bv_d"][None, :])
        bvb = cst.tile([P, CC], F32)
        nc.gpsimd.partition_broadcast(bvb[:], bvrow[:])
        # b1 as per-partition [128, KF] for the gelu bias operand
        b1p = cst.tile([P, KF], F32)
        nc.gpsimd.dma_start(b1p[:], t["b1_d"].rearrange("(k p) -> p k", p=P))
        bo = cst.tile([1, D], F32R)
        b2 = cst.tile([1, D], F32R)
        nc.sync.dma_start(bo[:], t["bo_d"][None, :])
        nc.sync.dma_start(b2[:], t["b2_d"][None, :])

        # ---------------- helpers ----------------
        def layernorm(src_tiles, dst_tiles, sc):
            # var = E[x^2] - mu^2 (safe: |mu| << std for this data), so the
            # normalize is a single fused (x - mu) * inv DVE pass.
            for mt in range(NT):
                xt = src_tiles[mt]
                mu = sc.tile([P, 1], F32, tag="mu", name="mu")
                nc.vector.tensor_reduce(out=mu[:], in_=xt[:], op=ALU.add,
                                        axis=mybir.AxisListType.X)
                mus = sc.tile([P, 1], F32, tag="mus", name="mus")
                nc.scalar.mul(mus[:], mu[:], 1.0 / D)
                sq = sc.tile([P, D], F32, tag="sq", name="sq")
                sumsq = sc.tile([P, 1], F32, tag="sumsq", name="sumsq")
                nc.scalar.activation(sq[:], xt[:], AF.Square, accum_out=sumsq[:])
                mu2 = sc.tile([P, 1], F32, tag="mu2", name="mu2")
                nc.scalar.activation(mu2[:], mus[:], AF.Square)
                vpe = sc.tile([P, 1], F32, tag="vpe", name="vpe")
                # vpe = sumsq/D - mu2 + eps  (two tiny fused scalar ops)
                nc.vector.tensor_scalar(out=vpe[:], in0=sumsq[:],
                                        scalar1=1.0 / D, scalar2=mu2[:],
                                        op0=ALU.mult, op1=ALU.subtract)
                std = sc.tile([P, 1], F32, tag="std", name="std")
                nc.scalar.activation(std[:], vpe[:], AF.Sqrt, bias=epsc[:])
                inv = sc.tile([P, 1], F32, tag="inv", name="inv")
                nc.vector.reciprocal(inv[:], std[:])
                nc.vector.tensor_scalar(out=dst_tiles[mt][:], in0=xt[:],
                                        scalar1=mus[:], scalar2=inv[:],
                                        op0=ALU.subtract, op1=ALU.mult)

        def transpose_apply(src_tiles, dst_tiles, g_sb, b_sb, tp):
            for k in range(KD):
                for mt in range(NT):
                    ps = tp.tile([P, P], F32, tag="tps", name="tps")
                    nc.tensor.transpose(ps[:], src_tiles[mt][:, k * P:(k + 1) * P],
                                        ident[:])
                    nc.vector.tensor_scalar(
                        out=dst_tiles[k][:, mt * P:(mt + 1) * P], in0=ps[:],
                        scalar1=g_sb[:, k:k + 1], scalar2=b_sb[:, k:k + 1],
                        op0=ALU.mult, op1=ALU.add)

        # ============ phase A: LN1 + transpose + AllGather ============
        if True:
            with tc.tile_pool(name="lnA", bufs=1) as sc, \
                 tc.tile_pool(name="tpA", bufs=4, space="PSUM") as tp:
                h = [sc.tile([P, D], F32, tag=f"h{mt}", name=f"h{mt}") for mt in range(NT)]
                layernorm(xs, h, sc)
                hT = [sc.tile([P, TS], F32R, tag=f"hT{k}", name=f"hT{k}") for k in range(KD)]
                transpose_apply(h, hT, ln1g, ln1b, tp)
                for k in range(KD):
                    nc.sync.dma_start(t["ag1_in"][k * P:(k + 1) * P, :], hT[k][:])

            # ============ phases B+C: QKV + attention ============
            with tc.tile_pool(name="qkvP", bufs=1) as qkv:
                qT = [qkv.tile([P, S], F32R, tag=f"qT{m}", name=f"qT{m}") for m in range(2)]
                kT = [qkv.tile([P, S], F32R, tag=f"kT{m}", name=f"kT{m}") for m in range(2)]
                vo = [qkv.tile([P, HC, DH + 1], F32R, tag=f"vo{tm}", name=f"vo{tm}")
                      for tm in range(S // P)]

                wp_cm = tc.tile_pool(name="wqkv", bufs=1)
                wp = wp_cm.__enter__()
                # weight loads traced before the collective: DMA covers AG1
                wq = [wp.tile([P, CC], F32R, tag=f"wq{k}", name=f"wq{k}") for k in range(KD)]
                wk = [wp.tile([P, CC], F32R, tag=f"wk{k}", name=f"wk{k}") for k in range(KD)]
                wv = [wp.tile([P, CC], F32R, tag=f"wv{k}", name=f"wv{k}") for k in range(KD)]
                for k in range(KD):
                    nc.sync.dma_start(wq[k][:], t["wq_d"][k * P:(k + 1) * P, :])
                    nc.sync.dma_start(wk[k][:], t["wk_d"][k * P:(k + 1) * P, :])
                    nc.sync.dma_start(wv[k][:], t["wv_d"][k * P:(k + 1) * P, :])

                nc.gpsimd.collective_compute(
                    "AllGather", ALU.bypass, ins=[t["ag1_in"][:]],
                    outs=[t["ag1_out"][:]], replica_groups=GROUPS,
                )

                with tc.tile_pool(name="hTst", bufs=2) as st, \
                     tc.tile_pool(name="projPS", bufs=4, space="PSUM") as pps:
                    for qc in range(G):
                        hTq = [st.tile([P, TS], F32R, tag=f"hTq{k}", name=f"hTq{k}")
                               for k in range(KD)]
                        for k in range(KD):
                            nc.sync.dma_start(
                                hTq[k][:], t["ag1_out"][qc, k * P:(k + 1) * P, :])
                        for (w_sb, b_sb, out_sb) in ((wq, bqp, qT), (wk, bkp, kT)):
                            for m in range(2):
                                ps = pps.tile([P, TS], F32, tag="pps", name="pps")
                                for k in range(KD):
                                    nc.tensor.matmul(
                                        ps[:], w_sb[k][:, m * P:(m + 1) * P],
                                        hTq[k][:], start=(k == 0),
                                        stop=(k == KD - 1))
                                nc.vector.tensor_scalar(
                                    out=out_sb[m][:, qc * TS:(qc + 1) * TS],
                                    in0=ps[:], scalar1=b_sb[:, m:m + 1],
                                    scalar2=None, op0=ALU.add)
                        for mt in range(NT):
                            tm = qc * NT + mt
                            ps = pps.tile([P, CC], F32, tag="vps", name="vps")
                            for k in range(KD):
                                nc.tensor.matmul(
                                    ps[:], hTq[k][:, mt * P:(mt + 1) * P],
                                    wv[k][:], start=(k == 0), stop=(k == KD - 1))
                            nc.vector.tensor_tensor(
                                out=vo[tm][:, :, 0:DH],
                                in0=ps[:].rearrange("p (h e) -> p h e", h=HC),
                                in1=bvb[:].rearrange("p (h e) -> p h e", h=HC),
                                op=ALU.add)
                            nc.vector.tensor_copy(vo[tm][:, :, DH:DH + 1],
                                                  onescol4[:])

                wp_cm.__exit__(None, None, None)

                # ---- attention ----
                with (
                    tc.tile_pool(name="scPS", bufs=2, space="PSUM") as scp,
                    tc.tile_pool(name="avPS", bufs=2, space="PSUM") as avp,
                    tc.tile_pool(name="attnSB", bufs=3) as asb,
                ):
                    for hp in range(HC // 2):      # head pairs at PE rows 0/64
                        for qc in range(G):
                            kt_max = 4 * qc + 3
                            avs = [avp.tile([DH + 1, TS], F32, tag=f"av{j}",
                                            name=f"av{j}") for j in range(2)]
                            for kt in range(kt_max + 1):
                                # diag blocks: only columns >= v0 are ever read
                                w0 = P * max(0, kt - 4 * qc)
                                # both heads' score blocks into one 2-bank tile
                                sc_ps = scp.tile([P, 2, TS], F32,
                                                 tag="scp", name="scp")
                                for j in range(2):
                                    h_i = 2 * hp + j
                                    m = h_i // 2
                                    o = (h_i % 2) * DH
                                    nc.tensor.matmul(
                                        sc_ps[:, j, w0:],
                                        kT[m][o:o + DH, kt * P:(kt + 1) * P],
                                        qT[m][o:o + DH,
                                              qc * TS + w0:(qc + 1) * TS],
                                        start=True, stop=True)
                                e_r = asb.tile([P, 2, TS], F32R,
                                               tag="erp", name="erp")
                                if kt < 4 * qc:
                                    v0 = 0      # valid columns start
                                    nc.scalar.activation(
                                        e_r[:].rearrange("p a b -> p (a b)"),
                                        sc_ps[:].rearrange("p a b -> p (a b)"),
                                        AF.Exp, scale=0.125)
                                else:
                                    # diag block, shift s=-128*d: cols < 128*d
                                    # are fully masked -- never compute/read them
                                    d = kt - 4 * qc
                                    v0 = P * d
                                    e_f = asb.tile([P, 2, TS], F32,
                                                   tag="efp", name="efp")
                                    nc.scalar.activation(
                                        e_f[:, :, v0:], sc_ps[:, :, v0:],
                                        AF.Exp, scale=0.125)
                                    sh = 512 * qc - 128 * kt
                                    mdv = maskd[sh][:].rearrange(
                                        "p (a b) -> p a b", a=2)
                                    nc.vector.tensor_tensor(
                                        out=e_r[:, :, v0:],
                                        in0=e_f[:, :, v0:],
                                        in1=mdv[:, :, v0:],
                                        op=ALU.mult)
                                for j in range(2):
                                    h_i = 2 * hp + j
                                    nc.tensor.matmul(avs[j][:, v0:],
                                                     vo[kt][:, h_i, :],
                                                     e_r[:, j, v0:],
                                                     start=(kt == 0),
                                                     stop=(kt == kt_max))
                            for j in range(2):
                                h_i = 2 * hp + j
                                un = asb.tile([DH + 1, TS], F32,
                                              tag=f"un{j}", name=f"un{j}")
                                nc.vector.tensor_copy(un[:], avs[j][:])
                                rec = asb.tile([1, TS], F32,
                                               tag=f"rec{j}", name=f"rec{j}")
                                nc.vector.reciprocal(rec[:], un[DH:DH + 1, :])
                                rb = asb.tile([DH, TS], F32,
                                              tag=f"rb{j}", name=f"rb{j}")
                                nc.gpsimd.partition_broadcast(rb[:], rec[:])
                                chunk = asb.tile([DH, TS], F32R,
                                                 tag=f"chunk{j}", name=f"chunk{j}")
                                nc.vector.tensor_tensor(out=chunk[:],
                                                        in0=un[0:DH, :],
                                                        in1=rb[:], op=ALU.mult)
                                nc.sync.dma_start(
                                    t["a2a_in"][h_i * DH:(h_i + 1) * DH,
                                                qc * TS:(qc + 1) * TS],
                                    chunk[:])

            # prefetch pools traced before AG2 so DMA covers the collective
            pfw1_cm = tc.tile_pool(name="pfW1", bufs=1)
            pfw1 = pfw1_cm.__enter__()
            w1s0 = [pfw1.tile([P, 8 * P], F32R, tag=f"w1s{k}", name=f"w1s{k}")
                    for k in range(KD)]
            pfd_cm = tc.tile_pool(name="pfD", bufs=1)
            pfd = pfd_cm.__enter__()
            wo = [pfd.tile([P, D], F32R, tag=f"wo{k}", name=f"wo{k}") for k in range(KD)]
            for k in range(KD):
                nc.sync.dma_start(wo[k][:], t["wo_d"][k * P:(k + 1) * P, :])
                nc.sync.dma_start(w1s0[k][:], t["w1_d"][k * P:(k + 1) * P, 0:8 * P])

            nc.gpsimd.collective_compute(
                "AllGather", ALU.bypass, ins=[t["a2a_in"][:]],
                outs=[t["a2a_out"][:]], replica_groups=GROUPS,
            )

            # ============ phase D: out_proj + residual (in-place on xs) ====
            with tc.tile_pool(name="opPS", bufs=4, space="PSUM") as opp, \
                 tc.tile_pool(name="opSB", bufs=1) as osb:
                aT = [osb.tile([P, TS], F32R, tag=f"aT{k}", name=f"aT{k}") for k in range(KD)]
                off_sb = osb.tile([1, 1], mybir.dt.int32, tag="off", name="off")
                nc.sync.dma_start(off_sb[:], t["coff_d"][:])
                with nc.gpsimd.register("roff") as roff:
                    nc.gpsimd.reg_load(roff, off_sb[0:1, 0:1])
                    rv = nc.snap(roff)
                    for k in range(KD):
                        nc.gpsimd.dma_start(
                            aT[k][:],
                            t["a2a_out"][k // 2, (k % 2) * P:(k % 2 + 1) * P,
                                         bass.ds(rv, TS)])
                for mt in range(NT):
                    for n in range(2):
                        ps = opp.tile([P, TS], F32, tag="op", name="op")
                        nc.tensor.matmul(ps[:], ones128[:],
                                         bo[:, n * TS:(n + 1) * TS],
                                         start=True, stop=False)
                        for k in range(KD):
                            nc.tensor.matmul(
                                ps[:], aT[k][:, mt * P:(mt + 1) * P],
                                wo[k][:, n * TS:(n + 1) * TS],
                                start=False, stop=(k == KD - 1))
                        # residual written in place: xs becomes x2
                        nc.vector.tensor_tensor(
                            out=xs[mt][:, n * TS:(n + 1) * TS], in0=ps[:],
                            in1=xs[mt][:, n * TS:(n + 1) * TS], op=ALU.add)
            pfd_cm.__exit__(None, None, None)
            x2 = xs

            with tc.tile_pool(name="h2TP", bufs=1) as h2tp:
                h2T = [h2tp.tile([P, TS], F32R, tag=f"h2T{k}", name=f"h2T{k}")
                       for k in range(KD)]
                with tc.tile_pool(name="lnD", bufs=1) as sc, \
                     tc.tile_pool(name="tpD", bufs=4, space="PSUM") as tp:
                    h2 = [sc.tile([P, D], F32, tag=f"h2{mt}", name=f"h2{mt}")
                          for mt in range(NT)]
                    layernorm(x2, h2, sc)
                    transpose_apply(h2, h2T, ln2g, ln2b, tp)

                # ============ phase E: FFN ============
                with tc.tile_pool(name="gTP", bufs=1) as gtp:
                    gT = [gtp.tile([P, TS], F32R, tag=f"gT{mf}", name=f"gT{mf}")
                          for mf in range(KF)]
                    MFB = 4     # mf tiles per w1 stream block
                    with tc.tile_pool(name="w1st", bufs=1) as w1p, \
                         tc.tile_pool(name="gPS", bufs=4, space="PSUM") as gps:
                        for blk in range(KF // MFB):
                            if blk < 2:
                                # prefetched during AG2 (w1s0 holds blocks 0-1)
                                w1s = [w1s0[k][:, blk * MFB * P:(blk + 1) * MFB * P]
                                       for k in range(KD)]
                            else:
                                w1t = [w1p.tile([P, MFB * P], F32R,
                                                tag=f"w1b{k}", name=f"w1b{k}",
                                                bufs=2)
                                       for k in range(KD)]
                                for k in range(KD):
                                    nc.sync.dma_start(
                                        w1t[k][:],
                                        t["w1_d"][k * P:(k + 1) * P,
                                                  blk * MFB * P:(blk + 1) * MFB * P])
                                w1s = [w1t[k][:] for k in range(KD)]
                            for j in range(MFB):
                                mf = blk * MFB + j
                                ps = gps.tile([P, TS], F32, tag="g", name="g")
                                for k in range(KD):
                                    nc.tensor.matmul(
                                        ps[:], w1s[k][:, j * P:(j + 1) * P],
                                        h2T[k][:], start=(k == 0),
                                        stop=(k == KD - 1))
                                nc.scalar.activation(gT[mf][:], ps[:],
                                                     AF.Gelu,
                                                     bias=b1p[:, mf:mf + 1])

                    with tc.tile_pool(name="w2st", bufs=4) as w2p, \
                         tc.tile_pool(name="fPS", bufs=1, space="PSUM") as fps, \
                         tc.tile_pool(name="ySB", bufs=2) as ysb:
                        f_ps = [fps.tile([P, D], F32, tag=f"f{mt}", name=f"f{mt}")
                                for mt in range(NT)]
                        for mt in range(NT):
                            for n in range(2):
                                nc.tensor.matmul(
                                    f_ps[mt][:, n * TS:(n + 1) * TS],
                                    ones128[:], b2[:, n * TS:(n + 1) * TS],
                                    start=True, stop=False)
                        for k2 in range(KF):
                            w2t = w2p.tile([P, D], F32R, tag="w2", name="w2")
                            nc.sync.dma_start(
                                w2t[:], t["w2_d"][k2 * P:(k2 + 1) * P, :])
                            for mt in range(NT):
                                for n in range(2):
                                    nc.tensor.matmul(
                                        f_ps[mt][:, n * TS:(n + 1) * TS],
                                        gT[k2][:, mt * P:(mt + 1) * P],
                                        w2t[:, n * TS:(n + 1) * TS],
                                        start=False, stop=(k2 == KF - 1))
                        for mt in range(NT):
                            yt = ysb.tile([P, D], F32, tag="y", name="y")
                            nc.vector.tensor_tensor(out=yt[:],
                                                    in0=f_ps[mt][:],
                                                    in1=x2[mt][:],
                                                    op=ALU.add)
                            nc.sync.dma_start(
                                t["y_d"][mt * P:(mt + 1) * P, :], yt[:])
            pfw1_cm.__exit__(None, None, None)
            xsp_cm.__exit__(None, None, None)


def _in_maps(inputs):
    f32 = np.float32
    maps = []
    for c in range(NC):
        b, r = c // G, c % G
        c0 = r * CC
        m = {
            "x": np.ascontiguousarray(np.asarray(inputs["x"])[b, r * TS:(r + 1) * TS, :], f32),
            "ln1_g": np.ascontiguousarray(inputs["ln1_g"], f32),
            "ln1_b": np.ascontiguousarray(inputs["ln1_b"], f32),
            "Wq": np.ascontiguousarray(np.asarray(inputs["Wq"])[:, c0:c0 + CC], f32),
            "Wk": np.ascontiguousarray(np.asarray(inputs["Wk"])[:, c0:c0 + CC], f32),
            "Wv": np.ascontiguousarray(np.asarray(inputs["Wv"])[:, c0:c0 + CC], f32),
            "bq": np.ascontiguousarray(np.asarray(inputs["bq"])[c0:c0 + CC], f32),
            "bk": np.ascontiguousarray(np.asarray(inputs["bk"])[c0:c0 + CC], f32),
            "bv": np.ascontiguousarray(np.asarray(inputs["bv"])[c0:c0 + CC], f32),
            "Wo": np.ascontiguousarray(inputs["Wo"], f32),
            "bo": np.ascontiguousarray(inputs["bo"], f32),
            "ln2_g": np.ascontiguousarray(inputs["ln2_g"], f32),
            "ln2_b": np.ascontiguousarray(inputs["ln2_b"], f32),
            "W1": np.ascontiguousarray(inputs["W1"], f32),
            "b1": np.ascontiguousarray(inputs["b1"], f32),
            "W2": np.ascontiguousarray(inputs["W2"], f32),
            "b2": np.ascontiguousarray(inputs["b2"], f32),
            "coff": np.array([[r * TS]], dtype=np.int32),
        }
        maps.append(m)
    return maps


def _run(inputs, trace=False):
    if "nc" not in _CACHE:
        _CACHE["nc"] = build()
    nc = _CACHE["nc"]
    maps = _in_maps(inputs)
    res = run_bass_kernel_spmd(nc, maps, list(range(NC)), trace=trace)
    out = np.empty((B, S, D), np.float32)
    for c in range(NC):
        b, r = c // G, c % G
        out[b, r * TS:(r + 1) * TS, :] = res.results[c]["y"]
    return out, res


def kernel(**inputs):
    out, _ = _run(inputs, trace=False)
    return out


if __name__ == "__main__":
    build()
    print("build OK")
